# revision 22
# baseline (speedup 1.0000x reference)
"""Trainium2 Bass kernel for nn_BoundaryControlledMixer (4-layer Mamba stack +
boundary-controlled gate), tensor-parallel over d_inner across 8 NeuronCores.

Per core (owns E_loc = 192 of E = 1536 channels, full batch/sequence):
  - Activations flow feature-major [feat, token] so projections chain on the
    PE without transposes (matmul contracts the partition dim).
  - Selective scan: chunked form, chunk Q = 128 tokens (token-major).  With
    A[e,n] = A_n = -exp(A_log[n]) (channel-independent), for tokens in chunk:
        S'_t[e] = in-chunk centered cumsum of dt  ( (TRI - 1/2) @ dt , PE )
        P  = exp(+A_n S') = r^n,  E1 = exp(-A_n S') = rinv^n   (DVE doubling)
        qv[tau,(n,e)] = u[tau,e] B[tau,n] E1[tau,n,e]          (DVE)
        qv[0] += state_row                                     (chunk carry)
        cumQ = TRI @ qv                                        (PE, fp32 PSUM)
        y    = sum_n C[t,n] * P[t,n,e] * cumQ[t,n,e]           (DVE tree)
    Centering keeps |A_n S'| <= |A_n|*chunk_dtsum/2 < 60: no overflow.
    Carried state row = hc[last_token] (= P_end*cumQ_end) scaled by
    exp(A_n * m) for the next chunk (m = chunk midpoint value).
"""

import numpy as np

import concourse.bacc as bacc
import concourse.bass as bass
import concourse.mybir as mybir
import concourse.tile as tile
from concourse import masks
from concourse.bass_utils import run_bass_kernel_spmd

FP32 = mybir.dt.float32
BF16 = mybir.dt.bfloat16
AF = mybir.ActivationFunctionType
OP = mybir.AluOpType
AX = mybir.AxisListType

B, L, DM, NL = 2, 1024, 768, 4
E, N, K, R = 2 * DM, 16, 4, DM // 16
NC = 8
ELOC = E // NC            # 192
T = B * L                 # 2048
Q = 128
NCH = T // Q              # 16
CPB = L // Q              # 8
NH = N // 2               # 8
HW = NH * ELOC            # 1536
EPS = 1e-5
DTILES = DM // 128        # 6
GDM = DM // NC            # 96
LPD = L + 2 * K           # padded per-batch xp row

_CACHE = {}
_DEBUG = False


def _etiles():
    return [(0, 128), (128, 64)]


def _build(a_scales):
    nc = bacc.Bacc("TRN2", target_bir_lowering=False, debug=False)

    x_d = nc.dram_tensor("x", [T, DM], FP32, kind="ExternalInput")
    bprob_d = nc.dram_tensor("bprob", [1, T], BF16, kind="ExternalInput")
    w_in_d = nc.dram_tensor("w_in", [NL, 128, 6 * 2 * ELOC], BF16, kind="ExternalInput")
    conv_w_d = nc.dram_tensor("conv_w", [NL, 128, 2 * K], FP32, kind="ExternalInput")
    conv_b_d = nc.dram_tensor("conv_b", [NL, 128, 2], FP32, kind="ExternalInput")
    w_xp_d = nc.dram_tensor("w_xp", [NL, 128, 2 * (R + 2 * N)], BF16, kind="ExternalInput")
    w_dt_d = nc.dram_tensor("w_dt", [NL, R + 1, ELOC], FP32, kind="ExternalInput")
    w_out_d = nc.dram_tensor("w_out", [NL, 128, 2 * DM], BF16, kind="ExternalInput")
    ln_d = nc.dram_tensor("lnp", [NL, 128, 12], FP32, kind="ExternalInput")
    ssmd_d = nc.dram_tensor("ssmd", [NL, 128, 2], FP32, kind="ExternalInput")
    w_c1_d = nc.dram_tensor("w_c1", [128, 7 * GDM], BF16, kind="ExternalInput")
    b_c1_d = nc.dram_tensor("b_c1", [GDM, 1], FP32, kind="ExternalInput")
    w_c2_d = nc.dram_tensor("w_c2", [GDM + 1, DM], BF16, kind="ExternalInput")
    nrm_d = nc.dram_tensor("nrm", [4, DM], FP32, kind="ExternalInput")
    nrmc_d = nc.dram_tensor("nrmc", [128, 12], FP32, kind="ExternalInput")
    tri16_d = nc.dram_tensor("tri16", [128, 128], BF16, kind="ExternalInput")
    tricf_d = nc.dram_tensor("tricf", [128, 129], FP32, kind="ExternalInput")

    out_d = nc.dram_tensor("out", [T, DM], FP32, kind="ExternalOutput")
    gate_d = nc.dram_tensor("gate", [T, DM], FP32, kind="ExternalOutput")
    dbg = {}
    if _DEBUG:
        dbg["dt"] = nc.dram_tensor("dbg_dt", [T, ELOC], FP32, kind="ExternalOutput")
        dbg["sp"] = nc.dram_tensor("dbg_sp", [T, ELOC], FP32, kind="ExternalOutput")
        dbg["yssm"] = nc.dram_tensor("dbg_yssm", [T, ELOC], FP32, kind="ExternalOutput")
        dbg["hs"] = nc.dram_tensor("dbg_hs", [DM, T], FP32, kind="ExternalOutput")
        dbg["xc"] = nc.dram_tensor("dbg_xc", [ELOC, T], FP32, kind="ExternalOutput")
        dbg["hln"] = nc.dram_tensor("dbg_hln", [DM, T], FP32, kind="ExternalOutput")

    with tile.TileContext(nc) as tc:
        with tc.tile_pool(name="const", bufs=1) as constp, \
             tc.tile_pool(name="persist", bufs=1) as pers, \
             tc.tile_pool(name="wts", bufs=1) as wpool, \
             tc.tile_pool(name="act", bufs=1) as actp, \
             tc.tile_pool(name="st2", bufs=1) as st2, \
             tc.tile_pool(name="vol", bufs=2) as volp, \
             tc.tile_pool(name="rows", bufs=1) as rowp, \
             tc.tile_pool(name="ps_tri", bufs=2, space="PSUM") as ps_tri, \
             tc.tile_pool(name="ps_mm", bufs=2, space="PSUM") as ps_mm, \
             tc.tile_pool(name="dram", bufs=2, space="DRAM") as dramp:

            def pmm(shape, dt=FP32):
                return ps_mm.tile(shape, dt, name="pmm", tag="pmm")

            # ---------- constants ----------
            ident32 = constp.tile([128, 128], FP32)
            masks.make_identity(nc, ident32[:])
            ident16 = constp.tile([128, 128], BF16)
            masks.make_identity(nc, ident16[:])
            tri16 = constp.tile([128, 128], BF16)
            nc.sync.dma_start(tri16[:], tri16_d[:])
            tricf = constp.tile([128, 129], FP32)
            nc.sync.dma_start(tricf[:], tricf_d[:])
            tric32 = tricf[:, 0:128]
            halfcol32 = tricf[:, 128:129]
            onesrow16 = constp.tile([1, 512], BF16)
            nc.gpsimd.memset(onesrow16[:], 1.0)
            eps_ap = constp.tile([128, 1], FP32)
            nc.gpsimd.memset(eps_ap[:], EPS)
            nrow = constp.tile([97, DM], FP32)
            for _i in range(4):
                nc.sync.dma_start(nrow[_i * 32:_i * 32 + 1, :], nrm_d[_i:_i + 1, :])
            nrmc = constp.tile([128, 12], FP32)
            nc.sync.dma_start(nrmc[:], nrmc_d[:])

            # ---------- x -> feature-major fp32 residual ----------
            residual = [pers.tile([128, T], FP32, name=f"res{j}") for j in range(DTILES)]
            for c in range(NCH):
                x_tm_c = st2.tile([128, DM], FP32, name="x_tm_c", tag="x_tm_c")
                nc.sync.dma_start(x_tm_c[:], x_d[c * Q:(c + 1) * Q, :])
                for j in range(DTILES):
                    pt = pmm([128, 128])
                    nc.tensor.transpose(pt[:], x_tm_c[:, j * 128:(j + 1) * 128], ident32[:])
                    nc.scalar.copy(residual[j][:, c * Q:(c + 1) * Q], pt[:])

            # ---------- fused feature-major LayerNorm ----------
            def ln_fm(lnw_aps, lnb_aps, consume, dbg_dst=None):
                stats16 = actp.tile([1, T], BF16, name="stats16", tag="stats16")
                stats16b = actp.tile([1, T], BF16, name="stats16b", tag="stats16b")

                for f in range(T // 512):
                    fs = slice(f * 512, (f + 1) * 512)
                    sp1 = pmm([1, 512])
                    sp2 = pmm([1, 512])
                    for j in range(DTILES):
                        nc.tensor.matmul(sp1[:], halfcol32, residual[j][:, fs],
                                         start=(j == 0), stop=(j == DTILES - 1))
                    nc.scalar.activation(stats16[0:1, fs], sp1[:], AF.Copy, scale=2.0 / DM)
                    for j in range(DTILES):
                        sqj = st2.tile([128, 512], FP32, name="ln_sqj", tag="ln_sqj")
                        nc.vector.tensor_tensor(sqj[:], residual[j][:, fs],
                                                residual[j][:, fs], OP.mult)
                        nc.tensor.matmul(sp2[:], halfcol32, sqj[:],
                                         start=(j == 0), stop=(j == DTILES - 1))
                    nc.scalar.activation(stats16b[0:1, fs], sp2[:], AF.Copy, scale=2.0 / DM)

                for f in range(T // 512):
                    fs = slice(f * 512, (f + 1) * 512)
                    rp = pmm([128, 512])
                    nc.tensor.matmul(rp[:], onesrow16[:1, :128], stats16[0:1, fs],
                                     start=True, stop=True)
                    meanr = st2.tile([128, 512], FP32, name="ln_meanr", tag="ln_meanr")
                    nc.scalar.copy(meanr[:], rp[:])
                    rp2 = pmm([128, 512])
                    nc.tensor.matmul(rp2[:], onesrow16[:1, :128], stats16b[0:1, fs],
                                     start=True, stop=True)
                    invr = st2.tile([128, 512], FP32, name="ln_invr", tag="ln_invr")
                    nc.scalar.copy(invr[:], rp2[:])
                    # var = E[x^2] - mean^2 ; inv = exp(-0.5*ln(var+eps))
                    c2r = st2.tile([128, 512], FP32, name="ln_c2r", tag="ln_c2r")
                    nc.vector.tensor_tensor(c2r[:], meanr[:], meanr[:], OP.mult)
                    nc.vector.tensor_tensor(invr[:], invr[:], c2r[:], OP.subtract)
                    nc.scalar.activation(invr[:], invr[:], AF.Ln, bias=eps_ap[:])
                    nc.scalar.activation(invr[:], invr[:], AF.Exp, scale=-0.5)
                    nc.vector.tensor_tensor(c2r[:], meanr[:], invr[:], OP.mult)
                    slices = []
                    for j in range(DTILES):
                        tmp = st2.tile([128, 512], BF16, name="ln_tmp", tag="ln_tmp", bufs=2)
                        nc.vector.tensor_tensor(tmp[:], residual[j][:, fs], invr[:], OP.mult)
                        nc.vector.tensor_tensor(tmp[:], tmp[:], c2r[:], OP.subtract)
                        hlnf = st2.tile([128, 512], BF16, name="hlnf", tag=f"hlnf{j}")
                        nc.scalar.activation(hlnf[:], tmp[:], AF.Identity,
                                             scale=lnw_aps[j], bias=lnb_aps[j])
                        slices.append(hlnf)
                        if dbg_dst is not None:
                            hld = st2.tile([128, 512], FP32, name="hld", tag="hld")
                            nc.vector.tensor_copy(hld[:], hlnf[:])
                            nc.sync.dma_start(dbg_dst[j * 128:(j + 1) * 128, fs], hld[:])
                    consume(f, slices)

            # ================= layers =================
            for li in range(NL):
                w_in = wpool.tile([128, 6 * 2 * ELOC], BF16, name="w_in_sb", tag="w_in_sb")
                nc.sync.dma_start(w_in[:], w_in_d[li])
                w_cw = wpool.tile([128, 2 * K], FP32, name="w_cw_sb", tag="w_cw_sb")
                nc.sync.dma_start(w_cw[:], conv_w_d[li])
                w_cb = wpool.tile([128, 2], FP32, name="w_cb_sb", tag="w_cb_sb")
                nc.sync.dma_start(w_cb[:], conv_b_d[li])
                w_xp = wpool.tile([128, 2 * (R + 2 * N)], BF16, name="w_xp_sb", tag="w_xp_sb")
                nc.sync.dma_start(w_xp[:], w_xp_d[li])
                w_dt = wpool.tile([R + 1, ELOC], FP32, name="w_dt_sb", tag="w_dt_sb")
                nc.sync.dma_start(w_dt[:], w_dt_d[li])
                w_out = wpool.tile([128, 2 * DM], BF16, name="w_out_sb", tag="w_out_sb")
                nc.sync.dma_start(w_out[:], w_out_d[li])
                w_ln = wpool.tile([128, 12], FP32, name="w_ln_sb", tag="w_ln_sb")
                nc.sync.dma_start(w_ln[:], ln_d[li])
                w_D = wpool.tile([128, 2], FP32, name="w_D_sb", tag="w_D_sb")
                nc.sync.dma_start(w_D[:], ssmd_d[li])

                # ---- LN fused with in_proj ----
                xp_t = [actp.tile([128, B * LPD], BF16, name="xp_pad0", tag="xp_pad0"),
                        actp.tile([64, B * LPD], BF16, name="xp_pad1", tag="xp_pad1")]
                z_t = [actp.tile([128, T], BF16, name="z0", tag="z0"),
                       actp.tile([64, T], BF16, name="z1", tag="z1")]
                for ti in range(2):
                    nc.vector.memset(xp_t[ti][:, 0:K], 0.0)
                    nc.vector.memset(xp_t[ti][:, LPD:LPD + K], 0.0)

                def padcol(fs, fl):
                    b_ = fs // L
                    off = b_ * LPD + K + (fs - b_ * L)
                    return slice(off, off + fl)

                def consume_inproj(f, sl6):
                    fs = f * 512
                    for mt in range(3):
                        pt = pmm([128, 512])
                        for kt in range(DTILES):
                            nc.tensor.matmul(
                                pt[:], w_in[:, kt * 384 + mt * 128:kt * 384 + (mt + 1) * 128],
                                sl6[kt][:], start=(kt == 0), stop=(kt == DTILES - 1))
                        if mt == 0:
                            nc.scalar.copy(xp_t[0][:, padcol(fs, 512)], pt[:])
                        elif mt == 1:
                            nc.scalar.copy(xp_t[1][:, padcol(fs, 512)], pt[0:64, :])
                            nc.scalar.copy(z_t[0][0:64, fs:fs + 512], pt[64:128, :])
                        else:
                            nc.scalar.copy(z_t[0][64:128, fs:fs + 512], pt[0:64, :])
                            nc.scalar.copy(z_t[1][:, fs:fs + 512], pt[64:128, :])

                ln_fm([w_ln[:, 2 * j:2 * j + 1] for j in range(DTILES)],
                      [w_ln[:, 2 * j + 1:2 * j + 2] for j in range(DTILES)],
                      consume_inproj,
                      dbg_dst=dbg["hln"] if (_DEBUG and li == 0) else None)

                # ---- conv + silu ----
                xc = [actp.tile([128, T], BF16, name="xc0", tag="xc0"),
                      actp.tile([64, T], BF16, name="xc1", tag="xc1")]
                for ti, (eo, el) in enumerate(_etiles()):
                    for b_ in range(B):
                        acc = st2.tile([el, L], FP32, name="cacc", tag=f"cacc{ti}")
                        cb = b_ * LPD + K
                        nc.vector.tensor_scalar(acc[:], xp_t[ti][:el, cb - 3:cb - 3 + L],
                                                w_cw[0:el, ti * K:ti * K + 1], None, OP.mult)
                        for j in range(1, K):
                            nc.vector.scalar_tensor_tensor(
                                acc[:], xp_t[ti][:el, cb - 3 + j:cb - 3 + j + L],
                                w_cw[0:el, ti * K + j:ti * K + j + 1],
                                acc[:], OP.mult, OP.add)
                        nc.scalar.activation(xc[ti][:el, b_ * L:(b_ + 1) * L], acc[:],
                                             AF.Silu, bias=w_cb[0:el, ti:ti + 1])
                if _DEBUG and li == 0:
                    for ti, (eo, el) in enumerate(_etiles()):
                        xcd = st2.tile([el, T], FP32, name="xcd", tag="xcd")
                        nc.vector.tensor_copy(xcd[:], xc[ti][:el, :])
                        nc.sync.dma_start(dbg["xc"][eo:eo + el, :], xcd[:])

                # ---- x_proj partial + AllReduce ----
                dbl_in = dramp.tile([R + 2 * N, T], FP32, name="dbl_in", tag="dbl_in")
                dbl_out = dramp.tile([R + 2 * N, T], FP32, name="dbl_out", tag="dbl_out")
                for f in range(T // 512):
                    fs = slice(f * 512, (f + 1) * 512)
                    pt = pmm([80, 512])
                    for ti, (eo, el) in enumerate(_etiles()):
                        nc.tensor.matmul(pt[:], w_xp[0:el, ti * 80:(ti + 1) * 80],
                                         xc[ti][:el, fs], start=(ti == 0), stop=(ti == 1))
                    dblf = st2.tile([80, 512], FP32, name="dblf", tag="dblf")
                    nc.scalar.copy(dblf[:], pt[:])
                    nc.sync.dma_start(dbl_in[:, fs], dblf[:])
                nc.gpsimd.collective_compute("AllReduce", OP.add,
                                             replica_groups=[list(range(NC))],
                                             ins=[dbl_in[:]], outs=[dbl_out[:]])
                # B/C rows -> token-major
                bc_tm = actp.tile([128, NCH * 2 * N], BF16, name="bc_tm", tag="bc_tm")
                for c in range(NCH):
                    bcf = st2.tile([2 * N, Q], FP32, name="bcf", tag="bcf")
                    nc.sync.dma_start(bcf[:], dbl_out[R:R + 2 * N, c * Q:(c + 1) * Q])
                    bc6 = st2.tile([2 * N, Q], BF16, name="bc6", tag="bc6")
                    nc.vector.tensor_copy(bc6[:], bcf[:])
                    ptb = pmm([128, 32], BF16)
                    nc.tensor.transpose(ptb[:], bc6[:], ident16[:32, :32])
                    nc.scalar.copy(bc_tm[:, c * 32:(c + 1) * 32], ptb[:])

                # ---- per-chunk SSM ----
                y_fm = [actp.tile([128, T], BF16, name="yfm0", tag="yfm0"),
                        actp.tile([64, T], BF16, name="yfm1", tag="yfm1")]
                rowbuf = rowp.tile([1, 2 * N * ELOC], BF16, name="rowbuf", tag="rowbuf")
                cq_row = [rowbuf[0:1, h * HW:(h + 1) * HW] for h in range(2)]
                for c in range(NCH):
                    dtf = st2.tile([R + 1, Q], FP32, name="dtf", tag="dtf")
                    nc.vector.memset(dtf[0:1, :], 1.0)
                    nc.sync.dma_start(dtf[1:R + 1, :], dbl_out[0:R, c * Q:(c + 1) * Q])
                    ptd = pmm([128, ELOC])
                    nc.tensor.matmul(ptd[:], dtf[:], w_dt[:], start=True, stop=True)
                    dt_c = st2.tile([128, ELOC], FP32, name="dt_c", tag="dt_c")
                    nc.scalar.activation(dt_c[:], ptd[:], AF.Exp)
                    nc.scalar.activation(dt_c[:], dt_c[:], AF.Ln, bias=1.0)
                    if _DEBUG and li == 0:
                        nc.sync.dma_start(dbg["dt"][c * Q:(c + 1) * Q, :], dt_c[:])
                    pts = pmm([128, ELOC])
                    nc.tensor.matmul(pts[:], tric32, dt_c[:], start=True, stop=True)
                    sp_c = st2.tile([128, ELOC], FP32, name="sp_c", tag="sp_c")
                    nc.scalar.copy(sp_c[:], pts[:])
                    if _DEBUG and li == 0:
                        nc.sync.dma_start(dbg["sp"][c * Q:(c + 1) * Q, :], sp_c[:])
                    ptmm = pmm([1, ELOC])
                    nc.tensor.matmul(ptmm[:], halfcol32, dt_c[:], start=True, stop=True)
                    m_c = st2.tile([1, ELOC], FP32, name="m_c", tag="m_c")
                    nc.scalar.copy(m_c[:], ptmm[:])
                    exr = rowbuf[0:1, N * ELOC:2 * N * ELOC]
                    for n in range(N):
                        nc.scalar.activation(exr[:, n * ELOC:(n + 1) * ELOC], m_c[:],
                                             AF.Exp, scale=float(a_scales[n]))
                    xct = st2.tile([128, ELOC], BF16, name="xct", tag="xct")
                    nc.sync.dma_start_transpose(xct[:, 0:128], xc[0][:, c * Q:(c + 1) * Q])
                    ptx = pmm([128, 64], BF16)
                    nc.tensor.transpose(ptx[:], xc[1][:64, c * Q:(c + 1) * Q], ident16[:64, :64])
                    nc.scalar.copy(xct[:, 128:ELOC], ptx[:])
                    u_c = st2.tile([128, ELOC], BF16, name="u_c", tag="u_c")
                    nc.vector.tensor_tensor(u_c[:], dt_c[:], xct[:], OP.mult)
                    r_c = st2.tile([128, ELOC], BF16, name="r_c", tag="r_c")
                    nc.scalar.activation(r_c[:], sp_c[:], AF.Exp, scale=float(a_scales[0]))
                    ri_c = st2.tile([128, ELOC], BF16, name="ri_c", tag="ri_c")
                    nc.scalar.activation(ri_c[:], sp_c[:], AF.Exp, scale=float(-a_scales[0]))

                    def chain(base, tag):
                        t_ = volp.tile([128, N * ELOC], BF16, name="chn", tag=tag, bufs=1)
                        v = t_[:].rearrange("p (n e) -> p n e", n=N)
                        nc.vector.tensor_copy(v[:, 0, :], base[:])
                        nc.vector.tensor_tensor(v[:, 1, :], base[:], base[:], OP.mult)
                        for lo in (2, 4, 8):
                            nc.vector.tensor_tensor(
                                v[:, lo:2 * lo, :], v[:, 0:lo, :],
                                v[:, lo - 1:lo, :].broadcast_to([128, lo, ELOC]),
                                OP.mult)
                        return t_

                    P_c = chain(r_c, "P_c")
                    E_c = chain(ri_c, "E_c")

                    hcs = []
                    for h in range(2):
                        hsl = slice(h * HW, (h + 1) * HW)
                        qv = volp.tile([128, HW], BF16, name="qv", tag=f"qv{h}", bufs=1)
                        nc.vector.tensor_tensor(
                            qv[:].rearrange("p (n e) -> p n e", n=NH),
                            u_c[:].unsqueeze(1).broadcast_to([128, NH, ELOC]),
                            bc_tm[:, c * 32 + h * NH:c * 32 + (h + 1) * NH]
                            .unsqueeze(2).broadcast_to([128, NH, ELOC]),
                            OP.mult)
                        nc.vector.tensor_tensor(qv[:], qv[:], E_c[:, hsl], OP.mult)
                        if c % CPB != 0:
                            if h == 0:
                                nc.vector.tensor_tensor(rowbuf[0:1, 0:N * ELOC],
                                                        rowbuf[0:1, 0:N * ELOC],
                                                        exr[:], OP.mult)
                            nc.vector.tensor_tensor(qv[0:1, :], qv[0:1, :],
                                                    cq_row[h], OP.add)
                        tp = ps_tri.tile([128, HW], FP32, name="tp", tag="tri")
                        for fsub in range(HW // 512):
                            nc.tensor.matmul(tp[:, fsub * 512:(fsub + 1) * 512], tri16[:],
                                             qv[:, fsub * 512:(fsub + 1) * 512],
                                             start=True, stop=True)
                        hc = volp.tile([128, HW], BF16, name="hc", tag=f"qv{h}", bufs=1)
                        nc.vector.tensor_tensor(hc[:], tp[:], P_c[:, hsl], OP.mult)
                        nc.sync.dma_start(cq_row[h], hc[127:128, :])
                        nc.vector.tensor_tensor(
                            hc[:].rearrange("p (n e) -> p n e", n=NH),
                            hc[:].rearrange("p (n e) -> p n e", n=NH),
                            bc_tm[:, c * 32 + N + h * NH:c * 32 + N + (h + 1) * NH]
                            .unsqueeze(2).broadcast_to([128, NH, ELOC]),
                            OP.mult)
                        hcs.append(hc)
                    nc.vector.tensor_tensor(hcs[0][:], hcs[0][:], hcs[1][:], OP.add)
                    h3 = hcs[0][:].rearrange("p (n e) -> p n e", n=NH)
                    for lev in (4, 2, 1):
                        nc.vector.tensor_tensor(h3[:, 0:lev, :], h3[:, 0:lev, :],
                                                h3[:, lev:2 * lev, :], OP.add)
                    y_c = st2.tile([128, ELOC], BF16, name="y_c", tag="y_c", bufs=3)
                    nc.vector.tensor_copy(y_c[:], h3[:, 0, :])
                    if _DEBUG and li == 0:
                        ydd = st2.tile([128, ELOC], FP32, name="ydd", tag="ydd")
                        nc.vector.tensor_copy(ydd[:], y_c[:])
                        nc.sync.dma_start(dbg["yssm"][c * Q:(c + 1) * Q, :], ydd[:])
                    nc.sync.dma_start_transpose(y_fm[0][:, c * Q:(c + 1) * Q], y_c[:, 0:128])
                    pty = pmm([64, 128], BF16)
                    nc.tensor.transpose(pty[:], y_c[:, 128:ELOC], ident16[:])
                    nc.scalar.copy(y_fm[1][:64, c * Q:(c + 1) * Q], pty[:])

                # ---- D-term, z-gate ----
                for ti, (eo, el) in enumerate(_etiles()):
                    nc.vector.scalar_tensor_tensor(y_fm[ti][:el, :], xc[ti][:el, :],
                                                   w_D[0:el, ti:ti + 1], y_fm[ti][:el, :],
                                                   OP.mult, OP.add)
                    nc.scalar.activation(z_t[ti][:el, :], z_t[ti][:el, :], AF.Silu)
                    nc.vector.tensor_tensor(y_fm[ti][:el, :], y_fm[ti][:el, :],
                                            z_t[ti][:el, :], OP.mult)

                # ---- out_proj partial + AllReduce + residual update ----
                op_in = dramp.tile([DM, T], FP32, name="op_in", tag="op_in")
                op_out = dramp.tile([DM, T], FP32, name="op_out", tag="op_out")
                for mt in range(DTILES):
                    for f in range(T // 512):
                        fs = slice(f * 512, (f + 1) * 512)
                        pt = pmm([128, 512])
                        for ti, (eo, el) in enumerate(_etiles()):
                            nc.tensor.matmul(
                                pt[:], w_out[0:el, ti * DM + mt * 128:ti * DM + (mt + 1) * 128],
                                y_fm[ti][:el, fs], start=(ti == 0), stop=(ti == 1))
                        opf = st2.tile([128, 512], FP32, name="opf", tag="opf")
                        nc.scalar.copy(opf[:], pt[:])
                        nc.sync.dma_start(op_in[mt * 128:(mt + 1) * 128, fs], opf[:])
                nc.gpsimd.collective_compute("AllReduce", OP.add,
                                             replica_groups=[list(range(NC))],
                                             ins=[op_in[:]], outs=[op_out[:]])
                for j in range(DTILES):
                    for f in range(T // 512):
                        fs = slice(f * 512, (f + 1) * 512)
                        hs_f = st2.tile([128, 512], FP32, name="hs_f", tag="hs_f")
                        nc.sync.dma_start(hs_f[:], op_out[j * 128:(j + 1) * 128, fs])
                        nc.vector.tensor_tensor(residual[j][:, fs], residual[j][:, fs],
                                                hs_f[:], OP.add)
                        if _DEBUG and li == 0:
                            nc.sync.dma_start(dbg["hs"][j * 128:(j + 1) * 128, fs], hs_f[:])

            # ================= final stage =================
            mixed = [actp.tile([128, T], BF16, name=f"mx{j}", tag=t)
                     for j, t in enumerate(["xp_pad0", "z0", "xc0", "yfm0", "mxa", "mxb"])]

            def consume_mixed(f, sl6):
                fs = slice(f * 512, (f + 1) * 512)
                for j in range(DTILES):
                    nc.vector.tensor_copy(mixed[j][:, fs], sl6[j][:])

            ln_fm([nrmc[:, 2 * j:2 * j + 1] for j in range(DTILES)],
                  [nrmc[:, 2 * j + 1:2 * j + 2] for j in range(DTILES)],
                  consume_mixed)

            xfm16 = [actp.tile([128, T], BF16, name=f"xfm{j}", tag=t)
                     for j, t in enumerate(["xp_pad1", "z1", "xc1", "yfm1", "xfa", "xfb"])]
            for c in range(NCH):
                x_tm_c = st2.tile([128, DM], FP32, name="x_tm_c2", tag="x_tm_c")
                nc.sync.dma_start(x_tm_c[:], x_d[c * Q:(c + 1) * Q, :])
                for j in range(DTILES):
                    ptt = pmm([128, 128])
                    nc.tensor.transpose(ptt[:], x_tm_c[:, j * 128:(j + 1) * 128], ident32[:])
                    nc.scalar.copy(xfm16[j][:, c * Q:(c + 1) * Q], ptt[:])
            brow = actp.tile([1, T], BF16, name="brow", tag="stats16")
            nc.sync.dma_start(brow[:], bprob_d[:])

            wc1 = wpool.tile([128, 7 * GDM], BF16, name="wc1", tag="w_in_sb")
            nc.sync.dma_start(wc1[:], w_c1_d[:])
            bc1 = wpool.tile([GDM, 1], FP32, name="bc1", tag="w_cb_sb")
            nc.sync.dma_start(bc1[:], b_c1_d[:])
            wc2 = wpool.tile([GDM + 1, DM], BF16, name="wc2", tag="w_out_sb")
            nc.sync.dma_start(wc2[:], w_c2_d[:])

            h1 = actp.tile([GDM + 1, T], BF16, name="h1", tag="h1")
            nc.vector.memset(h1[GDM:GDM + 1, :], 1.0)
            for f in range(T // 512):
                fs = slice(f * 512, (f + 1) * 512)
                pt = pmm([GDM, 512])
                for kt in range(DTILES):
                    nc.tensor.matmul(pt[:], wc1[:, kt * GDM:(kt + 1) * GDM],
                                     xfm16[kt][:, fs], start=(kt == 0), stop=False)
                nc.tensor.matmul(pt[:], wc1[0:1, 6 * GDM:7 * GDM], brow[:, fs],
                                 start=False, stop=True)
                nc.scalar.activation(h1[0:GDM, fs], pt[:], AF.Silu, bias=bc1[:, 0:1])

            g_in = dramp.tile([T, DM], FP32, name="g_in", tag="g_in")
            g_out = dramp.tile([T, DM], FP32, name="g_out", tag="g_out")
            for c in range(NCH):
                h2sb = st2.tile([128, DM], FP32, name="h2sb", tag="h2sb")
                for fs2 in range(2):
                    pt = pmm([128, 384])
                    nc.tensor.matmul(pt[:], h1[:, c * Q:(c + 1) * Q],
                                     wc2[:, fs2 * 384:(fs2 + 1) * 384],
                                     start=True, stop=True)
                    nc.scalar.copy(h2sb[:, fs2 * 384:(fs2 + 1) * 384], pt[:])
                nc.sync.dma_start(g_in[c * Q:(c + 1) * Q, :], h2sb[:])
            nc.gpsimd.collective_compute("AllReduce", OP.add,
                                         replica_groups=[list(range(NC))],
                                         ins=[g_in[:]], outs=[g_out[:]])

            n16 = actp.tile([1, DM], BF16, name="n16", tag="n16")
            n16b = actp.tile([1, DM], BF16, name="n16b", tag="n16b")
            nc.vector.tensor_copy(n16[:], nrow[64:65, :])
            nc.vector.tensor_copy(n16b[:], nrow[96:97, :])
            nfw_rep = actp.tile([128, DM], BF16, name="nfw_rep", tag="nfw_rep")
            nfb_rep = actp.tile([128, DM], BF16, name="nfb_rep", tag="nfb_rep")
            for fs2 in range(2):
                rp = pmm([128, 384])
                nc.tensor.matmul(rp[:], onesrow16[:1, :128],
                                 n16[0:1, fs2 * 384:(fs2 + 1) * 384], start=True, stop=True)
                nc.scalar.copy(nfw_rep[:, fs2 * 384:(fs2 + 1) * 384], rp[:])
                rp2 = pmm([128, 384])
                nc.tensor.matmul(rp2[:], onesrow16[:1, :128],
                                 n16b[0:1, fs2 * 384:(fs2 + 1) * 384], start=True, stop=True)
                nc.scalar.copy(nfb_rep[:, fs2 * 384:(fs2 + 1) * 384], rp2[:])

            for c in range(NCH):
                mixed_tm = st2.tile([128, DM], BF16, name="mixed_tm", tag="mixed_tm")
                for j in range(DTILES):
                    ptt = pmm([128, 128], BF16)
                    nc.tensor.transpose(ptt[:], mixed[j][:, c * Q:(c + 1) * Q], ident16[:])
                    nc.scalar.copy(mixed_tm[:, j * 128:(j + 1) * 128], ptt[:])
                xt = st2.tile([128, DM], FP32, name="xt", tag="x_tm_c")
                nc.sync.dma_start(xt[:], x_d[c * Q:(c + 1) * Q, :])
                gt = st2.tile([128, DM], FP32, name="gt", tag="cacc0")
                nc.sync.dma_start(gt[:], g_out[c * Q:(c + 1) * Q, :])
                nc.scalar.activation(gt[:], gt[:], AF.Sigmoid)
                nc.sync.dma_start(gate_d[c * Q:(c + 1) * Q, :], gt[:])
                ot = st2.tile([128, DM], FP32, name="ot", tag="cacc1")
                nc.vector.tensor_tensor(ot[:], mixed_tm[:], xt[:], OP.subtract)
                nc.vector.tensor_tensor(ot[:], ot[:], gt[:], OP.mult)
                nc.vector.tensor_tensor(ot[:], ot[:], xt[:], OP.add)
                st = st2.tile([128, 1], FP32, name="st", tag="st")
                nc.vector.tensor_reduce(st[:], ot[:], axis=AX.X, op=OP.add)
                nc.scalar.activation(st[:], st[:], AF.Copy, scale=1.0 / DM)
                nc.vector.tensor_scalar(ot[:], ot[:], st[:, 0:1], None, OP.subtract)
                sq2 = st2.tile([128, DM], FP32, name="sq2", tag="h2sb")
                nc.vector.tensor_tensor(sq2[:], ot[:], ot[:], OP.mult)
                v2 = st2.tile([128, 1], FP32, name="v2", tag="v2")
                nc.vector.tensor_reduce(v2[:], sq2[:], axis=AX.X, op=OP.add)
                nc.scalar.activation(v2[:], v2[:], AF.Ln, bias=eps_ap[:], scale=1.0 / DM)
                nc.scalar.activation(v2[:], v2[:], AF.Exp, scale=-0.5)
                nc.vector.tensor_scalar(ot[:], ot[:], v2[:, 0:1], None, OP.mult)
                nc.vector.tensor_tensor(ot[:], ot[:], nfw_rep[:], OP.mult)
                nc.vector.tensor_tensor(ot[:], ot[:], nfb_rep[:], OP.add)
                nc.sync.dma_start(out_d[c * Q:(c + 1) * Q, :], ot[:])

    nc.compile()
    return nc


def _pack_fm(arr, pad_to=128):
    arr = np.asarray(arr)
    if arr.ndim == 1:
        arr = arr[:, None]
    F, W = arr.shape
    nblk = (F + pad_to - 1) // pad_to
    outp = np.zeros((pad_to, nblk * W), dtype=arr.dtype)
    for b_ in range(nblk):
        blk = arr[b_ * pad_to:(b_ + 1) * pad_to]
        outp[:blk.shape[0], b_ * W:(b_ + 1) * W] = blk
    return outp


def _prep_inputs(inputs):
    f32 = np.float32
    x = np.ascontiguousarray(np.asarray(inputs["x"], f32).reshape(T, DM))
    bprob = np.ascontiguousarray(np.asarray(inputs["boundary_prob"], f32).reshape(1, T))
    idx = np.arange(128)
    tri = (idx[:, None] <= idx[None, :]).astype(f32)          # [tau, t']
    tricf = np.concatenate([tri - 0.5, np.full((128, 1), 0.5, f32)], axis=1)
    maps = []
    for c in range(NC):
        sl = slice(c * ELOC, (c + 1) * ELOC)
        w_in = np.stack([_pack_fm(
            np.concatenate([np.asarray(inputs["in_proj_w"][i])[sl],
                            np.asarray(inputs["in_proj_w"][i])[E + c * ELOC:E + (c + 1) * ELOC]],
                           axis=0).T.astype(f32))
            for i in range(NL)])
        w_xp = np.stack([_pack_fm(np.asarray(inputs["x_proj_w"][i], f32)[:, sl].T)
                         for i in range(NL)])
        w_dt = np.stack([
            np.concatenate([np.asarray(inputs["dt_proj_b"][i], f32)[None, sl],
                            np.asarray(inputs["dt_proj_w"][i], f32)[sl].T], axis=0)
            for i in range(NL)])
        w_out = np.stack([_pack_fm(np.asarray(inputs["out_proj_w"][i], f32)[:, sl].T)
                          for i in range(NL)])
        lnp = np.stack([_pack_fm(np.stack([np.asarray(inputs["ln_w"][i], f32),
                                           np.asarray(inputs["ln_b"][i], f32)], axis=1))
                        for i in range(NL)])
        gsl = slice(c * GDM, (c + 1) * GDM)
        cw1 = np.asarray(inputs["ctrl_w1"], f32)
        w_c1 = np.concatenate([_pack_fm(cw1[gsl, :DM].T),
                               _pack_fm(cw1[gsl, DM:DM + 1].T)], axis=1)
        w_c2 = np.concatenate([np.asarray(inputs["ctrl_w2"], f32)[:, gsl].T,
                               (np.asarray(inputs["ctrl_b2"], f32) / NC)[None, :]], axis=0)
        nrm = np.stack([np.asarray(inputs["normf_w"], f32), np.asarray(inputs["normf_b"], f32),
                        np.asarray(inputs["out_ln_w"], f32), np.asarray(inputs["out_ln_b"], f32)])
        nrmc = _pack_fm(np.stack([np.asarray(inputs["normf_w"], f32),
                                  np.asarray(inputs["normf_b"], f32)], axis=1))
        maps.append({
            "x": x, "bprob": bprob, "w_in": w_in,
            "conv_w": np.stack([_pack_fm(np.asarray(inputs["conv_w"][i], f32)[sl])
                                for i in range(NL)]),
            "conv_b": np.stack([_pack_fm(np.asarray(inputs["conv_b"][i], f32)[sl])
                                for i in range(NL)]),
            "w_xp": w_xp, "w_dt": w_dt, "w_out": w_out, "lnp": lnp,
            "ssmd": np.stack([_pack_fm(np.asarray(inputs["ssm_D"][i], f32)[sl])
                              for i in range(NL)]),
            "w_c1": w_c1,
            "b_c1": np.asarray(inputs["ctrl_b1"], f32)[gsl][:, None],
            "w_c2": w_c2, "nrm": nrm, "nrmc": nrmc,
            "tri16": tri, "tricf": tricf,
        })
    return maps


def kernel(**inputs):
    import ml_dtypes
    maps = _prep_inputs(inputs)
    A = -np.exp(np.asarray(inputs["A_log"], np.float32))
    a_scales = A[0, 0, :]
    for i in range(NL):
        assert np.allclose(A[i], np.broadcast_to(a_scales, (E, N)), rtol=1e-5, atol=1e-6), \
            "kernel assumes channel-independent A"
    key = tuple(np.round(np.asarray(a_scales, np.float64), 6).tolist())
    if key not in _CACHE:
        _CACHE[key] = _build(a_scales)
    nc = _CACHE[key]
    for m in maps:
        for k in ("w_in", "w_xp", "w_out", "w_c1", "w_c2", "bprob", "tri16"):
            m[k] = np.asarray(m[k], dtype=ml_dtypes.bfloat16)
    res = run_bass_kernel_spmd(nc, maps, list(range(NC)))
    kernel._res = res
    r0 = res.results[0]
    out = np.asarray(r0["out"], np.float32).reshape(B, L, DM)
    gate = np.asarray(r0["gate"], np.float32).reshape(B, L, DM)
    return out, gate


# revision 23
# speedup vs baseline: 1.1283x; 1.1283x over previous
"""Trainium2 Bass kernel for nn_BoundaryControlledMixer (4-layer Mamba stack +
boundary-controlled gate), tensor-parallel over d_inner across 8 NeuronCores.

Per core (owns E_loc = 192 of E = 1536 channels, full batch/sequence):
  - Activations flow feature-major [feat, token] so projections chain on the
    PE without transposes (matmul contracts the partition dim).
  - Selective scan: chunked form, chunk Q = 128 tokens (token-major).  With
    A[e,n] = A_n = -exp(A_log[n]) (channel-independent), for tokens in chunk:
        S'_t[e] = in-chunk centered cumsum of dt  ( (TRI - 1/2) @ dt , PE )
        P  = exp(+A_n S') = r^n,  E1 = exp(-A_n S') = rinv^n   (DVE doubling)
        qv[tau,(n,e)] = u[tau,e] B[tau,n] E1[tau,n,e]          (DVE)
        qv[0] += state_row                                     (chunk carry)
        cumQ = TRI @ qv                                        (PE, fp32 PSUM)
        y    = sum_n C[t,n] * P[t,n,e] * cumQ[t,n,e]           (DVE tree)
    Centering keeps |A_n S'| <= |A_n|*chunk_dtsum/2 < 60: no overflow.
    Carried state row = hc[last_token] (= P_end*cumQ_end) scaled by
    exp(A_n * m) for the next chunk (m = chunk midpoint value).
"""

import numpy as np

import concourse.bacc as bacc
import concourse.bass as bass
import concourse.mybir as mybir
import concourse.tile as tile
from concourse import masks
from concourse.bass_utils import run_bass_kernel_spmd

FP32 = mybir.dt.float32
BF16 = mybir.dt.bfloat16
AF = mybir.ActivationFunctionType
OP = mybir.AluOpType
AX = mybir.AxisListType

B, L, DM, NL = 2, 1024, 768, 4
E, N, K, R = 2 * DM, 16, 4, DM // 16
NC = 8
ELOC = E // NC            # 192
T = B * L                 # 2048
Q = 128
NCH = T // Q              # 16
CPB = L // Q              # 8
NH = N // 2               # 8
HW = NH * ELOC            # 1536
EPS = 1e-5
DTILES = DM // 128        # 6
GDM = DM // NC            # 96
LPD = L + 2 * K           # padded per-batch xp row

_CACHE = {}
_DEBUG = False


def _etiles():
    return [(0, 128), (128, 64)]


def _build(a_scales):
    nc = bacc.Bacc("TRN2", target_bir_lowering=False, debug=False)

    x_d = nc.dram_tensor("x", [T, DM], FP32, kind="ExternalInput")
    bprob_d = nc.dram_tensor("bprob", [1, T], BF16, kind="ExternalInput")
    w_in_d = nc.dram_tensor("w_in", [NL, 128, 6 * 2 * ELOC], BF16, kind="ExternalInput")
    conv_w_d = nc.dram_tensor("conv_w", [NL, 128, 2 * K], FP32, kind="ExternalInput")
    conv_b_d = nc.dram_tensor("conv_b", [NL, 128, 2], FP32, kind="ExternalInput")
    w_xp_d = nc.dram_tensor("w_xp", [NL, 128, 2 * (R + 2 * N)], BF16, kind="ExternalInput")
    w_dt_d = nc.dram_tensor("w_dt", [NL, R + 1, ELOC], FP32, kind="ExternalInput")
    w_out_d = nc.dram_tensor("w_out", [NL, 128, 2 * DM], BF16, kind="ExternalInput")
    ln_d = nc.dram_tensor("lnp", [NL, 128, 12], FP32, kind="ExternalInput")
    ssmd_d = nc.dram_tensor("ssmd", [NL, 128, 2], FP32, kind="ExternalInput")
    w_c1_d = nc.dram_tensor("w_c1", [128, 7 * GDM], BF16, kind="ExternalInput")
    b_c1_d = nc.dram_tensor("b_c1", [GDM, 1], FP32, kind="ExternalInput")
    w_c2_d = nc.dram_tensor("w_c2", [GDM + 1, DM], BF16, kind="ExternalInput")
    nrm_d = nc.dram_tensor("nrm", [4, DM], FP32, kind="ExternalInput")
    nrmc_d = nc.dram_tensor("nrmc", [128, 12], FP32, kind="ExternalInput")
    tri16_d = nc.dram_tensor("tri16", [128, 128], BF16, kind="ExternalInput")
    tricf_d = nc.dram_tensor("tricf", [128, 129], FP32, kind="ExternalInput")

    out_d = nc.dram_tensor("out", [T, DM], FP32, kind="ExternalOutput")
    gate_d = nc.dram_tensor("gate", [T, DM], FP32, kind="ExternalOutput")
    dbg = {}
    if _DEBUG:
        dbg["dt"] = nc.dram_tensor("dbg_dt", [T, ELOC], FP32, kind="ExternalOutput")
        dbg["sp"] = nc.dram_tensor("dbg_sp", [T, ELOC], FP32, kind="ExternalOutput")
        dbg["yssm"] = nc.dram_tensor("dbg_yssm", [T, ELOC], FP32, kind="ExternalOutput")
        dbg["hs"] = nc.dram_tensor("dbg_hs", [DM, T], FP32, kind="ExternalOutput")
        dbg["xc"] = nc.dram_tensor("dbg_xc", [ELOC, T], FP32, kind="ExternalOutput")
        dbg["hln"] = nc.dram_tensor("dbg_hln", [DM, T], FP32, kind="ExternalOutput")

    with tile.TileContext(nc) as tc:
        with tc.tile_pool(name="const", bufs=1) as constp, \
             tc.tile_pool(name="persist", bufs=1) as pers, \
             tc.tile_pool(name="wts", bufs=1) as wpool, \
             tc.tile_pool(name="act", bufs=1) as actp, \
             tc.tile_pool(name="st2", bufs=1) as st2, \
             tc.tile_pool(name="vol", bufs=2) as volp, \
             tc.tile_pool(name="rows", bufs=1) as rowp, \
             tc.tile_pool(name="ps_tri", bufs=2, space="PSUM") as ps_tri, \
             tc.tile_pool(name="ps_mm", bufs=2, space="PSUM") as ps_mm, \
             tc.tile_pool(name="dram", bufs=2, space="DRAM") as dramp:

            def pmm(shape, dt=FP32):
                return ps_mm.tile(shape, dt, name="pmm", tag="pmm")

            # ---------- constants ----------
            ident32 = constp.tile([128, 128], FP32)
            masks.make_identity(nc, ident32[:])
            ident16 = constp.tile([128, 128], BF16)
            masks.make_identity(nc, ident16[:])
            tri16 = constp.tile([128, 128], BF16)
            nc.sync.dma_start(tri16[:], tri16_d[:])
            tricf = constp.tile([128, 129], FP32)
            nc.sync.dma_start(tricf[:], tricf_d[:])
            tric32 = tricf[:, 0:128]
            halfcol32 = tricf[:, 128:129]
            onesrow16 = constp.tile([1, 512], BF16)
            nc.gpsimd.memset(onesrow16[:], 1.0)
            eps_ap = constp.tile([128, 1], FP32)
            nc.gpsimd.memset(eps_ap[:], EPS)
            nrow = constp.tile([97, DM], FP32)
            for _i in range(4):
                nc.sync.dma_start(nrow[_i * 32:_i * 32 + 1, :], nrm_d[_i:_i + 1, :])
            nrmc = constp.tile([128, 12], FP32)
            nc.sync.dma_start(nrmc[:], nrmc_d[:])

            # ---------- x -> feature-major fp32 residual ----------
            residual = [pers.tile([128, T], FP32, name=f"res{j}") for j in range(DTILES)]
            for c in range(NCH):
                x_tm_c = st2.tile([128, DM], FP32, name="x_tm_c", tag="x_tm_c")
                nc.sync.dma_start(x_tm_c[:], x_d[c * Q:(c + 1) * Q, :])
                for j in range(DTILES):
                    pt = pmm([128, 128])
                    nc.tensor.transpose(pt[:], x_tm_c[:, j * 128:(j + 1) * 128], ident32[:])
                    nc.scalar.copy(residual[j][:, c * Q:(c + 1) * Q], pt[:])

            # ---------- fused feature-major LayerNorm ----------
            def ln_fm(lnw_aps, lnb_aps, consume, dbg_dst=None):
                stats16 = actp.tile([1, T], BF16, name="stats16", tag="stats16")
                stats16b = actp.tile([1, T], BF16, name="stats16b", tag="stats16b")

                for f in range(T // 512):
                    fs = slice(f * 512, (f + 1) * 512)
                    sp1 = pmm([1, 512])
                    sp2 = pmm([1, 512])
                    for j in range(DTILES):
                        nc.tensor.matmul(sp1[:], halfcol32, residual[j][:, fs],
                                         start=(j == 0), stop=(j == DTILES - 1))
                    nc.scalar.activation(stats16[0:1, fs], sp1[:], AF.Copy, scale=2.0 / DM)
                    for j in range(DTILES):
                        sqj = st2.tile([128, 512], FP32, name="ln_sqj", tag="ln_sqj")
                        nc.vector.tensor_tensor(sqj[:], residual[j][:, fs],
                                                residual[j][:, fs], OP.mult)
                        nc.tensor.matmul(sp2[:], halfcol32, sqj[:],
                                         start=(j == 0), stop=(j == DTILES - 1))
                    nc.scalar.activation(stats16b[0:1, fs], sp2[:], AF.Copy, scale=2.0 / DM)

                for f in range(T // 512):
                    fs = slice(f * 512, (f + 1) * 512)
                    rp = pmm([128, 512])
                    nc.tensor.matmul(rp[:], onesrow16[:1, :128], stats16[0:1, fs],
                                     start=True, stop=True)
                    meanr = st2.tile([128, 512], FP32, name="ln_meanr", tag="ln_meanr")
                    nc.scalar.copy(meanr[:], rp[:])
                    rp2 = pmm([128, 512])
                    nc.tensor.matmul(rp2[:], onesrow16[:1, :128], stats16b[0:1, fs],
                                     start=True, stop=True)
                    invr = st2.tile([128, 512], FP32, name="ln_invr", tag="ln_invr")
                    nc.scalar.copy(invr[:], rp2[:])
                    # var = E[x^2] - mean^2 ; inv = exp(-0.5*ln(var+eps))
                    c2r = st2.tile([128, 512], FP32, name="ln_c2r", tag="ln_c2r")
                    nc.vector.tensor_tensor(c2r[:], meanr[:], meanr[:], OP.mult)
                    nc.vector.tensor_tensor(invr[:], invr[:], c2r[:], OP.subtract)
                    nc.scalar.activation(invr[:], invr[:], AF.Ln, bias=eps_ap[:])
                    nc.scalar.activation(invr[:], invr[:], AF.Exp, scale=-0.5)
                    nc.vector.tensor_tensor(c2r[:], meanr[:], invr[:], OP.mult)
                    slices = []
                    for j in range(DTILES):
                        tmp = st2.tile([128, 512], BF16, name="ln_tmp", tag="ln_tmp", bufs=2)
                        nc.vector.tensor_tensor(tmp[:], residual[j][:, fs], invr[:], OP.mult)
                        nc.vector.tensor_tensor(tmp[:], tmp[:], c2r[:], OP.subtract)
                        hlnf = st2.tile([128, 512], BF16, name="hlnf", tag=f"hlnf{j}")
                        nc.scalar.activation(hlnf[:], tmp[:], AF.Identity,
                                             scale=lnw_aps[j], bias=lnb_aps[j])
                        slices.append(hlnf)
                        if dbg_dst is not None:
                            hld = st2.tile([128, 512], FP32, name="hld", tag="hld")
                            nc.vector.tensor_copy(hld[:], hlnf[:])
                            nc.sync.dma_start(dbg_dst[j * 128:(j + 1) * 128, fs], hld[:])
                    consume(f, slices)

            # ================= layers =================
            for li in range(NL):
                w_in = wpool.tile([128, 6 * 2 * ELOC], BF16, name="w_in_sb", tag="w_in_sb")
                nc.sync.dma_start(w_in[:], w_in_d[li])
                w_cw = wpool.tile([128, 2 * K], FP32, name="w_cw_sb", tag="w_cw_sb")
                nc.sync.dma_start(w_cw[:], conv_w_d[li])
                w_cb = wpool.tile([128, 2], FP32, name="w_cb_sb", tag="w_cb_sb")
                nc.sync.dma_start(w_cb[:], conv_b_d[li])
                w_xp = wpool.tile([128, 2 * (R + 2 * N)], BF16, name="w_xp_sb", tag="w_xp_sb")
                nc.sync.dma_start(w_xp[:], w_xp_d[li])
                w_dt = wpool.tile([R + 1, ELOC], FP32, name="w_dt_sb", tag="w_dt_sb")
                nc.sync.dma_start(w_dt[:], w_dt_d[li])
                w_out = wpool.tile([128, 2 * DM], BF16, name="w_out_sb", tag="w_out_sb")
                nc.sync.dma_start(w_out[:], w_out_d[li])
                w_ln = wpool.tile([128, 12], FP32, name="w_ln_sb", tag="w_ln_sb")
                nc.sync.dma_start(w_ln[:], ln_d[li])
                w_D = wpool.tile([128, 2], FP32, name="w_D_sb", tag="w_D_sb")
                nc.sync.dma_start(w_D[:], ssmd_d[li])

                # ---- LN fused with in_proj ----
                xp_t = [actp.tile([128, B * LPD], BF16, name="xp_pad0", tag="xp_pad0"),
                        actp.tile([64, B * LPD], BF16, name="xp_pad1", tag="xp_pad1")]
                z_t = [actp.tile([128, T], BF16, name="z0", tag="z0"),
                       actp.tile([64, T], BF16, name="z1", tag="z1")]
                for ti in range(2):
                    nc.vector.memset(xp_t[ti][:, 0:K], 0.0)
                    nc.vector.memset(xp_t[ti][:, LPD:LPD + K], 0.0)

                def padcol(fs, fl):
                    b_ = fs // L
                    off = b_ * LPD + K + (fs - b_ * L)
                    return slice(off, off + fl)

                def consume_inproj(f, sl6):
                    fs = f * 512
                    for mt in range(3):
                        pt = pmm([128, 512])
                        for kt in range(DTILES):
                            nc.tensor.matmul(
                                pt[:], w_in[:, kt * 384 + mt * 128:kt * 384 + (mt + 1) * 128],
                                sl6[kt][:], start=(kt == 0), stop=(kt == DTILES - 1))
                        if mt == 0:
                            nc.scalar.copy(xp_t[0][:, padcol(fs, 512)], pt[:])
                        elif mt == 1:
                            nc.scalar.copy(xp_t[1][:, padcol(fs, 512)], pt[0:64, :])
                            nc.scalar.copy(z_t[0][0:64, fs:fs + 512], pt[64:128, :])
                        else:
                            nc.scalar.copy(z_t[0][64:128, fs:fs + 512], pt[0:64, :])
                            nc.scalar.copy(z_t[1][:, fs:fs + 512], pt[64:128, :])

                ln_fm([w_ln[:, 2 * j:2 * j + 1] for j in range(DTILES)],
                      [w_ln[:, 2 * j + 1:2 * j + 2] for j in range(DTILES)],
                      consume_inproj,
                      dbg_dst=dbg["hln"] if (_DEBUG and li == 0) else None)

                # ---- conv + silu ----
                xc = [actp.tile([128, T], BF16, name="xc0", tag="xc0"),
                      actp.tile([64, T], BF16, name="xc1", tag="xc1")]
                for ti, (eo, el) in enumerate(_etiles()):
                    for b_ in range(B):
                        acc = st2.tile([el, L], FP32, name="cacc", tag=f"cacc{ti}")
                        cb = b_ * LPD + K
                        nc.vector.tensor_scalar(acc[:], xp_t[ti][:el, cb - 3:cb - 3 + L],
                                                w_cw[0:el, ti * K:ti * K + 1], None, OP.mult)
                        for j in range(1, K):
                            nc.vector.scalar_tensor_tensor(
                                acc[:], xp_t[ti][:el, cb - 3 + j:cb - 3 + j + L],
                                w_cw[0:el, ti * K + j:ti * K + j + 1],
                                acc[:], OP.mult, OP.add)
                        nc.scalar.activation(xc[ti][:el, b_ * L:(b_ + 1) * L], acc[:],
                                             AF.Silu, bias=w_cb[0:el, ti:ti + 1])
                if _DEBUG and li == 0:
                    for ti, (eo, el) in enumerate(_etiles()):
                        xcd = st2.tile([el, T], FP32, name="xcd", tag="xcd")
                        nc.vector.tensor_copy(xcd[:], xc[ti][:el, :])
                        nc.sync.dma_start(dbg["xc"][eo:eo + el, :], xcd[:])

                # ---- x_proj partial + AllReduce ----
                dbl_in = dramp.tile([R + 2 * N, T], FP32, name="dbl_in", tag="dbl_in")
                dbl_out = dramp.tile([R + 2 * N, T], FP32, name="dbl_out", tag="dbl_out")
                for f in range(T // 512):
                    fs = slice(f * 512, (f + 1) * 512)
                    pt = pmm([80, 512])
                    for ti, (eo, el) in enumerate(_etiles()):
                        nc.tensor.matmul(pt[:], w_xp[0:el, ti * 80:(ti + 1) * 80],
                                         xc[ti][:el, fs], start=(ti == 0), stop=(ti == 1))
                    dblf = st2.tile([80, 512], FP32, name="dblf", tag="dblf")
                    nc.scalar.copy(dblf[:], pt[:])
                    nc.sync.dma_start(dbl_in[:, fs], dblf[:])
                nc.gpsimd.collective_compute("AllReduce", OP.add,
                                             replica_groups=[list(range(NC))],
                                             ins=[dbl_in[:]], outs=[dbl_out[:]])
                # B/C rows -> token-major
                bc_tm = actp.tile([128, NCH * 2 * N], BF16, name="bc_tm", tag="bc_tm")
                for c in range(NCH):
                    bcf = st2.tile([2 * N, Q], FP32, name="bcf", tag="bcf")
                    nc.sync.dma_start(bcf[:], dbl_out[R:R + 2 * N, c * Q:(c + 1) * Q])
                    bc6 = st2.tile([2 * N, Q], BF16, name="bc6", tag="bc6")
                    nc.vector.tensor_copy(bc6[:], bcf[:])
                    ptb = pmm([128, 32], BF16)
                    nc.tensor.transpose(ptb[:], bc6[:], ident16[:32, :32])
                    nc.scalar.copy(bc_tm[:, c * 32:(c + 1) * 32], ptb[:])

                # ---- per-chunk SSM ----
                y_fm = [actp.tile([128, T], BF16, name="yfm0", tag="yfm0"),
                        actp.tile([64, T], BF16, name="yfm1", tag="yfm1")]
                rowbuf = rowp.tile([1, 2 * N * ELOC], BF16, name="rowbuf", tag="rowbuf")
                cq_row = [rowbuf[0:1, h * HW:(h + 1) * HW] for h in range(2)]
                for c in range(NCH):
                    dtf = st2.tile([R + 1, Q], FP32, name="dtf", tag="dtf", bufs=2)
                    nc.vector.memset(dtf[0:1, :], 1.0)
                    nc.sync.dma_start(dtf[1:R + 1, :], dbl_out[0:R, c * Q:(c + 1) * Q])
                    ptd = pmm([128, ELOC])
                    nc.tensor.matmul(ptd[:], dtf[:], w_dt[:], start=True, stop=True)
                    dt_c = st2.tile([128, ELOC], FP32, name="dt_c", tag="dt_c", bufs=2)
                    nc.scalar.activation(dt_c[:], ptd[:], AF.Exp)
                    nc.scalar.activation(dt_c[:], dt_c[:], AF.Ln, bias=1.0)
                    if _DEBUG and li == 0:
                        nc.sync.dma_start(dbg["dt"][c * Q:(c + 1) * Q, :], dt_c[:])
                    pts = pmm([128, ELOC])
                    nc.tensor.matmul(pts[:], tric32, dt_c[:], start=True, stop=True)
                    sp_c = st2.tile([128, ELOC], FP32, name="sp_c", tag="sp_c", bufs=2)
                    nc.scalar.copy(sp_c[:], pts[:])
                    if _DEBUG and li == 0:
                        nc.sync.dma_start(dbg["sp"][c * Q:(c + 1) * Q, :], sp_c[:])
                    ptmm = pmm([1, ELOC])
                    nc.tensor.matmul(ptmm[:], halfcol32, dt_c[:], start=True, stop=True)
                    m_c = st2.tile([1, ELOC], FP32, name="m_c", tag="m_c", bufs=2)
                    nc.scalar.copy(m_c[:], ptmm[:])
                    exr = rowbuf[0:1, N * ELOC:2 * N * ELOC]
                    for n in range(N):
                        nc.scalar.activation(exr[:, n * ELOC:(n + 1) * ELOC], m_c[:],
                                             AF.Exp, scale=float(a_scales[n]))
                    xct = st2.tile([128, ELOC], BF16, name="xct", tag="xct", bufs=2)
                    nc.sync.dma_start_transpose(xct[:, 0:128], xc[0][:, c * Q:(c + 1) * Q])
                    ptx = pmm([128, 64], BF16)
                    nc.tensor.transpose(ptx[:], xc[1][:64, c * Q:(c + 1) * Q], ident16[:64, :64])
                    nc.scalar.copy(xct[:, 128:ELOC], ptx[:])
                    u_c = st2.tile([128, ELOC], BF16, name="u_c", tag="u_c", bufs=2)
                    nc.vector.tensor_tensor(u_c[:], dt_c[:], xct[:], OP.mult)
                    r_c = st2.tile([128, ELOC], BF16, name="r_c", tag="r_c", bufs=2)
                    nc.scalar.activation(r_c[:], sp_c[:], AF.Exp, scale=float(a_scales[0]))
                    ri_c = st2.tile([128, ELOC], BF16, name="ri_c", tag="ri_c", bufs=2)
                    nc.scalar.activation(ri_c[:], sp_c[:], AF.Exp, scale=float(-a_scales[0]))

                    def chain(base, tag):
                        t_ = volp.tile([128, N * ELOC], BF16, name="chn", tag=tag, bufs=1)
                        v = t_[:].rearrange("p (n e) -> p n e", n=N)
                        nc.vector.tensor_copy(v[:, 0, :], base[:])
                        nc.vector.tensor_tensor(v[:, 1, :], base[:], base[:], OP.mult)
                        for lo in (2, 4, 8):
                            nc.vector.tensor_tensor(
                                v[:, lo:2 * lo, :], v[:, 0:lo, :],
                                v[:, lo - 1:lo, :].broadcast_to([128, lo, ELOC]),
                                OP.mult)
                        return t_

                    P_c = chain(r_c, "P_c")
                    E_c = chain(ri_c, "E_c")

                    hcs = []
                    for h in range(2):
                        hsl = slice(h * HW, (h + 1) * HW)
                        qv = volp.tile([128, HW], BF16, name="qv", tag=f"qv{h}", bufs=1)
                        nc.vector.tensor_tensor(
                            qv[:].rearrange("p (n e) -> p n e", n=NH),
                            u_c[:].unsqueeze(1).broadcast_to([128, NH, ELOC]),
                            bc_tm[:, c * 32 + h * NH:c * 32 + (h + 1) * NH]
                            .unsqueeze(2).broadcast_to([128, NH, ELOC]),
                            OP.mult)
                        nc.vector.tensor_tensor(qv[:], qv[:], E_c[:, hsl], OP.mult)
                        if c % CPB != 0:
                            if h == 0:
                                nc.vector.tensor_tensor(rowbuf[0:1, 0:N * ELOC],
                                                        rowbuf[0:1, 0:N * ELOC],
                                                        exr[:], OP.mult)
                            nc.vector.tensor_tensor(qv[0:1, :], qv[0:1, :],
                                                    cq_row[h], OP.add)
                        tp = ps_tri.tile([128, HW], FP32, name="tp", tag="tri")
                        for fsub in range(HW // 512):
                            nc.tensor.matmul(tp[:, fsub * 512:(fsub + 1) * 512], tri16[:],
                                             qv[:, fsub * 512:(fsub + 1) * 512],
                                             start=True, stop=True)
                        hc = volp.tile([128, HW], BF16, name="hc", tag=f"qv{h}", bufs=1)
                        nc.vector.tensor_tensor(hc[:], tp[:], P_c[:, hsl], OP.mult)
                        nc.sync.dma_start(cq_row[h], hc[127:128, :])
                        nc.vector.tensor_tensor(
                            hc[:].rearrange("p (n e) -> p n e", n=NH),
                            hc[:].rearrange("p (n e) -> p n e", n=NH),
                            bc_tm[:, c * 32 + N + h * NH:c * 32 + N + (h + 1) * NH]
                            .unsqueeze(2).broadcast_to([128, NH, ELOC]),
                            OP.mult)
                        hcs.append(hc)
                    nc.vector.tensor_tensor(hcs[0][:], hcs[0][:], hcs[1][:], OP.add)
                    h3 = hcs[0][:].rearrange("p (n e) -> p n e", n=NH)
                    for lev in (4, 2, 1):
                        nc.vector.tensor_tensor(h3[:, 0:lev, :], h3[:, 0:lev, :],
                                                h3[:, lev:2 * lev, :], OP.add)
                    y_c = st2.tile([128, ELOC], BF16, name="y_c", tag="y_c", bufs=3)
                    nc.vector.tensor_copy(y_c[:], h3[:, 0, :])
                    if _DEBUG and li == 0:
                        ydd = st2.tile([128, ELOC], FP32, name="ydd", tag="ydd")
                        nc.vector.tensor_copy(ydd[:], y_c[:])
                        nc.sync.dma_start(dbg["yssm"][c * Q:(c + 1) * Q, :], ydd[:])
                    nc.sync.dma_start_transpose(y_fm[0][:, c * Q:(c + 1) * Q], y_c[:, 0:128])
                    pty = pmm([64, 128], BF16)
                    nc.tensor.transpose(pty[:], y_c[:, 128:ELOC], ident16[:])
                    nc.scalar.copy(y_fm[1][:64, c * Q:(c + 1) * Q], pty[:])

                # ---- D-term, z-gate ----
                for ti, (eo, el) in enumerate(_etiles()):
                    nc.vector.scalar_tensor_tensor(y_fm[ti][:el, :], xc[ti][:el, :],
                                                   w_D[0:el, ti:ti + 1], y_fm[ti][:el, :],
                                                   OP.mult, OP.add)
                    nc.scalar.activation(z_t[ti][:el, :], z_t[ti][:el, :], AF.Silu)
                    nc.vector.tensor_tensor(y_fm[ti][:el, :], y_fm[ti][:el, :],
                                            z_t[ti][:el, :], OP.mult)

                # ---- out_proj partial + AllReduce + residual update ----
                op_in = dramp.tile([DM, T], FP32, name="op_in", tag="op_in")
                op_out = dramp.tile([DM, T], FP32, name="op_out", tag="op_out")
                for mt in range(DTILES):
                    for f in range(T // 512):
                        fs = slice(f * 512, (f + 1) * 512)
                        pt = pmm([128, 512])
                        for ti, (eo, el) in enumerate(_etiles()):
                            nc.tensor.matmul(
                                pt[:], w_out[0:el, ti * DM + mt * 128:ti * DM + (mt + 1) * 128],
                                y_fm[ti][:el, fs], start=(ti == 0), stop=(ti == 1))
                        opf = st2.tile([128, 512], FP32, name="opf", tag="opf")
                        nc.scalar.copy(opf[:], pt[:])
                        nc.sync.dma_start(op_in[mt * 128:(mt + 1) * 128, fs], opf[:])
                nc.gpsimd.collective_compute("AllReduce", OP.add,
                                             replica_groups=[list(range(NC))],
                                             ins=[op_in[:]], outs=[op_out[:]])
                for j in range(DTILES):
                    for f in range(T // 512):
                        fs = slice(f * 512, (f + 1) * 512)
                        hs_f = st2.tile([128, 512], FP32, name="hs_f", tag="hs_f")
                        nc.sync.dma_start(hs_f[:], op_out[j * 128:(j + 1) * 128, fs])
                        nc.vector.tensor_tensor(residual[j][:, fs], residual[j][:, fs],
                                                hs_f[:], OP.add)
                        if _DEBUG and li == 0:
                            nc.sync.dma_start(dbg["hs"][j * 128:(j + 1) * 128, fs], hs_f[:])

            # ================= final stage =================
            mixed = [actp.tile([128, T], BF16, name=f"mx{j}", tag=t)
                     for j, t in enumerate(["xp_pad0", "z0", "xc0", "yfm0", "mxa", "mxb"])]

            def consume_mixed(f, sl6):
                fs = slice(f * 512, (f + 1) * 512)
                for j in range(DTILES):
                    nc.vector.tensor_copy(mixed[j][:, fs], sl6[j][:])

            ln_fm([nrmc[:, 2 * j:2 * j + 1] for j in range(DTILES)],
                  [nrmc[:, 2 * j + 1:2 * j + 2] for j in range(DTILES)],
                  consume_mixed)

            xfm16 = [actp.tile([128, T], BF16, name=f"xfm{j}", tag=t)
                     for j, t in enumerate(["xp_pad1", "z1", "xc1", "yfm1", "xfa", "xfb"])]
            for c in range(NCH):
                x_tm_c = st2.tile([128, DM], FP32, name="x_tm_c2", tag="x_tm_c")
                nc.sync.dma_start(x_tm_c[:], x_d[c * Q:(c + 1) * Q, :])
                for j in range(DTILES):
                    ptt = pmm([128, 128])
                    nc.tensor.transpose(ptt[:], x_tm_c[:, j * 128:(j + 1) * 128], ident32[:])
                    nc.scalar.copy(xfm16[j][:, c * Q:(c + 1) * Q], ptt[:])
            brow = actp.tile([1, T], BF16, name="brow", tag="stats16")
            nc.sync.dma_start(brow[:], bprob_d[:])

            wc1 = wpool.tile([128, 7 * GDM], BF16, name="wc1", tag="w_in_sb")
            nc.sync.dma_start(wc1[:], w_c1_d[:])
            bc1 = wpool.tile([GDM, 1], FP32, name="bc1", tag="w_cb_sb")
            nc.sync.dma_start(bc1[:], b_c1_d[:])
            wc2 = wpool.tile([GDM + 1, DM], BF16, name="wc2", tag="w_out_sb")
            nc.sync.dma_start(wc2[:], w_c2_d[:])

            h1 = actp.tile([GDM + 1, T], BF16, name="h1", tag="h1")
            nc.vector.memset(h1[GDM:GDM + 1, :], 1.0)
            for f in range(T // 512):
                fs = slice(f * 512, (f + 1) * 512)
                pt = pmm([GDM, 512])
                for kt in range(DTILES):
                    nc.tensor.matmul(pt[:], wc1[:, kt * GDM:(kt + 1) * GDM],
                                     xfm16[kt][:, fs], start=(kt == 0), stop=False)
                nc.tensor.matmul(pt[:], wc1[0:1, 6 * GDM:7 * GDM], brow[:, fs],
                                 start=False, stop=True)
                nc.scalar.activation(h1[0:GDM, fs], pt[:], AF.Silu, bias=bc1[:, 0:1])

            g_in = dramp.tile([T, DM], FP32, name="g_in", tag="g_in")
            g_out = dramp.tile([T, DM], FP32, name="g_out", tag="g_out")
            for c in range(NCH):
                h2sb = st2.tile([128, DM], FP32, name="h2sb", tag="h2sb")
                for fs2 in range(2):
                    pt = pmm([128, 384])
                    nc.tensor.matmul(pt[:], h1[:, c * Q:(c + 1) * Q],
                                     wc2[:, fs2 * 384:(fs2 + 1) * 384],
                                     start=True, stop=True)
                    nc.scalar.copy(h2sb[:, fs2 * 384:(fs2 + 1) * 384], pt[:])
                nc.sync.dma_start(g_in[c * Q:(c + 1) * Q, :], h2sb[:])
            nc.gpsimd.collective_compute("AllReduce", OP.add,
                                         replica_groups=[list(range(NC))],
                                         ins=[g_in[:]], outs=[g_out[:]])

            n16 = actp.tile([1, DM], BF16, name="n16", tag="n16")
            n16b = actp.tile([1, DM], BF16, name="n16b", tag="n16b")
            nc.vector.tensor_copy(n16[:], nrow[64:65, :])
            nc.vector.tensor_copy(n16b[:], nrow[96:97, :])
            nfw_rep = actp.tile([128, DM], BF16, name="nfw_rep", tag="nfw_rep")
            nfb_rep = actp.tile([128, DM], BF16, name="nfb_rep", tag="nfb_rep")
            for fs2 in range(2):
                rp = pmm([128, 384])
                nc.tensor.matmul(rp[:], onesrow16[:1, :128],
                                 n16[0:1, fs2 * 384:(fs2 + 1) * 384], start=True, stop=True)
                nc.scalar.copy(nfw_rep[:, fs2 * 384:(fs2 + 1) * 384], rp[:])
                rp2 = pmm([128, 384])
                nc.tensor.matmul(rp2[:], onesrow16[:1, :128],
                                 n16b[0:1, fs2 * 384:(fs2 + 1) * 384], start=True, stop=True)
                nc.scalar.copy(nfb_rep[:, fs2 * 384:(fs2 + 1) * 384], rp2[:])

            for c in range(NCH):
                mixed_tm = st2.tile([128, DM], BF16, name="mixed_tm", tag="mixed_tm")
                for j in range(DTILES):
                    ptt = pmm([128, 128], BF16)
                    nc.tensor.transpose(ptt[:], mixed[j][:, c * Q:(c + 1) * Q], ident16[:])
                    nc.scalar.copy(mixed_tm[:, j * 128:(j + 1) * 128], ptt[:])
                xt = st2.tile([128, DM], FP32, name="xt", tag="x_tm_c")
                nc.sync.dma_start(xt[:], x_d[c * Q:(c + 1) * Q, :])
                gt = st2.tile([128, DM], FP32, name="gt", tag="cacc0")
                nc.sync.dma_start(gt[:], g_out[c * Q:(c + 1) * Q, :])
                nc.scalar.activation(gt[:], gt[:], AF.Sigmoid)
                nc.sync.dma_start(gate_d[c * Q:(c + 1) * Q, :], gt[:])
                ot = st2.tile([128, DM], FP32, name="ot", tag="cacc1")
                nc.vector.tensor_tensor(ot[:], mixed_tm[:], xt[:], OP.subtract)
                nc.vector.tensor_tensor(ot[:], ot[:], gt[:], OP.mult)
                nc.vector.tensor_tensor(ot[:], ot[:], xt[:], OP.add)
                st = st2.tile([128, 1], FP32, name="st", tag="st")
                nc.vector.tensor_reduce(st[:], ot[:], axis=AX.X, op=OP.add)
                nc.scalar.activation(st[:], st[:], AF.Copy, scale=1.0 / DM)
                nc.vector.tensor_scalar(ot[:], ot[:], st[:, 0:1], None, OP.subtract)
                sq2 = st2.tile([128, DM], FP32, name="sq2", tag="h2sb")
                nc.vector.tensor_tensor(sq2[:], ot[:], ot[:], OP.mult)
                v2 = st2.tile([128, 1], FP32, name="v2", tag="v2")
                nc.vector.tensor_reduce(v2[:], sq2[:], axis=AX.X, op=OP.add)
                nc.scalar.activation(v2[:], v2[:], AF.Ln, bias=eps_ap[:], scale=1.0 / DM)
                nc.scalar.activation(v2[:], v2[:], AF.Exp, scale=-0.5)
                nc.vector.tensor_scalar(ot[:], ot[:], v2[:, 0:1], None, OP.mult)
                nc.vector.tensor_tensor(ot[:], ot[:], nfw_rep[:], OP.mult)
                nc.vector.tensor_tensor(ot[:], ot[:], nfb_rep[:], OP.add)
                nc.sync.dma_start(out_d[c * Q:(c + 1) * Q, :], ot[:])

    nc.compile()
    return nc


def _pack_fm(arr, pad_to=128):
    arr = np.asarray(arr)
    if arr.ndim == 1:
        arr = arr[:, None]
    F, W = arr.shape
    nblk = (F + pad_to - 1) // pad_to
    outp = np.zeros((pad_to, nblk * W), dtype=arr.dtype)
    for b_ in range(nblk):
        blk = arr[b_ * pad_to:(b_ + 1) * pad_to]
        outp[:blk.shape[0], b_ * W:(b_ + 1) * W] = blk
    return outp


def _prep_inputs(inputs):
    f32 = np.float32
    x = np.ascontiguousarray(np.asarray(inputs["x"], f32).reshape(T, DM))
    bprob = np.ascontiguousarray(np.asarray(inputs["boundary_prob"], f32).reshape(1, T))
    idx = np.arange(128)
    tri = (idx[:, None] <= idx[None, :]).astype(f32)          # [tau, t']
    tricf = np.concatenate([tri - 0.5, np.full((128, 1), 0.5, f32)], axis=1)
    maps = []
    for c in range(NC):
        sl = slice(c * ELOC, (c + 1) * ELOC)
        w_in = np.stack([_pack_fm(
            np.concatenate([np.asarray(inputs["in_proj_w"][i])[sl],
                            np.asarray(inputs["in_proj_w"][i])[E + c * ELOC:E + (c + 1) * ELOC]],
                           axis=0).T.astype(f32))
            for i in range(NL)])
        w_xp = np.stack([_pack_fm(np.asarray(inputs["x_proj_w"][i], f32)[:, sl].T)
                         for i in range(NL)])
        w_dt = np.stack([
            np.concatenate([np.asarray(inputs["dt_proj_b"][i], f32)[None, sl],
                            np.asarray(inputs["dt_proj_w"][i], f32)[sl].T], axis=0)
            for i in range(NL)])
        w_out = np.stack([_pack_fm(np.asarray(inputs["out_proj_w"][i], f32)[:, sl].T)
                          for i in range(NL)])
        lnp = np.stack([_pack_fm(np.stack([np.asarray(inputs["ln_w"][i], f32),
                                           np.asarray(inputs["ln_b"][i], f32)], axis=1))
                        for i in range(NL)])
        gsl = slice(c * GDM, (c + 1) * GDM)
        cw1 = np.asarray(inputs["ctrl_w1"], f32)
        w_c1 = np.concatenate([_pack_fm(cw1[gsl, :DM].T),
                               _pack_fm(cw1[gsl, DM:DM + 1].T)], axis=1)
        w_c2 = np.concatenate([np.asarray(inputs["ctrl_w2"], f32)[:, gsl].T,
                               (np.asarray(inputs["ctrl_b2"], f32) / NC)[None, :]], axis=0)
        nrm = np.stack([np.asarray(inputs["normf_w"], f32), np.asarray(inputs["normf_b"], f32),
                        np.asarray(inputs["out_ln_w"], f32), np.asarray(inputs["out_ln_b"], f32)])
        nrmc = _pack_fm(np.stack([np.asarray(inputs["normf_w"], f32),
                                  np.asarray(inputs["normf_b"], f32)], axis=1))
        maps.append({
            "x": x, "bprob": bprob, "w_in": w_in,
            "conv_w": np.stack([_pack_fm(np.asarray(inputs["conv_w"][i], f32)[sl])
                                for i in range(NL)]),
            "conv_b": np.stack([_pack_fm(np.asarray(inputs["conv_b"][i], f32)[sl])
                                for i in range(NL)]),
            "w_xp": w_xp, "w_dt": w_dt, "w_out": w_out, "lnp": lnp,
            "ssmd": np.stack([_pack_fm(np.asarray(inputs["ssm_D"][i], f32)[sl])
                              for i in range(NL)]),
            "w_c1": w_c1,
            "b_c1": np.asarray(inputs["ctrl_b1"], f32)[gsl][:, None],
            "w_c2": w_c2, "nrm": nrm, "nrmc": nrmc,
            "tri16": tri, "tricf": tricf,
        })
    return maps


def kernel(**inputs):
    import ml_dtypes
    maps = _prep_inputs(inputs)
    A = -np.exp(np.asarray(inputs["A_log"], np.float32))
    a_scales = A[0, 0, :]
    for i in range(NL):
        assert np.allclose(A[i], np.broadcast_to(a_scales, (E, N)), rtol=1e-5, atol=1e-6), \
            "kernel assumes channel-independent A"
    key = tuple(np.round(np.asarray(a_scales, np.float64), 6).tolist())
    if key not in _CACHE:
        _CACHE[key] = _build(a_scales)
    nc = _CACHE[key]
    for m in maps:
        for k in ("w_in", "w_xp", "w_out", "w_c1", "w_c2", "bprob", "tri16"):
            m[k] = np.asarray(m[k], dtype=ml_dtypes.bfloat16)
    res = run_bass_kernel_spmd(nc, maps, list(range(NC)))
    kernel._res = res
    r0 = res.results[0]
    out = np.asarray(r0["out"], np.float32).reshape(B, L, DM)
    gate = np.asarray(r0["gate"], np.float32).reshape(B, L, DM)
    return out, gate


# revision 30
# speedup vs baseline: 1.1373x; 1.0079x over previous
"""Trainium2 Bass kernel for nn_BoundaryControlledMixer (4-layer Mamba stack +
boundary-controlled gate), tensor-parallel over d_inner across 8 NeuronCores.

Per core (owns E_loc = 192 of E = 1536 channels, full batch/sequence):
  - Activations flow feature-major [feat, token] so projections chain on the
    PE without transposes (matmul contracts the partition dim).
  - Selective scan: chunked form, chunk Q = 128 tokens (token-major).  With
    A[e,n] = A_n = -exp(A_log[n]) (channel-independent), for tokens in chunk:
        S'_t[e] = in-chunk centered cumsum of dt  ( (TRI - 1/2) @ dt , PE )
        P  = exp(+A_n S') = r^n,  E1 = exp(-A_n S') = rinv^n   (DVE doubling)
        qv[tau,(n,e)] = u[tau,e] B[tau,n] E1[tau,n,e]          (DVE)
        qv[0] += state_row                                     (chunk carry)
        cumQ = TRI @ qv                                        (PE, fp32 PSUM)
        y    = sum_n C[t,n] * P[t,n,e] * cumQ[t,n,e]           (DVE tree)
    Centering keeps |A_n S'| <= |A_n|*chunk_dtsum/2 < 60: no overflow.
    Carried state row = hc[last_token] (= P_end*cumQ_end) scaled by
    exp(A_n * m) for the next chunk (m = chunk midpoint value).
"""

import numpy as np

import concourse.bacc as bacc
import concourse.bass as bass
import concourse.mybir as mybir
import concourse.tile as tile
from concourse import masks
from concourse.bass_utils import run_bass_kernel_spmd

FP32 = mybir.dt.float32
BF16 = mybir.dt.bfloat16
AF = mybir.ActivationFunctionType
OP = mybir.AluOpType
AX = mybir.AxisListType

B, L, DM, NL = 2, 1024, 768, 4
E, N, K, R = 2 * DM, 16, 4, DM // 16
NC = 8
ELOC = E // NC            # 192
T = B * L                 # 2048
Q = 128
NCH = T // Q              # 16
CPB = L // Q              # 8
NH = N // 2               # 8
HW = NH * ELOC            # 1536
EPS = 1e-5
DTILES = DM // 128        # 6
GDM = DM // NC            # 96
LPD = L + 2 * K           # padded per-batch xp row

_CACHE = {}
_DEBUG = False


def _etiles():
    return [(0, 128), (128, 64)]


def _build(a_scales):
    nc = bacc.Bacc("TRN2", target_bir_lowering=False, debug=False)

    x_d = nc.dram_tensor("x", [T, DM], FP32, kind="ExternalInput")
    bprob_d = nc.dram_tensor("bprob", [1, T], BF16, kind="ExternalInput")
    w_in_d = nc.dram_tensor("w_in", [NL, 128, 6 * 2 * ELOC], BF16, kind="ExternalInput")
    conv_w_d = nc.dram_tensor("conv_w", [NL, 128, 2 * K], FP32, kind="ExternalInput")
    conv_b_d = nc.dram_tensor("conv_b", [NL, 128, 2], FP32, kind="ExternalInput")
    w_xp_d = nc.dram_tensor("w_xp", [NL, 128, 2 * (R + 2 * N)], BF16, kind="ExternalInput")
    w_dt_d = nc.dram_tensor("w_dt", [NL, R + 1, ELOC], FP32, kind="ExternalInput")
    w_out_d = nc.dram_tensor("w_out", [NL, 128, 2 * DM], BF16, kind="ExternalInput")
    ln_d = nc.dram_tensor("lnp", [NL, 128, 12], FP32, kind="ExternalInput")
    ssmd_d = nc.dram_tensor("ssmd", [NL, 128, 2], FP32, kind="ExternalInput")
    w_c1_d = nc.dram_tensor("w_c1", [128, 7 * GDM], BF16, kind="ExternalInput")
    b_c1_d = nc.dram_tensor("b_c1", [GDM, 1], FP32, kind="ExternalInput")
    w_c2_d = nc.dram_tensor("w_c2", [GDM + 1, DM], BF16, kind="ExternalInput")
    nrm_d = nc.dram_tensor("nrm", [4, DM], FP32, kind="ExternalInput")
    nrmc_d = nc.dram_tensor("nrmc", [128, 12], FP32, kind="ExternalInput")
    tri16_d = nc.dram_tensor("tri16", [128, 128], BF16, kind="ExternalInput")
    tricf_d = nc.dram_tensor("tricf", [128, 129], FP32, kind="ExternalInput")

    out_d = nc.dram_tensor("out", [T, DM], FP32, kind="ExternalOutput")
    gate_d = nc.dram_tensor("gate", [T, DM], FP32, kind="ExternalOutput")
    dbg = {}
    if _DEBUG:
        dbg["dt"] = nc.dram_tensor("dbg_dt", [T, ELOC], FP32, kind="ExternalOutput")
        dbg["sp"] = nc.dram_tensor("dbg_sp", [T, ELOC], FP32, kind="ExternalOutput")
        dbg["yssm"] = nc.dram_tensor("dbg_yssm", [T, ELOC], FP32, kind="ExternalOutput")
        dbg["hs"] = nc.dram_tensor("dbg_hs", [DM, T], FP32, kind="ExternalOutput")
        dbg["xc"] = nc.dram_tensor("dbg_xc", [ELOC, T], FP32, kind="ExternalOutput")
        dbg["hln"] = nc.dram_tensor("dbg_hln", [DM, T], FP32, kind="ExternalOutput")

    with tile.TileContext(nc) as tc:
        with tc.tile_pool(name="const", bufs=1) as constp, \
             tc.tile_pool(name="persist", bufs=1) as pers, \
             tc.tile_pool(name="wts", bufs=1) as wpool, \
             tc.tile_pool(name="act", bufs=1) as actp, \
             tc.tile_pool(name="st2", bufs=1) as st2, \
             tc.tile_pool(name="vol", bufs=2) as volp, \
             tc.tile_pool(name="rows", bufs=1) as rowp, \
             tc.tile_pool(name="ps_tri", bufs=2, space="PSUM") as ps_tri, \
             tc.tile_pool(name="ps_mm", bufs=2, space="PSUM") as ps_mm, \
             tc.tile_pool(name="dram", bufs=2, space="DRAM") as dramp:

            def pmm(shape, dt=FP32):
                return ps_mm.tile(shape, dt, name="pmm", tag="pmm")

            # ---------- constants ----------
            ident32 = constp.tile([128, 128], FP32)
            masks.make_identity(nc, ident32[:])
            ident16 = constp.tile([128, 128], BF16)
            masks.make_identity(nc, ident16[:])
            tri16 = constp.tile([128, 128], BF16)
            nc.sync.dma_start(tri16[:], tri16_d[:])
            tricf = constp.tile([128, 129], FP32)
            nc.sync.dma_start(tricf[:], tricf_d[:])
            tric32 = tricf[:, 0:128]
            halfcol32 = tricf[:, 128:129]
            onesrow16 = constp.tile([1, 512], BF16)
            nc.gpsimd.memset(onesrow16[:], 1.0)
            halfcol16 = constp.tile([128, 1], BF16)
            nc.gpsimd.memset(halfcol16[:], 0.5)
            eps_ap = constp.tile([128, 1], FP32)
            nc.gpsimd.memset(eps_ap[:], EPS)
            nrow = constp.tile([97, DM], FP32)
            for _i in range(4):
                nc.sync.dma_start(nrow[_i * 32:_i * 32 + 1, :], nrm_d[_i:_i + 1, :])
            nrmc = constp.tile([128, 12], FP32)
            nc.sync.dma_start(nrmc[:], nrmc_d[:])

            # ---------- x -> feature-major fp32 residual ----------
            residual = [pers.tile([128, T], FP32, name=f"res{j}") for j in range(DTILES)]
            for c in range(NCH):
                x_tm_c = st2.tile([128, DM], FP32, name="x_tm_c", tag="x_tm_c")
                nc.sync.dma_start(x_tm_c[:], x_d[c * Q:(c + 1) * Q, :])
                for j in range(DTILES):
                    pt = pmm([128, 128])
                    nc.tensor.transpose(pt[:], x_tm_c[:, j * 128:(j + 1) * 128], ident32[:])
                    nc.scalar.copy(residual[j][:, c * Q:(c + 1) * Q], pt[:])

            # ---------- fused feature-major LayerNorm ----------
            def ln_fm(lnw_aps, lnb_aps, consume, dbg_dst=None):
                stats16 = actp.tile([1, T], BF16, name="stats16", tag="stats16")
                stats16b = actp.tile([1, T], BF16, name="stats16b", tag="stats16b")

                for f in range(T // 512):
                    fs = slice(f * 512, (f + 1) * 512)
                    sp1 = pmm([1, 512])
                    sp2 = pmm([1, 512])
                    for j in range(DTILES):
                        nc.tensor.matmul(sp1[:], halfcol32, residual[j][:, fs],
                                         start=(j == 0), stop=(j == DTILES - 1))
                    nc.scalar.activation(stats16[0:1, fs], sp1[:], AF.Copy, scale=2.0 / DM)
                    for j in range(DTILES):
                        sqj = st2.tile([128, 512], BF16, name="ln_sqj", tag="ln_sqj")
                        nc.vector.tensor_tensor(sqj[:], residual[j][:, fs],
                                                residual[j][:, fs], OP.mult)
                        nc.tensor.matmul(sp2[:], halfcol16[:], sqj[:],
                                         start=(j == 0), stop=(j == DTILES - 1))
                    nc.scalar.activation(stats16b[0:1, fs], sp2[:], AF.Copy, scale=2.0 / DM)

                for f in range(T // 512):
                    fs = slice(f * 512, (f + 1) * 512)
                    rp = pmm([128, 512])
                    nc.tensor.matmul(rp[:], onesrow16[:1, :128], stats16[0:1, fs],
                                     start=True, stop=True)
                    meanr = st2.tile([128, 512], FP32, name="ln_meanr", tag="ln_meanr")
                    nc.scalar.copy(meanr[:], rp[:])
                    rp2 = pmm([128, 512])
                    nc.tensor.matmul(rp2[:], onesrow16[:1, :128], stats16b[0:1, fs],
                                     start=True, stop=True)
                    invr = st2.tile([128, 512], FP32, name="ln_invr", tag="ln_invr")
                    nc.scalar.copy(invr[:], rp2[:])
                    # var = E[x^2] - mean^2 ; inv = exp(-0.5*ln(var+eps))
                    c2r = st2.tile([128, 512], FP32, name="ln_c2r", tag="ln_c2r")
                    nc.vector.tensor_tensor(c2r[:], meanr[:], meanr[:], OP.mult)
                    nc.vector.tensor_tensor(invr[:], invr[:], c2r[:], OP.subtract)
                    nc.scalar.activation(invr[:], invr[:], AF.Ln, bias=eps_ap[:])
                    nc.scalar.activation(invr[:], invr[:], AF.Exp, scale=-0.5)
                    nc.vector.tensor_tensor(c2r[:], meanr[:], invr[:], OP.mult)
                    slices = []
                    for j in range(DTILES):
                        tmp = st2.tile([128, 512], BF16, name="ln_tmp", tag="ln_tmp", bufs=2)
                        nc.vector.tensor_tensor(tmp[:], residual[j][:, fs], invr[:], OP.mult)
                        nc.vector.tensor_tensor(tmp[:], tmp[:], c2r[:], OP.subtract)
                        hlnf = st2.tile([128, 512], BF16, name="hlnf", tag=f"hlnf{j}")
                        nc.scalar.activation(hlnf[:], tmp[:], AF.Identity,
                                             scale=lnw_aps[j], bias=lnb_aps[j])
                        slices.append(hlnf)
                        if dbg_dst is not None:
                            hld = st2.tile([128, 512], FP32, name="hld", tag="hld")
                            nc.vector.tensor_copy(hld[:], hlnf[:])
                            nc.sync.dma_start(dbg_dst[j * 128:(j + 1) * 128, fs], hld[:])
                    consume(f, slices)

            # ================= layers =================
            for li in range(NL):
                w_in = wpool.tile([128, 6 * 2 * ELOC], BF16, name="w_in_sb", tag="w_in_sb")
                nc.sync.dma_start(w_in[:], w_in_d[li])
                w_cw = wpool.tile([128, 2 * K], FP32, name="w_cw_sb", tag="w_cw_sb")
                nc.sync.dma_start(w_cw[:], conv_w_d[li])
                w_cb = wpool.tile([128, 2], FP32, name="w_cb_sb", tag="w_cb_sb")
                nc.sync.dma_start(w_cb[:], conv_b_d[li])
                w_xp = wpool.tile([128, 2 * (R + 2 * N)], BF16, name="w_xp_sb", tag="w_xp_sb")
                nc.sync.dma_start(w_xp[:], w_xp_d[li])
                w_dt = wpool.tile([R + 1, ELOC], FP32, name="w_dt_sb", tag="w_dt_sb")
                nc.sync.dma_start(w_dt[:], w_dt_d[li])
                w_out = wpool.tile([128, 2 * DM], BF16, name="w_out_sb", tag="w_out_sb")
                nc.sync.dma_start(w_out[:], w_out_d[li])
                w_ln = wpool.tile([128, 12], FP32, name="w_ln_sb", tag="w_ln_sb")
                nc.sync.dma_start(w_ln[:], ln_d[li])
                w_D = wpool.tile([128, 2], FP32, name="w_D_sb", tag="w_D_sb")
                nc.sync.dma_start(w_D[:], ssmd_d[li])

                # ---- LN fused with in_proj ----
                xp_t = [actp.tile([128, B * LPD], BF16, name="xp_pad0", tag="xp_pad0"),
                        actp.tile([64, B * LPD], BF16, name="xp_pad1", tag="xp_pad1")]
                z_t = [actp.tile([128, T], BF16, name="z0", tag="z0"),
                       actp.tile([64, T], BF16, name="z1", tag="z1")]
                for ti in range(2):
                    nc.vector.memset(xp_t[ti][:, 0:K], 0.0)
                    nc.vector.memset(xp_t[ti][:, LPD:LPD + K], 0.0)

                def padcol(fs, fl):
                    b_ = fs // L
                    off = b_ * LPD + K + (fs - b_ * L)
                    return slice(off, off + fl)

                def consume_inproj(f, sl6):
                    fs = f * 512
                    for mt in range(3):
                        pt = pmm([128, 512])
                        for kt in range(DTILES):
                            nc.tensor.matmul(
                                pt[:], w_in[:, kt * 384 + mt * 128:kt * 384 + (mt + 1) * 128],
                                sl6[kt][:], start=(kt == 0), stop=(kt == DTILES - 1))
                        if mt == 0:
                            nc.scalar.copy(xp_t[0][:, padcol(fs, 512)], pt[:])
                        elif mt == 1:
                            nc.scalar.copy(xp_t[1][:, padcol(fs, 512)], pt[0:64, :])
                            nc.scalar.copy(z_t[0][0:64, fs:fs + 512], pt[64:128, :])
                        else:
                            nc.scalar.copy(z_t[0][64:128, fs:fs + 512], pt[0:64, :])
                            nc.scalar.copy(z_t[1][:, fs:fs + 512], pt[64:128, :])

                ln_fm([w_ln[:, 2 * j:2 * j + 1] for j in range(DTILES)],
                      [w_ln[:, 2 * j + 1:2 * j + 2] for j in range(DTILES)],
                      consume_inproj,
                      dbg_dst=dbg["hln"] if (_DEBUG and li == 0) else None)

                # ---- conv + silu ----
                xc = [actp.tile([128, T], BF16, name="xc0", tag="xc0"),
                      actp.tile([64, T], BF16, name="xc1", tag="xc1")]
                for ti, (eo, el) in enumerate(_etiles()):
                    for b_ in range(B):
                        acc = st2.tile([el, L], FP32, name="cacc", tag="cacc", bufs=2)
                        cb = b_ * LPD + K
                        nc.vector.tensor_scalar(acc[:], xp_t[ti][:el, cb - 3:cb - 3 + L],
                                                w_cw[0:el, ti * K:ti * K + 1], None, OP.mult)
                        for j in range(1, K):
                            nc.vector.scalar_tensor_tensor(
                                acc[:], xp_t[ti][:el, cb - 3 + j:cb - 3 + j + L],
                                w_cw[0:el, ti * K + j:ti * K + j + 1],
                                acc[:], OP.mult, OP.add)
                        nc.scalar.activation(xc[ti][:el, b_ * L:(b_ + 1) * L], acc[:],
                                             AF.Silu, bias=w_cb[0:el, ti:ti + 1])
                if _DEBUG and li == 0:
                    for ti, (eo, el) in enumerate(_etiles()):
                        xcd = st2.tile([el, T], FP32, name="xcd", tag="xcd")
                        nc.vector.tensor_copy(xcd[:], xc[ti][:el, :])
                        nc.sync.dma_start(dbg["xc"][eo:eo + el, :], xcd[:])

                # ---- x_proj partial + AllReduce ----
                dbl_in = dramp.tile([R + 2 * N, T], FP32, name="dbl_in", tag="dbl_in")
                dbl_out = dramp.tile([R + 2 * N, T], FP32, name="dbl_out", tag="dbl_out")
                for f in range(T // 512):
                    fs = slice(f * 512, (f + 1) * 512)
                    pt = pmm([80, 512])
                    for ti, (eo, el) in enumerate(_etiles()):
                        nc.tensor.matmul(pt[:], w_xp[0:el, ti * 80:(ti + 1) * 80],
                                         xc[ti][:el, fs], start=(ti == 0), stop=(ti == 1))
                    dblf = st2.tile([80, 512], FP32, name="dblf", tag="dblf")
                    nc.scalar.copy(dblf[:], pt[:])
                    nc.sync.dma_start(dbl_in[:, fs], dblf[:])
                nc.gpsimd.collective_compute("AllReduce", OP.add,
                                             replica_groups=[list(range(NC))],
                                             ins=[dbl_in[:]], outs=[dbl_out[:]])
                # B/C rows -> token-major
                bc_tm = actp.tile([128, NCH * 2 * N], BF16, name="bc_tm", tag="bc_tm")
                for c in range(NCH):
                    bcf = st2.tile([2 * N, Q], FP32, name="bcf", tag="bcf")
                    nc.sync.dma_start(bcf[:], dbl_out[R:R + 2 * N, c * Q:(c + 1) * Q])
                    bc6 = st2.tile([2 * N, Q], BF16, name="bc6", tag="bc6")
                    nc.vector.tensor_copy(bc6[:], bcf[:])
                    ptb = pmm([128, 32], BF16)
                    nc.tensor.transpose(ptb[:], bc6[:], ident16[:32, :32])
                    nc.scalar.copy(bc_tm[:, c * 32:(c + 1) * 32], ptb[:])

                # ---- per-chunk SSM ----
                y_fm = [actp.tile([128, T], BF16, name="yfm0", tag="yfm0"),
                        actp.tile([64, T], BF16, name="yfm1", tag="yfm1")]
                rowbuf = rowp.tile([1, 2 * N * ELOC], BF16, name="rowbuf", tag="rowbuf")
                cq_row = [rowbuf[0:1, h * HW:(h + 1) * HW] for h in range(2)]
                for c in range(NCH):
                    dtf = st2.tile([R + 1, Q], FP32, name="dtf", tag="dtf", bufs=2)
                    nc.vector.memset(dtf[0:1, :], 1.0)
                    nc.sync.dma_start(dtf[1:R + 1, :], dbl_out[0:R, c * Q:(c + 1) * Q])
                    ptd = pmm([128, ELOC])
                    nc.tensor.matmul(ptd[:], dtf[:], w_dt[:], start=True, stop=True)
                    dt_c = st2.tile([128, ELOC], FP32, name="dt_c", tag="dt_c", bufs=2)
                    nc.scalar.activation(dt_c[:], ptd[:], AF.Exp)
                    nc.scalar.activation(dt_c[:], dt_c[:], AF.Ln, bias=1.0)
                    if _DEBUG and li == 0:
                        nc.sync.dma_start(dbg["dt"][c * Q:(c + 1) * Q, :], dt_c[:])
                    pts = pmm([128, ELOC])
                    nc.tensor.matmul(pts[:], tric32, dt_c[:], start=True, stop=True)
                    sp_c = st2.tile([128, ELOC], FP32, name="sp_c", tag="sp_c", bufs=2)
                    nc.scalar.copy(sp_c[:], pts[:])
                    if _DEBUG and li == 0:
                        nc.sync.dma_start(dbg["sp"][c * Q:(c + 1) * Q, :], sp_c[:])
                    ptmm = pmm([1, ELOC])
                    nc.tensor.matmul(ptmm[:], halfcol32, dt_c[:], start=True, stop=True)
                    m_c = st2.tile([1, ELOC], FP32, name="m_c", tag="m_c", bufs=2)
                    nc.scalar.copy(m_c[:], ptmm[:])
                    exr = rowbuf[0:1, N * ELOC:2 * N * ELOC]
                    for n in range(N):
                        nc.scalar.activation(exr[:, n * ELOC:(n + 1) * ELOC], m_c[:],
                                             AF.Exp, scale=float(a_scales[n]))
                    xct = st2.tile([128, ELOC], BF16, name="xct", tag="xct", bufs=2)
                    nc.sync.dma_start_transpose(xct[:, 0:128], xc[0][:, c * Q:(c + 1) * Q])
                    ptx = pmm([128, 64], BF16)
                    nc.tensor.transpose(ptx[:], xc[1][:64, c * Q:(c + 1) * Q], ident16[:64, :64])
                    nc.scalar.copy(xct[:, 128:ELOC], ptx[:])
                    u_c = st2.tile([128, ELOC], BF16, name="u_c", tag="u_c", bufs=2)
                    nc.vector.tensor_tensor(u_c[:], dt_c[:], xct[:], OP.mult)
                    r_c = st2.tile([128, ELOC], BF16, name="r_c", tag="r_c", bufs=2)
                    nc.scalar.activation(r_c[:], sp_c[:], AF.Exp, scale=float(a_scales[0]))
                    ri_c = st2.tile([128, ELOC], BF16, name="ri_c", tag="ri_c", bufs=2)
                    nc.scalar.activation(ri_c[:], sp_c[:], AF.Exp, scale=float(-a_scales[0]))

                    def chain(base, tag):
                        t_ = volp.tile([128, N * ELOC], BF16, name="chn", tag=tag, bufs=1)
                        v = t_[:].rearrange("p (n e) -> p n e", n=N)
                        nc.vector.tensor_copy(v[:, 0, :], base[:])
                        nc.vector.tensor_tensor(v[:, 1, :], base[:], base[:], OP.mult)
                        for lo in (2, 4, 8):
                            nc.vector.tensor_tensor(
                                v[:, lo:2 * lo, :], v[:, 0:lo, :],
                                v[:, lo - 1:lo, :].broadcast_to([128, lo, ELOC]),
                                OP.mult)
                        return t_

                    P_c = chain(r_c, "P_c")
                    E_c = chain(ri_c, "E_c")

                    hcs = []
                    for h in range(2):
                        hsl = slice(h * HW, (h + 1) * HW)
                        qv = volp.tile([128, HW], BF16, name="qv", tag=f"qv{h}", bufs=2)
                        nc.vector.tensor_tensor(
                            qv[:].rearrange("p (n e) -> p n e", n=NH),
                            u_c[:].unsqueeze(1).broadcast_to([128, NH, ELOC]),
                            bc_tm[:, c * 32 + h * NH:c * 32 + (h + 1) * NH]
                            .unsqueeze(2).broadcast_to([128, NH, ELOC]),
                            OP.mult)
                        nc.vector.tensor_tensor(qv[:], qv[:], E_c[:, hsl], OP.mult)
                        if c % CPB != 0:
                            if h == 0:
                                nc.vector.tensor_tensor(rowbuf[0:1, 0:N * ELOC],
                                                        rowbuf[0:1, 0:N * ELOC],
                                                        exr[:], OP.mult)
                            nc.vector.tensor_tensor(qv[0:1, :], qv[0:1, :],
                                                    cq_row[h], OP.add)
                        tp = ps_tri.tile([128, HW], FP32, name="tp", tag="tri")
                        for fsub in range(HW // 512):
                            nc.tensor.matmul(tp[:, fsub * 512:(fsub + 1) * 512], tri16[:],
                                             qv[:, fsub * 512:(fsub + 1) * 512],
                                             start=True, stop=True)
                        hc = volp.tile([128, HW], BF16, name="hc", tag=f"qv{h}", bufs=2)
                        nc.vector.tensor_tensor(hc[:], tp[:], P_c[:, hsl], OP.mult)
                        nc.sync.dma_start(cq_row[h], hc[127:128, :])
                        nc.vector.tensor_tensor(
                            hc[:].rearrange("p (n e) -> p n e", n=NH),
                            hc[:].rearrange("p (n e) -> p n e", n=NH),
                            bc_tm[:, c * 32 + N + h * NH:c * 32 + N + (h + 1) * NH]
                            .unsqueeze(2).broadcast_to([128, NH, ELOC]),
                            OP.mult)
                        hcs.append(hc)
                    nc.vector.tensor_tensor(hcs[0][:], hcs[0][:], hcs[1][:], OP.add)
                    h3 = hcs[0][:].rearrange("p (n e) -> p n e", n=NH)
                    for lev in (4, 2, 1):
                        nc.vector.tensor_tensor(h3[:, 0:lev, :], h3[:, 0:lev, :],
                                                h3[:, lev:2 * lev, :], OP.add)
                    y_c = st2.tile([128, ELOC], BF16, name="y_c", tag="y_c", bufs=2)
                    nc.vector.tensor_copy(y_c[:], h3[:, 0, :])
                    if _DEBUG and li == 0:
                        ydd = st2.tile([128, ELOC], FP32, name="ydd", tag="ydd")
                        nc.vector.tensor_copy(ydd[:], y_c[:])
                        nc.sync.dma_start(dbg["yssm"][c * Q:(c + 1) * Q, :], ydd[:])
                    nc.sync.dma_start_transpose(y_fm[0][:, c * Q:(c + 1) * Q], y_c[:, 0:128])
                    pty = pmm([64, 128], BF16)
                    nc.tensor.transpose(pty[:], y_c[:, 128:ELOC], ident16[:])
                    nc.scalar.copy(y_fm[1][:64, c * Q:(c + 1) * Q], pty[:])

                # ---- D-term, z-gate ----
                for ti, (eo, el) in enumerate(_etiles()):
                    nc.vector.scalar_tensor_tensor(y_fm[ti][:el, :], xc[ti][:el, :],
                                                   w_D[0:el, ti:ti + 1], y_fm[ti][:el, :],
                                                   OP.mult, OP.add)
                    nc.scalar.activation(z_t[ti][:el, :], z_t[ti][:el, :], AF.Silu)
                    nc.vector.tensor_tensor(y_fm[ti][:el, :], y_fm[ti][:el, :],
                                            z_t[ti][:el, :], OP.mult)

                # ---- out_proj partial + AllReduce + residual update ----
                op_in = dramp.tile([DM, T], FP32, name="op_in", tag="op_in")
                op_out = dramp.tile([DM, T], FP32, name="op_out", tag="op_out")
                for mt in range(DTILES):
                    for f in range(T // 512):
                        fs = slice(f * 512, (f + 1) * 512)
                        pt = pmm([128, 512])
                        for ti, (eo, el) in enumerate(_etiles()):
                            nc.tensor.matmul(
                                pt[:], w_out[0:el, ti * DM + mt * 128:ti * DM + (mt + 1) * 128],
                                y_fm[ti][:el, fs], start=(ti == 0), stop=(ti == 1))
                        opf = st2.tile([128, 512], FP32, name="opf", tag="opf")
                        nc.scalar.copy(opf[:], pt[:])
                        nc.sync.dma_start(op_in[mt * 128:(mt + 1) * 128, fs], opf[:])
                nc.gpsimd.collective_compute("AllReduce", OP.add,
                                             replica_groups=[list(range(NC))],
                                             ins=[op_in[:]], outs=[op_out[:]])
                for j in range(DTILES):
                    for f in range(T // 512):
                        fs = slice(f * 512, (f + 1) * 512)
                        hs_f = st2.tile([128, 512], FP32, name="hs_f", tag="hs_f")
                        nc.sync.dma_start(hs_f[:], op_out[j * 128:(j + 1) * 128, fs])
                        nc.vector.tensor_tensor(residual[j][:, fs], residual[j][:, fs],
                                                hs_f[:], OP.add)
                        if _DEBUG and li == 0:
                            nc.sync.dma_start(dbg["hs"][j * 128:(j + 1) * 128, fs], hs_f[:])

            # ================= final stage =================
            mixed = [(actp.tile([128, T], BF16, name=f"mx{j}", tag=t) if j < 4 else
                      volp.tile([128, T], BF16, name=f"mx{j}", tag=t, bufs=1))
                     for j, t in enumerate(["xp_pad0", "z0", "xc0", "yfm0", "P_c", "E_c"])]

            def consume_mixed(f, sl6):
                fs = slice(f * 512, (f + 1) * 512)
                for j in range(DTILES):
                    nc.vector.tensor_copy(mixed[j][:, fs], sl6[j][:])

            ln_fm([nrmc[:, 2 * j:2 * j + 1] for j in range(DTILES)],
                  [nrmc[:, 2 * j + 1:2 * j + 2] for j in range(DTILES)],
                  consume_mixed)

            xfm16 = [(actp.tile([128, T], BF16, name=f"xfm{j}", tag=t) if j < 4 else
                      st2.tile([128, T], BF16, name=f"xfm{j}", tag=t, bufs=1))
                     for j, t in enumerate(["xp_pad1", "z1", "xc1", "yfm1", "opf", "hs_f"])]
            for c in range(NCH):
                x_tm_c = st2.tile([128, DM], FP32, name="x_tm_c2", tag="x_tm_c")
                nc.sync.dma_start(x_tm_c[:], x_d[c * Q:(c + 1) * Q, :])
                for j in range(DTILES):
                    ptt = pmm([128, 128])
                    nc.tensor.transpose(ptt[:], x_tm_c[:, j * 128:(j + 1) * 128], ident32[:])
                    nc.scalar.copy(xfm16[j][:, c * Q:(c + 1) * Q], ptt[:])
            brow = actp.tile([1, T], BF16, name="brow", tag="stats16")
            nc.sync.dma_start(brow[:], bprob_d[:])

            wc1 = wpool.tile([128, 7 * GDM], BF16, name="wc1", tag="w_in_sb")
            nc.sync.dma_start(wc1[:], w_c1_d[:])
            bc1 = wpool.tile([GDM, 1], FP32, name="bc1", tag="w_cb_sb")
            nc.sync.dma_start(bc1[:], b_c1_d[:])
            wc2 = wpool.tile([GDM + 1, DM], BF16, name="wc2", tag="w_out_sb")
            nc.sync.dma_start(wc2[:], w_c2_d[:])

            h1 = actp.tile([GDM + 1, T], BF16, name="h1", tag="h1")
            nc.vector.memset(h1[GDM:GDM + 1, :], 1.0)
            for f in range(T // 512):
                fs = slice(f * 512, (f + 1) * 512)
                pt = pmm([GDM, 512])
                for kt in range(DTILES):
                    nc.tensor.matmul(pt[:], wc1[:, kt * GDM:(kt + 1) * GDM],
                                     xfm16[kt][:, fs], start=(kt == 0), stop=False)
                nc.tensor.matmul(pt[:], wc1[0:1, 6 * GDM:7 * GDM], brow[:, fs],
                                 start=False, stop=True)
                nc.scalar.activation(h1[0:GDM, fs], pt[:], AF.Silu, bias=bc1[:, 0:1])

            g_in = dramp.tile([T, DM], FP32, name="g_in", tag="g_in")
            g_out = dramp.tile([T, DM], FP32, name="g_out", tag="g_out")
            for c in range(NCH):
                h2sb = st2.tile([128, DM], FP32, name="h2sb", tag="h2sb")
                for fs2 in range(2):
                    pt = pmm([128, 384])
                    nc.tensor.matmul(pt[:], h1[:, c * Q:(c + 1) * Q],
                                     wc2[:, fs2 * 384:(fs2 + 1) * 384],
                                     start=True, stop=True)
                    nc.scalar.copy(h2sb[:, fs2 * 384:(fs2 + 1) * 384], pt[:])
                nc.sync.dma_start(g_in[c * Q:(c + 1) * Q, :], h2sb[:])
            nc.gpsimd.collective_compute("AllReduce", OP.add,
                                         replica_groups=[list(range(NC))],
                                         ins=[g_in[:]], outs=[g_out[:]])

            n16 = actp.tile([1, DM], BF16, name="n16", tag="n16")
            n16b = actp.tile([1, DM], BF16, name="n16b", tag="n16b")
            nc.vector.tensor_copy(n16[:], nrow[64:65, :])
            nc.vector.tensor_copy(n16b[:], nrow[96:97, :])
            nfw_rep = actp.tile([128, DM], BF16, name="nfw_rep", tag="nfw_rep")
            nfb_rep = actp.tile([128, DM], BF16, name="nfb_rep", tag="nfb_rep")
            for fs2 in range(2):
                rp = pmm([128, 384])
                nc.tensor.matmul(rp[:], onesrow16[:1, :128],
                                 n16[0:1, fs2 * 384:(fs2 + 1) * 384], start=True, stop=True)
                nc.scalar.copy(nfw_rep[:, fs2 * 384:(fs2 + 1) * 384], rp[:])
                rp2 = pmm([128, 384])
                nc.tensor.matmul(rp2[:], onesrow16[:1, :128],
                                 n16b[0:1, fs2 * 384:(fs2 + 1) * 384], start=True, stop=True)
                nc.scalar.copy(nfb_rep[:, fs2 * 384:(fs2 + 1) * 384], rp2[:])

            for c in range(NCH):
                mixed_tm = st2.tile([128, DM], BF16, name="mixed_tm", tag="mixed_tm")
                for j in range(DTILES):
                    ptt = pmm([128, 128], BF16)
                    nc.tensor.transpose(ptt[:], mixed[j][:, c * Q:(c + 1) * Q], ident16[:])
                    nc.scalar.copy(mixed_tm[:, j * 128:(j + 1) * 128], ptt[:])
                xt = st2.tile([128, DM], FP32, name="xt", tag="x_tm_c")
                nc.sync.dma_start(xt[:], x_d[c * Q:(c + 1) * Q, :])
                gt = st2.tile([128, DM], FP32, name="gt", tag="cacc", bufs=2)
                nc.sync.dma_start(gt[:], g_out[c * Q:(c + 1) * Q, :])
                nc.scalar.activation(gt[:], gt[:], AF.Sigmoid)
                nc.sync.dma_start(gate_d[c * Q:(c + 1) * Q, :], gt[:])
                ot = st2.tile([128, DM], FP32, name="ot", tag="cacc", bufs=2)
                nc.vector.tensor_tensor(ot[:], mixed_tm[:], xt[:], OP.subtract)
                nc.vector.tensor_tensor(ot[:], ot[:], gt[:], OP.mult)
                nc.vector.tensor_tensor(ot[:], ot[:], xt[:], OP.add)
                st = st2.tile([128, 1], FP32, name="st", tag="st")
                nc.vector.tensor_reduce(st[:], ot[:], axis=AX.X, op=OP.add)
                nc.scalar.activation(st[:], st[:], AF.Copy, scale=1.0 / DM)
                nc.vector.tensor_scalar(ot[:], ot[:], st[:, 0:1], None, OP.subtract)
                sq2 = st2.tile([128, DM], FP32, name="sq2", tag="h2sb")
                nc.vector.tensor_tensor(sq2[:], ot[:], ot[:], OP.mult)
                v2 = st2.tile([128, 1], FP32, name="v2", tag="v2")
                nc.vector.tensor_reduce(v2[:], sq2[:], axis=AX.X, op=OP.add)
                nc.scalar.activation(v2[:], v2[:], AF.Ln, bias=eps_ap[:], scale=1.0 / DM)
                nc.scalar.activation(v2[:], v2[:], AF.Exp, scale=-0.5)
                nc.vector.tensor_scalar(ot[:], ot[:], v2[:, 0:1], None, OP.mult)
                nc.vector.tensor_tensor(ot[:], ot[:], nfw_rep[:], OP.mult)
                nc.vector.tensor_tensor(ot[:], ot[:], nfb_rep[:], OP.add)
                nc.sync.dma_start(out_d[c * Q:(c + 1) * Q, :], ot[:])

    nc.compile()
    return nc


def _pack_fm(arr, pad_to=128):
    arr = np.asarray(arr)
    if arr.ndim == 1:
        arr = arr[:, None]
    F, W = arr.shape
    nblk = (F + pad_to - 1) // pad_to
    outp = np.zeros((pad_to, nblk * W), dtype=arr.dtype)
    for b_ in range(nblk):
        blk = arr[b_ * pad_to:(b_ + 1) * pad_to]
        outp[:blk.shape[0], b_ * W:(b_ + 1) * W] = blk
    return outp


def _prep_inputs(inputs):
    f32 = np.float32
    x = np.ascontiguousarray(np.asarray(inputs["x"], f32).reshape(T, DM))
    bprob = np.ascontiguousarray(np.asarray(inputs["boundary_prob"], f32).reshape(1, T))
    idx = np.arange(128)
    tri = (idx[:, None] <= idx[None, :]).astype(f32)          # [tau, t']
    tricf = np.concatenate([tri - 0.5, np.full((128, 1), 0.5, f32)], axis=1)
    maps = []
    for c in range(NC):
        sl = slice(c * ELOC, (c + 1) * ELOC)
        w_in = np.stack([_pack_fm(
            np.concatenate([np.asarray(inputs["in_proj_w"][i])[sl],
                            np.asarray(inputs["in_proj_w"][i])[E + c * ELOC:E + (c + 1) * ELOC]],
                           axis=0).T.astype(f32))
            for i in range(NL)])
        w_xp = np.stack([_pack_fm(np.asarray(inputs["x_proj_w"][i], f32)[:, sl].T)
                         for i in range(NL)])
        w_dt = np.stack([
            np.concatenate([np.asarray(inputs["dt_proj_b"][i], f32)[None, sl],
                            np.asarray(inputs["dt_proj_w"][i], f32)[sl].T], axis=0)
            for i in range(NL)])
        w_out = np.stack([_pack_fm(np.asarray(inputs["out_proj_w"][i], f32)[:, sl].T)
                          for i in range(NL)])
        lnp = np.stack([_pack_fm(np.stack([np.asarray(inputs["ln_w"][i], f32),
                                           np.asarray(inputs["ln_b"][i], f32)], axis=1))
                        for i in range(NL)])
        gsl = slice(c * GDM, (c + 1) * GDM)
        cw1 = np.asarray(inputs["ctrl_w1"], f32)
        w_c1 = np.concatenate([_pack_fm(cw1[gsl, :DM].T),
                               _pack_fm(cw1[gsl, DM:DM + 1].T)], axis=1)
        w_c2 = np.concatenate([np.asarray(inputs["ctrl_w2"], f32)[:, gsl].T,
                               (np.asarray(inputs["ctrl_b2"], f32) / NC)[None, :]], axis=0)
        nrm = np.stack([np.asarray(inputs["normf_w"], f32), np.asarray(inputs["normf_b"], f32),
                        np.asarray(inputs["out_ln_w"], f32), np.asarray(inputs["out_ln_b"], f32)])
        nrmc = _pack_fm(np.stack([np.asarray(inputs["normf_w"], f32),
                                  np.asarray(inputs["normf_b"], f32)], axis=1))
        maps.append({
            "x": x, "bprob": bprob, "w_in": w_in,
            "conv_w": np.stack([_pack_fm(np.asarray(inputs["conv_w"][i], f32)[sl])
                                for i in range(NL)]),
            "conv_b": np.stack([_pack_fm(np.asarray(inputs["conv_b"][i], f32)[sl])
                                for i in range(NL)]),
            "w_xp": w_xp, "w_dt": w_dt, "w_out": w_out, "lnp": lnp,
            "ssmd": np.stack([_pack_fm(np.asarray(inputs["ssm_D"][i], f32)[sl])
                              for i in range(NL)]),
            "w_c1": w_c1,
            "b_c1": np.asarray(inputs["ctrl_b1"], f32)[gsl][:, None],
            "w_c2": w_c2, "nrm": nrm, "nrmc": nrmc,
            "tri16": tri, "tricf": tricf,
        })
    return maps


def kernel(**inputs):
    import ml_dtypes
    maps = _prep_inputs(inputs)
    A = -np.exp(np.asarray(inputs["A_log"], np.float32))
    a_scales = A[0, 0, :]
    for i in range(NL):
        assert np.allclose(A[i], np.broadcast_to(a_scales, (E, N)), rtol=1e-5, atol=1e-6), \
            "kernel assumes channel-independent A"
    key = tuple(np.round(np.asarray(a_scales, np.float64), 6).tolist())
    if key not in _CACHE:
        _CACHE[key] = _build(a_scales)
    nc = _CACHE[key]
    for m in maps:
        for k in ("w_in", "w_xp", "w_out", "w_c1", "w_c2", "bprob", "tri16"):
            m[k] = np.asarray(m[k], dtype=ml_dtypes.bfloat16)
    res = run_bass_kernel_spmd(nc, maps, list(range(NC)))
    kernel._res = res
    r0 = res.results[0]
    out = np.asarray(r0["out"], np.float32).reshape(B, L, DM)
    gate = np.asarray(r0["gate"], np.float32).reshape(B, L, DM)
    return out, gate


# revision 32
# speedup vs baseline: 1.2188x; 1.0717x over previous
"""Trainium2 Bass kernel for nn_BoundaryControlledMixer (4-layer Mamba stack +
boundary-controlled gate), tensor-parallel over d_inner across 8 NeuronCores.

Per core (owns E_loc = 192 of E = 1536 channels, full batch/sequence):
  - Activations flow feature-major [feat, token] so projections chain on the
    PE without transposes (matmul contracts the partition dim).
  - Selective scan: chunked form, chunk Q = 128 tokens (token-major).  With
    A[e,n] = A_n = -exp(A_log[n]) (channel-independent), for tokens in chunk:
        S'_t[e] = in-chunk centered cumsum of dt  ( (TRI - 1/2) @ dt , PE )
        P  = exp(+A_n S') = r^n,  E1 = exp(-A_n S') = rinv^n   (DVE doubling)
        qv[tau,(n,e)] = u[tau,e] B[tau,n] E1[tau,n,e]          (DVE)
        qv[0] += state_row                                     (chunk carry)
        cumQ = TRI @ qv                                        (PE, fp32 PSUM)
        y    = sum_n C[t,n] * P[t,n,e] * cumQ[t,n,e]           (DVE tree)
    Centering keeps |A_n S'| <= |A_n|*chunk_dtsum/2 < 60: no overflow.
    Carried state row = hc[last_token] (= P_end*cumQ_end) scaled by
    exp(A_n * m) for the next chunk (m = chunk midpoint value).
"""

import numpy as np

import concourse.bacc as bacc
import concourse.bass as bass
import concourse.mybir as mybir
import concourse.tile as tile
from concourse import masks
from concourse.bass_utils import run_bass_kernel_spmd

FP32 = mybir.dt.float32
BF16 = mybir.dt.bfloat16
AF = mybir.ActivationFunctionType
OP = mybir.AluOpType
AX = mybir.AxisListType

B, L, DM, NL = 2, 1024, 768, 4
E, N, K, R = 2 * DM, 16, 4, DM // 16
NC = 8
ELOC = E // NC            # 192
T = B * L                 # 2048
Q = 128
NCH = T // Q              # 16
CPB = L // Q              # 8
NH = N // 2               # 8
HW = NH * ELOC            # 1536
EPS = 1e-5
DTILES = DM // 128        # 6
GDM = DM // NC            # 96
LPD = L + 2 * K           # padded per-batch xp row

_CACHE = {}
_DEBUG = False


def _etiles():
    return [(0, 128), (128, 64)]


def _build(a_scales):
    nc = bacc.Bacc("TRN2", target_bir_lowering=False, debug=False)

    x_d = nc.dram_tensor("x", [T, DM], FP32, kind="ExternalInput")
    bprob_d = nc.dram_tensor("bprob", [1, T], BF16, kind="ExternalInput")
    w_in_d = nc.dram_tensor("w_in", [NL, 128, 6 * 2 * ELOC], BF16, kind="ExternalInput")
    conv_w_d = nc.dram_tensor("conv_w", [NL, 128, 2 * K], FP32, kind="ExternalInput")
    conv_b_d = nc.dram_tensor("conv_b", [NL, 128, 2], FP32, kind="ExternalInput")
    w_xp_d = nc.dram_tensor("w_xp", [NL, 128, 2 * (R + 2 * N)], BF16, kind="ExternalInput")
    w_dt_d = nc.dram_tensor("w_dt", [NL, R + 1, ELOC], FP32, kind="ExternalInput")
    w_out_d = nc.dram_tensor("w_out", [NL, 128, 2 * DM], BF16, kind="ExternalInput")
    ln_d = nc.dram_tensor("lnp", [NL, 128, 12], FP32, kind="ExternalInput")
    ssmd_d = nc.dram_tensor("ssmd", [NL, 128, 2], FP32, kind="ExternalInput")
    w_c1_d = nc.dram_tensor("w_c1", [128, 7 * GDM], BF16, kind="ExternalInput")
    b_c1_d = nc.dram_tensor("b_c1", [GDM, 1], FP32, kind="ExternalInput")
    w_c2_d = nc.dram_tensor("w_c2", [GDM + 1, DM], BF16, kind="ExternalInput")
    nrm_d = nc.dram_tensor("nrm", [4, DM], FP32, kind="ExternalInput")
    nrmc_d = nc.dram_tensor("nrmc", [128, 12], FP32, kind="ExternalInput")
    tri16_d = nc.dram_tensor("tri16", [128, 128], BF16, kind="ExternalInput")
    tricf_d = nc.dram_tensor("tricf", [128, 129], FP32, kind="ExternalInput")

    out_d = nc.dram_tensor("out", [T, DM], FP32, kind="ExternalOutput")
    gate_d = nc.dram_tensor("gate", [T, DM], FP32, kind="ExternalOutput")
    dbg = {}
    if _DEBUG:
        dbg["dt"] = nc.dram_tensor("dbg_dt", [T, ELOC], FP32, kind="ExternalOutput")
        dbg["sp"] = nc.dram_tensor("dbg_sp", [T, ELOC], FP32, kind="ExternalOutput")
        dbg["yssm"] = nc.dram_tensor("dbg_yssm", [T, ELOC], FP32, kind="ExternalOutput")
        dbg["hs"] = nc.dram_tensor("dbg_hs", [DM, T], FP32, kind="ExternalOutput")
        dbg["xc"] = nc.dram_tensor("dbg_xc", [ELOC, T], FP32, kind="ExternalOutput")
        dbg["hln"] = nc.dram_tensor("dbg_hln", [DM, T], FP32, kind="ExternalOutput")

    with tile.TileContext(nc) as tc:
        with tc.tile_pool(name="const", bufs=1) as constp, \
             tc.tile_pool(name="persist", bufs=1) as pers, \
             tc.tile_pool(name="wts", bufs=1) as wpool, \
             tc.tile_pool(name="act", bufs=1) as actp, \
             tc.tile_pool(name="st2", bufs=1) as st2, \
             tc.tile_pool(name="vol", bufs=2) as volp, \
             tc.tile_pool(name="rows", bufs=1) as rowp, \
             tc.tile_pool(name="ps_tri", bufs=2, space="PSUM") as ps_tri, \
             tc.tile_pool(name="ps_mm", bufs=2, space="PSUM") as ps_mm, \
             tc.tile_pool(name="dram", bufs=2, space="DRAM") as dramp:

            def pmm(shape, dt=FP32):
                return ps_mm.tile(shape, dt, name="pmm", tag="pmm")

            # ---------- constants ----------
            ident32 = constp.tile([128, 128], FP32)
            masks.make_identity(nc, ident32[:])
            ident16 = constp.tile([128, 128], BF16)
            masks.make_identity(nc, ident16[:])
            tri16 = constp.tile([128, 128], BF16)
            nc.sync.dma_start(tri16[:], tri16_d[:])
            tricf = constp.tile([128, 129], FP32)
            nc.sync.dma_start(tricf[:], tricf_d[:])
            tric32 = tricf[:, 0:128]
            halfcol32 = tricf[:, 128:129]
            onesrow16 = constp.tile([1, 512], BF16)
            nc.gpsimd.memset(onesrow16[:], 1.0)
            halfcol16 = constp.tile([128, 1], BF16)
            nc.gpsimd.memset(halfcol16[:], 0.5)
            eps_ap = constp.tile([128, 1], FP32)
            nc.gpsimd.memset(eps_ap[:], EPS)
            nrow = constp.tile([97, DM], FP32)
            for _i in range(4):
                nc.sync.dma_start(nrow[_i * 32:_i * 32 + 1, :], nrm_d[_i:_i + 1, :])
            nrmc = constp.tile([128, 12], FP32)
            nc.sync.dma_start(nrmc[:], nrmc_d[:])

            # ---------- x -> feature-major fp32 residual ----------
            residual = [pers.tile([128, T], FP32, name=f"res{j}") for j in range(DTILES)]
            for c in range(NCH):
                x_tm_c = st2.tile([128, DM], FP32, name="x_tm_c", tag="x_tm_c")
                nc.sync.dma_start(x_tm_c[:], x_d[c * Q:(c + 1) * Q, :])
                for j in range(DTILES):
                    pt = pmm([128, 128])
                    nc.tensor.transpose(pt[:], x_tm_c[:, j * 128:(j + 1) * 128], ident32[:])
                    nc.scalar.copy(residual[j][:, c * Q:(c + 1) * Q], pt[:])

            # ---------- fused feature-major LayerNorm ----------
            def ln_fm(lnw_aps, lnb_aps, consume, dbg_dst=None):
                stats16 = actp.tile([1, T], BF16, name="stats16", tag="stats16")
                stats16b = actp.tile([1, T], BF16, name="stats16b", tag="stats16b")

                for f in range(T // 512):
                    fs = slice(f * 512, (f + 1) * 512)
                    sp1 = pmm([1, 512])
                    sp2 = pmm([1, 512])
                    for j in range(DTILES):
                        nc.tensor.matmul(sp1[:], halfcol32, residual[j][:, fs],
                                         start=(j == 0), stop=(j == DTILES - 1))
                    nc.scalar.activation(stats16[0:1, fs], sp1[:], AF.Copy, scale=2.0 / DM)
                    for j in range(DTILES):
                        sqj = st2.tile([128, 512], BF16, name="ln_sqj", tag="ln_sqj")
                        nc.vector.tensor_tensor(sqj[:], residual[j][:, fs],
                                                residual[j][:, fs], OP.mult)
                        nc.tensor.matmul(sp2[:], halfcol16[:], sqj[:],
                                         start=(j == 0), stop=(j == DTILES - 1))
                    nc.scalar.activation(stats16b[0:1, fs], sp2[:], AF.Copy, scale=2.0 / DM)

                for f in range(T // 512):
                    fs = slice(f * 512, (f + 1) * 512)
                    rp = pmm([128, 512])
                    nc.tensor.matmul(rp[:], onesrow16[:1, :128], stats16[0:1, fs],
                                     start=True, stop=True)
                    meanr = st2.tile([128, 512], FP32, name="ln_meanr", tag="ln_meanr")
                    nc.scalar.copy(meanr[:], rp[:])
                    rp2 = pmm([128, 512])
                    nc.tensor.matmul(rp2[:], onesrow16[:1, :128], stats16b[0:1, fs],
                                     start=True, stop=True)
                    invr = st2.tile([128, 512], FP32, name="ln_invr", tag="ln_invr")
                    nc.scalar.copy(invr[:], rp2[:])
                    # var = E[x^2] - mean^2 ; inv = exp(-0.5*ln(var+eps))
                    c2r = st2.tile([128, 512], FP32, name="ln_c2r", tag="ln_c2r")
                    nc.vector.tensor_tensor(c2r[:], meanr[:], meanr[:], OP.mult)
                    nc.vector.tensor_tensor(invr[:], invr[:], c2r[:], OP.subtract)
                    nc.scalar.activation(invr[:], invr[:], AF.Ln, bias=eps_ap[:])
                    nc.scalar.activation(invr[:], invr[:], AF.Exp, scale=-0.5)
                    nc.vector.tensor_tensor(c2r[:], meanr[:], invr[:], OP.mult)
                    slices = []
                    for j in range(DTILES):
                        tmp = st2.tile([128, 512], BF16, name="ln_tmp", tag="ln_tmp", bufs=2)
                        nc.vector.tensor_tensor(tmp[:], residual[j][:, fs], invr[:], OP.mult)
                        nc.vector.tensor_tensor(tmp[:], tmp[:], c2r[:], OP.subtract)
                        hlnf = st2.tile([128, 512], BF16, name="hlnf", tag=f"hlnf{j}")
                        nc.scalar.activation(hlnf[:], tmp[:], AF.Identity,
                                             scale=lnw_aps[j], bias=lnb_aps[j])
                        slices.append(hlnf)
                        if dbg_dst is not None:
                            hld = st2.tile([128, 512], FP32, name="hld", tag="hld")
                            nc.vector.tensor_copy(hld[:], hlnf[:])
                            nc.sync.dma_start(dbg_dst[j * 128:(j + 1) * 128, fs], hld[:])
                    consume(f, slices)

            # ================= layers =================
            for li in range(NL):
                w_in = wpool.tile([128, 6 * 2 * ELOC], BF16, name="w_in_sb", tag="w_in_sb")
                nc.sync.dma_start(w_in[:], w_in_d[li])
                w_cw = wpool.tile([128, 2 * K], FP32, name="w_cw_sb", tag="w_cw_sb")
                nc.sync.dma_start(w_cw[:], conv_w_d[li])
                w_cb = wpool.tile([128, 2], FP32, name="w_cb_sb", tag="w_cb_sb")
                nc.sync.dma_start(w_cb[:], conv_b_d[li])
                w_xp = wpool.tile([128, 2 * (R + 2 * N)], BF16, name="w_xp_sb", tag="w_xp_sb")
                nc.sync.dma_start(w_xp[:], w_xp_d[li])
                w_dt = wpool.tile([R + 1, ELOC], FP32, name="w_dt_sb", tag="w_dt_sb")
                nc.sync.dma_start(w_dt[:], w_dt_d[li])
                w_out = wpool.tile([128, 2 * DM], BF16, name="w_out_sb", tag="w_out_sb")
                nc.sync.dma_start(w_out[:], w_out_d[li])
                w_ln = wpool.tile([128, 12], FP32, name="w_ln_sb", tag="w_ln_sb")
                nc.sync.dma_start(w_ln[:], ln_d[li])
                w_D = wpool.tile([128, 2], FP32, name="w_D_sb", tag="w_D_sb")
                nc.sync.dma_start(w_D[:], ssmd_d[li])

                # ---- LN fused with in_proj ----
                xp_t = [actp.tile([128, B * LPD], BF16, name="xp_pad0", tag="xp_pad0"),
                        actp.tile([64, B * LPD], BF16, name="xp_pad1", tag="xp_pad1")]
                z_t = [actp.tile([128, T], BF16, name="z0", tag="z0"),
                       actp.tile([64, T], BF16, name="z1", tag="z1")]
                for ti in range(2):
                    nc.vector.memset(xp_t[ti][:, 0:K], 0.0)
                    nc.vector.memset(xp_t[ti][:, LPD:LPD + K], 0.0)

                def padcol(fs, fl):
                    b_ = fs // L
                    off = b_ * LPD + K + (fs - b_ * L)
                    return slice(off, off + fl)

                def consume_inproj(f, sl6):
                    fs = f * 512
                    for mt in range(3):
                        pt = pmm([128, 512])
                        for kt in range(DTILES):
                            nc.tensor.matmul(
                                pt[:], w_in[:, kt * 384 + mt * 128:kt * 384 + (mt + 1) * 128],
                                sl6[kt][:], start=(kt == 0), stop=(kt == DTILES - 1))
                        if mt == 0:
                            nc.scalar.copy(xp_t[0][:, padcol(fs, 512)], pt[:])
                        elif mt == 1:
                            nc.scalar.copy(xp_t[1][:, padcol(fs, 512)], pt[0:64, :])
                            nc.scalar.copy(z_t[0][0:64, fs:fs + 512], pt[64:128, :])
                        else:
                            nc.scalar.copy(z_t[0][64:128, fs:fs + 512], pt[0:64, :])
                            nc.scalar.copy(z_t[1][:, fs:fs + 512], pt[64:128, :])

                ln_fm([w_ln[:, 2 * j:2 * j + 1] for j in range(DTILES)],
                      [w_ln[:, 2 * j + 1:2 * j + 2] for j in range(DTILES)],
                      consume_inproj,
                      dbg_dst=dbg["hln"] if (_DEBUG and li == 0) else None)

                # ---- conv + silu ----
                xc = [actp.tile([128, T], BF16, name="xc0", tag="xc0"),
                      actp.tile([64, T], BF16, name="xc1", tag="xc1")]
                for ti, (eo, el) in enumerate(_etiles()):
                    for b_ in range(B):
                        acc = st2.tile([el, L], FP32, name="cacc", tag="cacc", bufs=2)
                        cb = b_ * LPD + K
                        nc.vector.tensor_scalar(acc[:], xp_t[ti][:el, cb - 3:cb - 3 + L],
                                                w_cw[0:el, ti * K:ti * K + 1], None, OP.mult)
                        for j in range(1, K):
                            nc.vector.scalar_tensor_tensor(
                                acc[:], xp_t[ti][:el, cb - 3 + j:cb - 3 + j + L],
                                w_cw[0:el, ti * K + j:ti * K + j + 1],
                                acc[:], OP.mult, OP.add)
                        nc.scalar.activation(xc[ti][:el, b_ * L:(b_ + 1) * L], acc[:],
                                             AF.Silu, bias=w_cb[0:el, ti:ti + 1])
                if _DEBUG and li == 0:
                    for ti, (eo, el) in enumerate(_etiles()):
                        xcd = st2.tile([el, T], FP32, name="xcd", tag="xcd")
                        nc.vector.tensor_copy(xcd[:], xc[ti][:el, :])
                        nc.sync.dma_start(dbg["xc"][eo:eo + el, :], xcd[:])

                # ---- x_proj partial + AllReduce ----
                dbl_in = dramp.tile([R + 2 * N, T], FP32, name="dbl_in", tag="dbl_in")
                dbl_out = dramp.tile([R + 2 * N, T], FP32, name="dbl_out", tag="dbl_out")
                for f in range(T // 512):
                    fs = slice(f * 512, (f + 1) * 512)
                    pt = pmm([80, 512])
                    for ti, (eo, el) in enumerate(_etiles()):
                        nc.tensor.matmul(pt[:], w_xp[0:el, ti * 80:(ti + 1) * 80],
                                         xc[ti][:el, fs], start=(ti == 0), stop=(ti == 1))
                    dblf = st2.tile([80, 512], FP32, name="dblf", tag="dblf")
                    nc.scalar.copy(dblf[:], pt[:])
                    nc.sync.dma_start(dbl_in[:, fs], dblf[:])
                nc.gpsimd.collective_compute("AllReduce", OP.add,
                                             replica_groups=[list(range(NC))],
                                             ins=[dbl_in[:]], outs=[dbl_out[:]])
                # B/C rows -> token-major
                bc_tm = actp.tile([128, NCH * 2 * N], BF16, name="bc_tm", tag="bc_tm")
                for c in range(NCH):
                    bcf = st2.tile([2 * N, Q], FP32, name="bcf", tag="bcf")
                    nc.sync.dma_start(bcf[:], dbl_out[R:R + 2 * N, c * Q:(c + 1) * Q])
                    bc6 = st2.tile([2 * N, Q], BF16, name="bc6", tag="bc6")
                    nc.vector.tensor_copy(bc6[:], bcf[:])
                    ptb = pmm([128, 32], BF16)
                    nc.tensor.transpose(ptb[:], bc6[:], ident16[:32, :32])
                    nc.scalar.copy(bc_tm[:, c * 32:(c + 1) * 32], ptb[:])

                # ---- per-chunk SSM ----
                y_fm = [actp.tile([128, T], BF16, name="yfm0", tag="yfm0"),
                        actp.tile([64, T], BF16, name="yfm1", tag="yfm1")]
                rowbuf = rowp.tile([1, 2 * N * ELOC], BF16, name="rowbuf", tag="rowbuf")
                cq_row = [rowbuf[0:1, h * HW:(h + 1) * HW] for h in range(2)]
                for c in range(NCH):
                    dtf = st2.tile([R + 1, Q], FP32, name="dtf", tag="dtf", bufs=2)
                    nc.vector.memset(dtf[0:1, :], 1.0)
                    nc.sync.dma_start(dtf[1:R + 1, :], dbl_out[0:R, c * Q:(c + 1) * Q])
                    ptd = pmm([128, ELOC])
                    nc.tensor.matmul(ptd[:], dtf[:], w_dt[:], start=True, stop=True)
                    dt_c = st2.tile([128, ELOC], FP32, name="dt_c", tag="dt_c", bufs=2)
                    nc.scalar.activation(dt_c[:], ptd[:], AF.Exp)
                    nc.scalar.activation(dt_c[:], dt_c[:], AF.Ln, bias=1.0)
                    if _DEBUG and li == 0:
                        nc.sync.dma_start(dbg["dt"][c * Q:(c + 1) * Q, :], dt_c[:])
                    pts = pmm([128, ELOC])
                    nc.tensor.matmul(pts[:], tric32, dt_c[:], start=True, stop=True)
                    sp_c = st2.tile([128, ELOC], FP32, name="sp_c", tag="sp_c", bufs=2)
                    nc.scalar.copy(sp_c[:], pts[:])
                    if _DEBUG and li == 0:
                        nc.sync.dma_start(dbg["sp"][c * Q:(c + 1) * Q, :], sp_c[:])
                    ptmm = pmm([1, ELOC])
                    nc.tensor.matmul(ptmm[:], halfcol32, dt_c[:], start=True, stop=True)
                    m_c = st2.tile([1, ELOC], FP32, name="m_c", tag="m_c", bufs=2)
                    nc.scalar.copy(m_c[:], ptmm[:])
                    exr = rowbuf[0:1, N * ELOC:2 * N * ELOC]
                    for n in range(N):
                        nc.scalar.activation(exr[:, n * ELOC:(n + 1) * ELOC], m_c[:],
                                             AF.Exp, scale=float(a_scales[n]))
                    xct = st2.tile([128, ELOC], BF16, name="xct", tag="xct", bufs=2)
                    nc.sync.dma_start_transpose(xct[:, 0:128], xc[0][:, c * Q:(c + 1) * Q])
                    ptx = pmm([128, 64], BF16)
                    nc.tensor.transpose(ptx[:], xc[1][:64, c * Q:(c + 1) * Q], ident16[:64, :64])
                    nc.scalar.copy(xct[:, 128:ELOC], ptx[:])
                    u_c = st2.tile([128, ELOC], BF16, name="u_c", tag="u_c", bufs=2)
                    nc.vector.tensor_tensor(u_c[:], dt_c[:], xct[:], OP.mult)
                    r_c = st2.tile([128, ELOC], BF16, name="r_c", tag="r_c", bufs=2)
                    nc.scalar.activation(r_c[:], sp_c[:], AF.Exp, scale=float(a_scales[0]))
                    ri_c = st2.tile([128, ELOC], BF16, name="ri_c", tag="ri_c", bufs=2)
                    nc.scalar.activation(ri_c[:], sp_c[:], AF.Exp, scale=float(-a_scales[0]))

                    def chain(base, tag, bufs=2):
                        t_ = volp.tile([128, N * ELOC], BF16, name="chn", tag=tag, bufs=bufs)
                        v = t_[:].rearrange("p (n e) -> p n e", n=N)
                        nc.vector.tensor_copy(v[:, 0, :], base[:])
                        nc.vector.tensor_tensor(v[:, 1, :], base[:], base[:], OP.mult)
                        for lo in (2, 4, 8):
                            nc.vector.tensor_tensor(
                                v[:, lo:2 * lo, :], v[:, 0:lo, :],
                                v[:, lo - 1:lo, :].broadcast_to([128, lo, ELOC]),
                                OP.mult)
                        return t_

                    P_c = chain(r_c, "P_c")
                    E_c = chain(ri_c, "E_c", bufs=1)

                    hcs = []
                    for h in range(2):
                        hsl = slice(h * HW, (h + 1) * HW)
                        qv = volp.tile([128, HW], BF16, name="qv", tag=f"qv{h}", bufs=2)
                        nc.vector.tensor_tensor(
                            qv[:].rearrange("p (n e) -> p n e", n=NH),
                            u_c[:].unsqueeze(1).broadcast_to([128, NH, ELOC]),
                            bc_tm[:, c * 32 + h * NH:c * 32 + (h + 1) * NH]
                            .unsqueeze(2).broadcast_to([128, NH, ELOC]),
                            OP.mult)
                        nc.vector.tensor_tensor(qv[:], qv[:], E_c[:, hsl], OP.mult)
                        if c % CPB != 0:
                            if h == 0:
                                nc.vector.tensor_tensor(rowbuf[0:1, 0:N * ELOC],
                                                        rowbuf[0:1, 0:N * ELOC],
                                                        exr[:], OP.mult)
                            nc.vector.tensor_tensor(qv[0:1, :], qv[0:1, :],
                                                    cq_row[h], OP.add)
                        tp = ps_tri.tile([128, HW], FP32, name="tp", tag="tri")
                        for fsub in range(HW // 512):
                            nc.tensor.matmul(tp[:, fsub * 512:(fsub + 1) * 512], tri16[:],
                                             qv[:, fsub * 512:(fsub + 1) * 512],
                                             start=True, stop=True)
                        hc = volp.tile([128, HW], BF16, name="hc", tag=f"qv{h}", bufs=2)
                        nc.vector.tensor_tensor(hc[:], tp[:], P_c[:, hsl], OP.mult)
                        nc.sync.dma_start(cq_row[h], hc[127:128, :])
                        nc.vector.tensor_tensor(
                            hc[:].rearrange("p (n e) -> p n e", n=NH),
                            hc[:].rearrange("p (n e) -> p n e", n=NH),
                            bc_tm[:, c * 32 + N + h * NH:c * 32 + N + (h + 1) * NH]
                            .unsqueeze(2).broadcast_to([128, NH, ELOC]),
                            OP.mult)
                        hcs.append(hc)
                    nc.vector.tensor_tensor(hcs[0][:], hcs[0][:], hcs[1][:], OP.add)
                    h3 = hcs[0][:].rearrange("p (n e) -> p n e", n=NH)
                    for lev in (4, 2, 1):
                        nc.vector.tensor_tensor(h3[:, 0:lev, :], h3[:, 0:lev, :],
                                                h3[:, lev:2 * lev, :], OP.add)
                    y_c = st2.tile([128, ELOC], BF16, name="y_c", tag="y_c", bufs=2)
                    nc.vector.tensor_copy(y_c[:], h3[:, 0, :])
                    if _DEBUG and li == 0:
                        ydd = st2.tile([128, ELOC], FP32, name="ydd", tag="ydd")
                        nc.vector.tensor_copy(ydd[:], y_c[:])
                        nc.sync.dma_start(dbg["yssm"][c * Q:(c + 1) * Q, :], ydd[:])
                    nc.sync.dma_start_transpose(y_fm[0][:, c * Q:(c + 1) * Q], y_c[:, 0:128])
                    pty = pmm([64, 128], BF16)
                    nc.tensor.transpose(pty[:], y_c[:, 128:ELOC], ident16[:])
                    nc.scalar.copy(y_fm[1][:64, c * Q:(c + 1) * Q], pty[:])

                # ---- D-term, z-gate ----
                for ti, (eo, el) in enumerate(_etiles()):
                    nc.vector.scalar_tensor_tensor(y_fm[ti][:el, :], xc[ti][:el, :],
                                                   w_D[0:el, ti:ti + 1], y_fm[ti][:el, :],
                                                   OP.mult, OP.add)
                    nc.scalar.activation(z_t[ti][:el, :], z_t[ti][:el, :], AF.Silu)
                    nc.vector.tensor_tensor(y_fm[ti][:el, :], y_fm[ti][:el, :],
                                            z_t[ti][:el, :], OP.mult)

                # ---- out_proj partial + AllReduce + residual update ----
                op_in = dramp.tile([DM, T], BF16, name="op_in", tag="op_in")
                op_out = dramp.tile([DM, T], BF16, name="op_out", tag="op_out")
                for mt in range(DTILES):
                    for f in range(T // 512):
                        fs = slice(f * 512, (f + 1) * 512)
                        pt = pmm([128, 512])
                        for ti, (eo, el) in enumerate(_etiles()):
                            nc.tensor.matmul(
                                pt[:], w_out[0:el, ti * DM + mt * 128:ti * DM + (mt + 1) * 128],
                                y_fm[ti][:el, fs], start=(ti == 0), stop=(ti == 1))
                        opf = st2.tile([128, 512], BF16, name="opf", tag="opf")
                        nc.scalar.copy(opf[:], pt[:])
                        nc.sync.dma_start(op_in[mt * 128:(mt + 1) * 128, fs], opf[:])
                nc.gpsimd.collective_compute("AllReduce", OP.add,
                                             replica_groups=[list(range(NC))],
                                             ins=[op_in[:]], outs=[op_out[:]])
                for j in range(DTILES):
                    for f in range(T // 512):
                        fs = slice(f * 512, (f + 1) * 512)
                        hs_f = st2.tile([128, 512], BF16, name="hs_f", tag="hs_f")
                        nc.sync.dma_start(hs_f[:], op_out[j * 128:(j + 1) * 128, fs])
                        nc.vector.tensor_tensor(residual[j][:, fs], residual[j][:, fs],
                                                hs_f[:], OP.add)
                        if _DEBUG and li == 0:
                            nc.sync.dma_start(dbg["hs"][j * 128:(j + 1) * 128, fs], hs_f[:])

            # ================= final stage =================
            mixed = [(actp.tile([128, T], BF16, name=f"mx{j}", tag=t) if j < 4 else
                      volp.tile([128, T], BF16, name=f"mx{j}", tag=t, bufs=(2 if t == "P_c" else 1)))
                     for j, t in enumerate(["xp_pad0", "z0", "xc0", "yfm0", "P_c", "E_c"])]

            def consume_mixed(f, sl6):
                fs = slice(f * 512, (f + 1) * 512)
                for j in range(DTILES):
                    nc.vector.tensor_copy(mixed[j][:, fs], sl6[j][:])

            ln_fm([nrmc[:, 2 * j:2 * j + 1] for j in range(DTILES)],
                  [nrmc[:, 2 * j + 1:2 * j + 2] for j in range(DTILES)],
                  consume_mixed)

            xfm16 = [(actp.tile([128, T], BF16, name=f"xfm{j}", tag=t) if j < 4 else
                      st2.tile([128, T], BF16, name=f"xfm{j}", tag=t, bufs=1))
                     for j, t in enumerate(["xp_pad1", "z1", "xc1", "yfm1", "opf", "hs_f"])]
            for c in range(NCH):
                x_tm_c = st2.tile([128, DM], FP32, name="x_tm_c2", tag="x_tm_c")
                nc.sync.dma_start(x_tm_c[:], x_d[c * Q:(c + 1) * Q, :])
                for j in range(DTILES):
                    ptt = pmm([128, 128])
                    nc.tensor.transpose(ptt[:], x_tm_c[:, j * 128:(j + 1) * 128], ident32[:])
                    nc.scalar.copy(xfm16[j][:, c * Q:(c + 1) * Q], ptt[:])
            brow = actp.tile([1, T], BF16, name="brow", tag="stats16")
            nc.sync.dma_start(brow[:], bprob_d[:])

            wc1 = wpool.tile([128, 7 * GDM], BF16, name="wc1", tag="w_in_sb")
            nc.sync.dma_start(wc1[:], w_c1_d[:])
            bc1 = wpool.tile([GDM, 1], FP32, name="bc1", tag="w_cb_sb")
            nc.sync.dma_start(bc1[:], b_c1_d[:])
            wc2 = wpool.tile([GDM + 1, DM], BF16, name="wc2", tag="w_out_sb")
            nc.sync.dma_start(wc2[:], w_c2_d[:])

            h1 = actp.tile([GDM + 1, T], BF16, name="h1", tag="h1")
            nc.vector.memset(h1[GDM:GDM + 1, :], 1.0)
            for f in range(T // 512):
                fs = slice(f * 512, (f + 1) * 512)
                pt = pmm([GDM, 512])
                for kt in range(DTILES):
                    nc.tensor.matmul(pt[:], wc1[:, kt * GDM:(kt + 1) * GDM],
                                     xfm16[kt][:, fs], start=(kt == 0), stop=False)
                nc.tensor.matmul(pt[:], wc1[0:1, 6 * GDM:7 * GDM], brow[:, fs],
                                 start=False, stop=True)
                nc.scalar.activation(h1[0:GDM, fs], pt[:], AF.Silu, bias=bc1[:, 0:1])

            g_in = dramp.tile([T, DM], FP32, name="g_in", tag="g_in")
            g_out = dramp.tile([T, DM], FP32, name="g_out", tag="g_out")
            for c in range(NCH):
                h2sb = st2.tile([128, DM], FP32, name="h2sb", tag="h2sb")
                for fs2 in range(2):
                    pt = pmm([128, 384])
                    nc.tensor.matmul(pt[:], h1[:, c * Q:(c + 1) * Q],
                                     wc2[:, fs2 * 384:(fs2 + 1) * 384],
                                     start=True, stop=True)
                    nc.scalar.copy(h2sb[:, fs2 * 384:(fs2 + 1) * 384], pt[:])
                nc.sync.dma_start(g_in[c * Q:(c + 1) * Q, :], h2sb[:])
            nc.gpsimd.collective_compute("AllReduce", OP.add,
                                         replica_groups=[list(range(NC))],
                                         ins=[g_in[:]], outs=[g_out[:]])

            n16 = actp.tile([1, DM], BF16, name="n16", tag="n16")
            n16b = actp.tile([1, DM], BF16, name="n16b", tag="n16b")
            nc.vector.tensor_copy(n16[:], nrow[64:65, :])
            nc.vector.tensor_copy(n16b[:], nrow[96:97, :])
            nfw_rep = actp.tile([128, DM], BF16, name="nfw_rep", tag="nfw_rep")
            nfb_rep = actp.tile([128, DM], BF16, name="nfb_rep", tag="nfb_rep")
            for fs2 in range(2):
                rp = pmm([128, 384])
                nc.tensor.matmul(rp[:], onesrow16[:1, :128],
                                 n16[0:1, fs2 * 384:(fs2 + 1) * 384], start=True, stop=True)
                nc.scalar.copy(nfw_rep[:, fs2 * 384:(fs2 + 1) * 384], rp[:])
                rp2 = pmm([128, 384])
                nc.tensor.matmul(rp2[:], onesrow16[:1, :128],
                                 n16b[0:1, fs2 * 384:(fs2 + 1) * 384], start=True, stop=True)
                nc.scalar.copy(nfb_rep[:, fs2 * 384:(fs2 + 1) * 384], rp2[:])

            for c in range(NCH):
                mixed_tm = st2.tile([128, DM], BF16, name="mixed_tm", tag="mixed_tm")
                for j in range(DTILES):
                    ptt = pmm([128, 128], BF16)
                    nc.tensor.transpose(ptt[:], mixed[j][:, c * Q:(c + 1) * Q], ident16[:])
                    nc.scalar.copy(mixed_tm[:, j * 128:(j + 1) * 128], ptt[:])
                xt = st2.tile([128, DM], FP32, name="xt", tag="x_tm_c")
                nc.sync.dma_start(xt[:], x_d[c * Q:(c + 1) * Q, :])
                gt = st2.tile([128, DM], FP32, name="gt", tag="cacc", bufs=2)
                nc.sync.dma_start(gt[:], g_out[c * Q:(c + 1) * Q, :])
                nc.scalar.activation(gt[:], gt[:], AF.Sigmoid)
                nc.sync.dma_start(gate_d[c * Q:(c + 1) * Q, :], gt[:])
                ot = st2.tile([128, DM], FP32, name="ot", tag="cacc", bufs=2)
                nc.vector.tensor_tensor(ot[:], mixed_tm[:], xt[:], OP.subtract)
                nc.vector.tensor_tensor(ot[:], ot[:], gt[:], OP.mult)
                nc.vector.tensor_tensor(ot[:], ot[:], xt[:], OP.add)
                st = st2.tile([128, 1], FP32, name="st", tag="st")
                nc.vector.tensor_reduce(st[:], ot[:], axis=AX.X, op=OP.add)
                nc.scalar.activation(st[:], st[:], AF.Copy, scale=1.0 / DM)
                nc.vector.tensor_scalar(ot[:], ot[:], st[:, 0:1], None, OP.subtract)
                sq2 = st2.tile([128, DM], FP32, name="sq2", tag="h2sb")
                nc.vector.tensor_tensor(sq2[:], ot[:], ot[:], OP.mult)
                v2 = st2.tile([128, 1], FP32, name="v2", tag="v2")
                nc.vector.tensor_reduce(v2[:], sq2[:], axis=AX.X, op=OP.add)
                nc.scalar.activation(v2[:], v2[:], AF.Ln, bias=eps_ap[:], scale=1.0 / DM)
                nc.scalar.activation(v2[:], v2[:], AF.Exp, scale=-0.5)
                nc.vector.tensor_scalar(ot[:], ot[:], v2[:, 0:1], None, OP.mult)
                nc.vector.tensor_tensor(ot[:], ot[:], nfw_rep[:], OP.mult)
                nc.vector.tensor_tensor(ot[:], ot[:], nfb_rep[:], OP.add)
                nc.sync.dma_start(out_d[c * Q:(c + 1) * Q, :], ot[:])

    nc.compile()
    return nc


def _pack_fm(arr, pad_to=128):
    arr = np.asarray(arr)
    if arr.ndim == 1:
        arr = arr[:, None]
    F, W = arr.shape
    nblk = (F + pad_to - 1) // pad_to
    outp = np.zeros((pad_to, nblk * W), dtype=arr.dtype)
    for b_ in range(nblk):
        blk = arr[b_ * pad_to:(b_ + 1) * pad_to]
        outp[:blk.shape[0], b_ * W:(b_ + 1) * W] = blk
    return outp


def _prep_inputs(inputs):
    f32 = np.float32
    x = np.ascontiguousarray(np.asarray(inputs["x"], f32).reshape(T, DM))
    bprob = np.ascontiguousarray(np.asarray(inputs["boundary_prob"], f32).reshape(1, T))
    idx = np.arange(128)
    tri = (idx[:, None] <= idx[None, :]).astype(f32)          # [tau, t']
    tricf = np.concatenate([tri - 0.5, np.full((128, 1), 0.5, f32)], axis=1)
    maps = []
    for c in range(NC):
        sl = slice(c * ELOC, (c + 1) * ELOC)
        w_in = np.stack([_pack_fm(
            np.concatenate([np.asarray(inputs["in_proj_w"][i])[sl],
                            np.asarray(inputs["in_proj_w"][i])[E + c * ELOC:E + (c + 1) * ELOC]],
                           axis=0).T.astype(f32))
            for i in range(NL)])
        w_xp = np.stack([_pack_fm(np.asarray(inputs["x_proj_w"][i], f32)[:, sl].T)
                         for i in range(NL)])
        w_dt = np.stack([
            np.concatenate([np.asarray(inputs["dt_proj_b"][i], f32)[None, sl],
                            np.asarray(inputs["dt_proj_w"][i], f32)[sl].T], axis=0)
            for i in range(NL)])
        w_out = np.stack([_pack_fm(np.asarray(inputs["out_proj_w"][i], f32)[:, sl].T)
                          for i in range(NL)])
        lnp = np.stack([_pack_fm(np.stack([np.asarray(inputs["ln_w"][i], f32),
                                           np.asarray(inputs["ln_b"][i], f32)], axis=1))
                        for i in range(NL)])
        gsl = slice(c * GDM, (c + 1) * GDM)
        cw1 = np.asarray(inputs["ctrl_w1"], f32)
        w_c1 = np.concatenate([_pack_fm(cw1[gsl, :DM].T),
                               _pack_fm(cw1[gsl, DM:DM + 1].T)], axis=1)
        w_c2 = np.concatenate([np.asarray(inputs["ctrl_w2"], f32)[:, gsl].T,
                               (np.asarray(inputs["ctrl_b2"], f32) / NC)[None, :]], axis=0)
        nrm = np.stack([np.asarray(inputs["normf_w"], f32), np.asarray(inputs["normf_b"], f32),
                        np.asarray(inputs["out_ln_w"], f32), np.asarray(inputs["out_ln_b"], f32)])
        nrmc = _pack_fm(np.stack([np.asarray(inputs["normf_w"], f32),
                                  np.asarray(inputs["normf_b"], f32)], axis=1))
        maps.append({
            "x": x, "bprob": bprob, "w_in": w_in,
            "conv_w": np.stack([_pack_fm(np.asarray(inputs["conv_w"][i], f32)[sl])
                                for i in range(NL)]),
            "conv_b": np.stack([_pack_fm(np.asarray(inputs["conv_b"][i], f32)[sl])
                                for i in range(NL)]),
            "w_xp": w_xp, "w_dt": w_dt, "w_out": w_out, "lnp": lnp,
            "ssmd": np.stack([_pack_fm(np.asarray(inputs["ssm_D"][i], f32)[sl])
                              for i in range(NL)]),
            "w_c1": w_c1,
            "b_c1": np.asarray(inputs["ctrl_b1"], f32)[gsl][:, None],
            "w_c2": w_c2, "nrm": nrm, "nrmc": nrmc,
            "tri16": tri, "tricf": tricf,
        })
    return maps


def kernel(**inputs):
    import ml_dtypes
    maps = _prep_inputs(inputs)
    A = -np.exp(np.asarray(inputs["A_log"], np.float32))
    a_scales = A[0, 0, :]
    for i in range(NL):
        assert np.allclose(A[i], np.broadcast_to(a_scales, (E, N)), rtol=1e-5, atol=1e-6), \
            "kernel assumes channel-independent A"
    key = tuple(np.round(np.asarray(a_scales, np.float64), 6).tolist())
    if key not in _CACHE:
        _CACHE[key] = _build(a_scales)
    nc = _CACHE[key]
    for m in maps:
        for k in ("w_in", "w_xp", "w_out", "w_c1", "w_c2", "bprob", "tri16"):
            m[k] = np.asarray(m[k], dtype=ml_dtypes.bfloat16)
    res = run_bass_kernel_spmd(nc, maps, list(range(NC)))
    kernel._res = res
    r0 = res.results[0]
    out = np.asarray(r0["out"], np.float32).reshape(B, L, DM)
    gate = np.asarray(r0["gate"], np.float32).reshape(B, L, DM)
    return out, gate


# revision 33
# speedup vs baseline: 1.2200x; 1.0009x over previous
"""Trainium2 Bass kernel for nn_BoundaryControlledMixer (4-layer Mamba stack +
boundary-controlled gate), tensor-parallel over d_inner across 8 NeuronCores.

Per core (owns E_loc = 192 of E = 1536 channels, full batch/sequence):
  - Activations flow feature-major [feat, token] so projections chain on the
    PE without transposes (matmul contracts the partition dim).
  - Selective scan: chunked form, chunk Q = 128 tokens (token-major).  With
    A[e,n] = A_n = -exp(A_log[n]) (channel-independent), for tokens in chunk:
        S'_t[e] = in-chunk centered cumsum of dt  ( (TRI - 1/2) @ dt , PE )
        P  = exp(+A_n S') = r^n,  E1 = exp(-A_n S') = rinv^n   (DVE doubling)
        qv[tau,(n,e)] = u[tau,e] B[tau,n] E1[tau,n,e]          (DVE)
        qv[0] += state_row                                     (chunk carry)
        cumQ = TRI @ qv                                        (PE, fp32 PSUM)
        y    = sum_n C[t,n] * P[t,n,e] * cumQ[t,n,e]           (DVE tree)
    Centering keeps |A_n S'| <= |A_n|*chunk_dtsum/2 < 60: no overflow.
    Carried state row = hc[last_token] (= P_end*cumQ_end) scaled by
    exp(A_n * m) for the next chunk (m = chunk midpoint value).
"""

import numpy as np

import concourse.bacc as bacc
import concourse.bass as bass
import concourse.mybir as mybir
import concourse.tile as tile
from concourse import masks
from concourse.bass_utils import run_bass_kernel_spmd

FP32 = mybir.dt.float32
BF16 = mybir.dt.bfloat16
AF = mybir.ActivationFunctionType
OP = mybir.AluOpType
AX = mybir.AxisListType

B, L, DM, NL = 2, 1024, 768, 4
E, N, K, R = 2 * DM, 16, 4, DM // 16
NC = 8
ELOC = E // NC            # 192
T = B * L                 # 2048
Q = 128
NCH = T // Q              # 16
CPB = L // Q              # 8
NH = N // 2               # 8
HW = NH * ELOC            # 1536
EPS = 1e-5
DTILES = DM // 128        # 6
GDM = DM // NC            # 96
LPD = L + 2 * K           # padded per-batch xp row

_CACHE = {}
_DEBUG = False


def _etiles():
    return [(0, 128), (128, 64)]


def _build(a_scales):
    nc = bacc.Bacc("TRN2", target_bir_lowering=False, debug=False)

    x_d = nc.dram_tensor("x", [T, DM], FP32, kind="ExternalInput")
    bprob_d = nc.dram_tensor("bprob", [1, T], BF16, kind="ExternalInput")
    w_in_d = nc.dram_tensor("w_in", [NL, 128, 6 * 2 * ELOC], BF16, kind="ExternalInput")
    conv_w_d = nc.dram_tensor("conv_w", [NL, 128, 2 * K], FP32, kind="ExternalInput")
    conv_b_d = nc.dram_tensor("conv_b", [NL, 128, 2], FP32, kind="ExternalInput")
    w_xp_d = nc.dram_tensor("w_xp", [NL, 128, 2 * (R + 2 * N)], BF16, kind="ExternalInput")
    w_dt_d = nc.dram_tensor("w_dt", [NL, R + 1, ELOC], FP32, kind="ExternalInput")
    w_out_d = nc.dram_tensor("w_out", [NL, 128, 2 * DM], BF16, kind="ExternalInput")
    ln_d = nc.dram_tensor("lnp", [NL, 128, 12], FP32, kind="ExternalInput")
    ssmd_d = nc.dram_tensor("ssmd", [NL, 128, 2], FP32, kind="ExternalInput")
    w_c1_d = nc.dram_tensor("w_c1", [128, 7 * GDM], BF16, kind="ExternalInput")
    b_c1_d = nc.dram_tensor("b_c1", [GDM, 1], FP32, kind="ExternalInput")
    w_c2_d = nc.dram_tensor("w_c2", [GDM + 1, DM], BF16, kind="ExternalInput")
    nrm_d = nc.dram_tensor("nrm", [4, DM], FP32, kind="ExternalInput")
    nrmc_d = nc.dram_tensor("nrmc", [128, 12], FP32, kind="ExternalInput")
    tri16_d = nc.dram_tensor("tri16", [128, 128], BF16, kind="ExternalInput")
    tricf_d = nc.dram_tensor("tricf", [128, 129], FP32, kind="ExternalInput")

    out_d = nc.dram_tensor("out", [T, DM], FP32, kind="ExternalOutput")
    gate_d = nc.dram_tensor("gate", [T, DM], FP32, kind="ExternalOutput")
    dbg = {}
    if _DEBUG:
        dbg["dt"] = nc.dram_tensor("dbg_dt", [T, ELOC], FP32, kind="ExternalOutput")
        dbg["sp"] = nc.dram_tensor("dbg_sp", [T, ELOC], FP32, kind="ExternalOutput")
        dbg["yssm"] = nc.dram_tensor("dbg_yssm", [T, ELOC], FP32, kind="ExternalOutput")
        dbg["hs"] = nc.dram_tensor("dbg_hs", [DM, T], FP32, kind="ExternalOutput")
        dbg["xc"] = nc.dram_tensor("dbg_xc", [ELOC, T], FP32, kind="ExternalOutput")
        dbg["hln"] = nc.dram_tensor("dbg_hln", [DM, T], FP32, kind="ExternalOutput")

    with tile.TileContext(nc) as tc:
        with tc.tile_pool(name="const", bufs=1) as constp, \
             tc.tile_pool(name="persist", bufs=1) as pers, \
             tc.tile_pool(name="wts", bufs=1) as wpool, \
             tc.tile_pool(name="act", bufs=1) as actp, \
             tc.tile_pool(name="st2", bufs=1) as st2, \
             tc.tile_pool(name="vol", bufs=2) as volp, \
             tc.tile_pool(name="rows", bufs=1) as rowp, \
             tc.tile_pool(name="ps_tri", bufs=2, space="PSUM") as ps_tri, \
             tc.tile_pool(name="ps_mm", bufs=2, space="PSUM") as ps_mm, \
             tc.tile_pool(name="dram", bufs=2, space="DRAM") as dramp:

            def pmm(shape, dt=FP32):
                return ps_mm.tile(shape, dt, name="pmm", tag="pmm")

            # ---------- constants ----------
            ident32 = constp.tile([128, 128], FP32)
            masks.make_identity(nc, ident32[:])
            ident16 = constp.tile([128, 128], BF16)
            masks.make_identity(nc, ident16[:])
            tri16 = constp.tile([128, 128], BF16)
            nc.sync.dma_start(tri16[:], tri16_d[:])
            tricf = constp.tile([128, 129], FP32)
            nc.sync.dma_start(tricf[:], tricf_d[:])
            tric32 = tricf[:, 0:128]
            halfcol32 = tricf[:, 128:129]
            onesrow16 = constp.tile([1, 512], BF16)
            nc.gpsimd.memset(onesrow16[:], 1.0)
            halfcol16 = constp.tile([128, 1], BF16)
            nc.gpsimd.memset(halfcol16[:], 0.5)
            eps_ap = constp.tile([128, 1], FP32)
            nc.gpsimd.memset(eps_ap[:], EPS)
            nrow = constp.tile([97, DM], FP32)
            for _i in range(4):
                nc.sync.dma_start(nrow[_i * 32:_i * 32 + 1, :], nrm_d[_i:_i + 1, :])
            nrmc = constp.tile([128, 12], FP32)
            nc.sync.dma_start(nrmc[:], nrmc_d[:])

            # ---------- x -> feature-major fp32 residual ----------
            residual = [pers.tile([128, T], FP32, name=f"res{j}") for j in range(DTILES)]
            for c in range(NCH):
                x_tm_c = st2.tile([128, DM], FP32, name="x_tm_c", tag="x_tm_c")
                nc.sync.dma_start(x_tm_c[:], x_d[c * Q:(c + 1) * Q, :])
                for j in range(DTILES):
                    pt = pmm([128, 128])
                    nc.tensor.transpose(pt[:], x_tm_c[:, j * 128:(j + 1) * 128], ident32[:])
                    nc.scalar.copy(residual[j][:, c * Q:(c + 1) * Q], pt[:])

            # ---------- fused feature-major LayerNorm ----------
            def ln_fm(lnw_aps, lnb_aps, consume, dbg_dst=None):
                stats16 = actp.tile([1, T], BF16, name="stats16", tag="stats16")
                stats16b = actp.tile([1, T], BF16, name="stats16b", tag="stats16b")

                for f in range(T // 512):
                    fs = slice(f * 512, (f + 1) * 512)
                    sp1 = pmm([1, 512])
                    sp2 = pmm([1, 512])
                    for j in range(DTILES):
                        nc.tensor.matmul(sp1[:], halfcol32, residual[j][:, fs],
                                         start=(j == 0), stop=(j == DTILES - 1))
                    nc.scalar.activation(stats16[0:1, fs], sp1[:], AF.Copy, scale=2.0 / DM)
                    for j in range(DTILES):
                        sqj = st2.tile([128, 512], BF16, name="ln_sqj", tag="ln_sqj")
                        nc.vector.tensor_tensor(sqj[:], residual[j][:, fs],
                                                residual[j][:, fs], OP.mult)
                        nc.tensor.matmul(sp2[:], halfcol16[:], sqj[:],
                                         start=(j == 0), stop=(j == DTILES - 1))
                    nc.scalar.activation(stats16b[0:1, fs], sp2[:], AF.Copy, scale=2.0 / DM)

                for f in range(T // 512):
                    fs = slice(f * 512, (f + 1) * 512)
                    rp = pmm([128, 512])
                    nc.tensor.matmul(rp[:], onesrow16[:1, :128], stats16[0:1, fs],
                                     start=True, stop=True)
                    meanr = st2.tile([128, 512], FP32, name="ln_meanr", tag="ln_meanr")
                    nc.scalar.copy(meanr[:], rp[:])
                    rp2 = pmm([128, 512])
                    nc.tensor.matmul(rp2[:], onesrow16[:1, :128], stats16b[0:1, fs],
                                     start=True, stop=True)
                    invr = st2.tile([128, 512], FP32, name="ln_invr", tag="ln_invr")
                    nc.scalar.copy(invr[:], rp2[:])
                    # var = E[x^2] - mean^2 ; inv = exp(-0.5*ln(var+eps))
                    c2r = st2.tile([128, 512], FP32, name="ln_c2r", tag="ln_c2r")
                    nc.vector.tensor_tensor(c2r[:], meanr[:], meanr[:], OP.mult)
                    nc.vector.tensor_tensor(invr[:], invr[:], c2r[:], OP.subtract)
                    nc.scalar.activation(invr[:], invr[:], AF.Ln, bias=eps_ap[:])
                    nc.scalar.activation(invr[:], invr[:], AF.Exp, scale=-0.5)
                    nc.vector.tensor_tensor(c2r[:], meanr[:], invr[:], OP.mult)
                    slices = []
                    for j in range(DTILES):
                        tmp = st2.tile([128, 512], BF16, name="ln_tmp", tag="ln_tmp", bufs=2)
                        nc.vector.tensor_tensor(tmp[:], residual[j][:, fs], invr[:], OP.mult)
                        nc.vector.tensor_tensor(tmp[:], tmp[:], c2r[:], OP.subtract)
                        hlnf = st2.tile([128, 512], BF16, name="hlnf", tag=f"hlnf{j}")
                        nc.scalar.activation(hlnf[:], tmp[:], AF.Identity,
                                             scale=lnw_aps[j], bias=lnb_aps[j])
                        slices.append(hlnf)
                        if dbg_dst is not None:
                            hld = st2.tile([128, 512], FP32, name="hld", tag="hld")
                            nc.vector.tensor_copy(hld[:], hlnf[:])
                            nc.sync.dma_start(dbg_dst[j * 128:(j + 1) * 128, fs], hld[:])
                    consume(f, slices)

            # ================= layers =================
            for li in range(NL):
                w_in = wpool.tile([128, 6 * 2 * ELOC], BF16, name="w_in_sb", tag="w_in_sb")
                nc.sync.dma_start(w_in[:], w_in_d[li])
                w_cw = wpool.tile([128, 2 * K], FP32, name="w_cw_sb", tag="w_cw_sb")
                nc.sync.dma_start(w_cw[:], conv_w_d[li])
                w_cb = wpool.tile([128, 2], FP32, name="w_cb_sb", tag="w_cb_sb")
                nc.sync.dma_start(w_cb[:], conv_b_d[li])
                w_xp = wpool.tile([128, 2 * (R + 2 * N)], BF16, name="w_xp_sb", tag="w_xp_sb")
                nc.sync.dma_start(w_xp[:], w_xp_d[li])
                w_dt = wpool.tile([R + 1, ELOC], FP32, name="w_dt_sb", tag="w_dt_sb")
                nc.sync.dma_start(w_dt[:], w_dt_d[li])
                w_out = wpool.tile([128, 2 * DM], BF16, name="w_out_sb", tag="w_out_sb")
                nc.sync.dma_start(w_out[:], w_out_d[li])
                w_ln = wpool.tile([128, 12], FP32, name="w_ln_sb", tag="w_ln_sb")
                nc.sync.dma_start(w_ln[:], ln_d[li])
                w_D = wpool.tile([128, 2], FP32, name="w_D_sb", tag="w_D_sb")
                nc.sync.dma_start(w_D[:], ssmd_d[li])

                # ---- LN fused with in_proj ----
                xp_t = [actp.tile([128, B * LPD], BF16, name="xp_pad0", tag="xp_pad0"),
                        actp.tile([64, B * LPD], BF16, name="xp_pad1", tag="xp_pad1")]
                z_t = [actp.tile([128, T], BF16, name="z0", tag="z0"),
                       actp.tile([64, T], BF16, name="z1", tag="z1")]
                for ti in range(2):
                    nc.vector.memset(xp_t[ti][:, 0:K], 0.0)
                    nc.vector.memset(xp_t[ti][:, LPD:LPD + K], 0.0)

                def padcol(fs, fl):
                    b_ = fs // L
                    off = b_ * LPD + K + (fs - b_ * L)
                    return slice(off, off + fl)

                def consume_inproj(f, sl6):
                    fs = f * 512
                    for mt in range(3):
                        pt = pmm([128, 512])
                        for kt in range(DTILES):
                            nc.tensor.matmul(
                                pt[:], w_in[:, kt * 384 + mt * 128:kt * 384 + (mt + 1) * 128],
                                sl6[kt][:], start=(kt == 0), stop=(kt == DTILES - 1))
                        if mt == 0:
                            nc.scalar.copy(xp_t[0][:, padcol(fs, 512)], pt[:])
                        elif mt == 1:
                            nc.scalar.copy(xp_t[1][:, padcol(fs, 512)], pt[0:64, :])
                            nc.scalar.copy(z_t[0][0:64, fs:fs + 512], pt[64:128, :])
                        else:
                            nc.scalar.copy(z_t[0][64:128, fs:fs + 512], pt[0:64, :])
                            nc.scalar.copy(z_t[1][:, fs:fs + 512], pt[64:128, :])

                ln_fm([w_ln[:, 2 * j:2 * j + 1] for j in range(DTILES)],
                      [w_ln[:, 2 * j + 1:2 * j + 2] for j in range(DTILES)],
                      consume_inproj,
                      dbg_dst=dbg["hln"] if (_DEBUG and li == 0) else None)

                # ---- conv + silu ----
                xc = [actp.tile([128, T], BF16, name="xc0", tag="xc0"),
                      actp.tile([64, T], BF16, name="xc1", tag="xc1")]
                for ti, (eo, el) in enumerate(_etiles()):
                    for b_ in range(B):
                        acc = st2.tile([el, L], FP32, name="cacc", tag="cacc", bufs=2)
                        cb = b_ * LPD + K
                        nc.vector.tensor_scalar(acc[:], xp_t[ti][:el, cb - 3:cb - 3 + L],
                                                w_cw[0:el, ti * K:ti * K + 1], None, OP.mult)
                        for j in range(1, K):
                            nc.vector.scalar_tensor_tensor(
                                acc[:], xp_t[ti][:el, cb - 3 + j:cb - 3 + j + L],
                                w_cw[0:el, ti * K + j:ti * K + j + 1],
                                acc[:], OP.mult, OP.add)
                        nc.scalar.activation(xc[ti][:el, b_ * L:(b_ + 1) * L], acc[:],
                                             AF.Silu, bias=w_cb[0:el, ti:ti + 1])
                if _DEBUG and li == 0:
                    for ti, (eo, el) in enumerate(_etiles()):
                        xcd = st2.tile([el, T], FP32, name="xcd", tag="xcd")
                        nc.vector.tensor_copy(xcd[:], xc[ti][:el, :])
                        nc.sync.dma_start(dbg["xc"][eo:eo + el, :], xcd[:])

                # ---- x_proj partial + AllReduce ----
                dbl_in = dramp.tile([R + 2 * N, T], FP32, name="dbl_in", tag="dbl_in")
                dbl_out = dramp.tile([R + 2 * N, T], FP32, name="dbl_out", tag="dbl_out")
                for f in range(T // 512):
                    fs = slice(f * 512, (f + 1) * 512)
                    pt = pmm([80, 512])
                    for ti, (eo, el) in enumerate(_etiles()):
                        nc.tensor.matmul(pt[:], w_xp[0:el, ti * 80:(ti + 1) * 80],
                                         xc[ti][:el, fs], start=(ti == 0), stop=(ti == 1))
                    dblf = st2.tile([80, 512], FP32, name="dblf", tag="dblf")
                    nc.scalar.copy(dblf[:], pt[:])
                    nc.sync.dma_start(dbl_in[:, fs], dblf[:])
                nc.gpsimd.collective_compute("AllReduce", OP.add,
                                             replica_groups=[list(range(NC))],
                                             ins=[dbl_in[:]], outs=[dbl_out[:]])
                # B/C rows -> token-major
                bc_tm = actp.tile([128, NCH * 2 * N], BF16, name="bc_tm", tag="bc_tm")
                for c in range(NCH):
                    bcf = st2.tile([2 * N, Q], FP32, name="bcf", tag="bcf")
                    nc.sync.dma_start(bcf[:], dbl_out[R:R + 2 * N, c * Q:(c + 1) * Q])
                    bc6 = st2.tile([2 * N, Q], BF16, name="bc6", tag="bc6")
                    nc.vector.tensor_copy(bc6[:], bcf[:])
                    ptb = pmm([128, 32], BF16)
                    nc.tensor.transpose(ptb[:], bc6[:], ident16[:32, :32])
                    nc.scalar.copy(bc_tm[:, c * 32:(c + 1) * 32], ptb[:])

                # ---- per-chunk SSM ----
                y_fm = [actp.tile([128, T], BF16, name="yfm0", tag="yfm0"),
                        actp.tile([64, T], BF16, name="yfm1", tag="yfm1")]
                rowbuf = rowp.tile([1, 2 * N * ELOC], BF16, name="rowbuf", tag="rowbuf")
                cq_row = [rowbuf[0:1, h * HW:(h + 1) * HW] for h in range(2)]
                def dt_stage(c):
                    dtf = st2.tile([R + 1, Q], FP32, name="dtf", tag="dtf", bufs=2)
                    nc.vector.memset(dtf[0:1, :], 1.0)
                    nc.sync.dma_start(dtf[1:R + 1, :], dbl_out[0:R, c * Q:(c + 1) * Q])
                    ptd = pmm([128, ELOC])
                    nc.tensor.matmul(ptd[:], dtf[:], w_dt[:], start=True, stop=True)
                    dt_c = st2.tile([128, ELOC], FP32, name="dt_c", tag="dt_c", bufs=2)
                    nc.scalar.activation(dt_c[:], ptd[:], AF.Exp)
                    return dt_c

                dt_pend = {}
                for c in range(NCH):
                    if c % 2 == 0:
                        dt_pend[c] = dt_stage(c)
                        dt_pend[c + 1] = dt_stage(c + 1)
                        nc.scalar.activation(dt_pend[c][:], dt_pend[c][:], AF.Ln, bias=1.0)
                        nc.scalar.activation(dt_pend[c + 1][:], dt_pend[c + 1][:], AF.Ln, bias=1.0)
                    dt_c = dt_pend.pop(c)
                    if _DEBUG and li == 0:
                        nc.sync.dma_start(dbg["dt"][c * Q:(c + 1) * Q, :], dt_c[:])
                    pts = pmm([128, ELOC])
                    nc.tensor.matmul(pts[:], tric32, dt_c[:], start=True, stop=True)
                    sp_c = st2.tile([128, ELOC], FP32, name="sp_c", tag="sp_c", bufs=2)
                    nc.scalar.copy(sp_c[:], pts[:])
                    if _DEBUG and li == 0:
                        nc.sync.dma_start(dbg["sp"][c * Q:(c + 1) * Q, :], sp_c[:])
                    ptmm = pmm([1, ELOC])
                    nc.tensor.matmul(ptmm[:], halfcol32, dt_c[:], start=True, stop=True)
                    m_c = st2.tile([1, ELOC], FP32, name="m_c", tag="m_c", bufs=2)
                    nc.scalar.copy(m_c[:], ptmm[:])
                    exr = rowbuf[0:1, N * ELOC:2 * N * ELOC]
                    for n in range(N):
                        nc.scalar.activation(exr[:, n * ELOC:(n + 1) * ELOC], m_c[:],
                                             AF.Exp, scale=float(a_scales[n]))
                    xct = st2.tile([128, ELOC], BF16, name="xct", tag="xct", bufs=2)
                    nc.sync.dma_start_transpose(xct[:, 0:128], xc[0][:, c * Q:(c + 1) * Q])
                    ptx = pmm([128, 64], BF16)
                    nc.tensor.transpose(ptx[:], xc[1][:64, c * Q:(c + 1) * Q], ident16[:64, :64])
                    nc.scalar.copy(xct[:, 128:ELOC], ptx[:])
                    u_c = st2.tile([128, ELOC], BF16, name="u_c", tag="u_c", bufs=2)
                    nc.vector.tensor_tensor(u_c[:], dt_c[:], xct[:], OP.mult)
                    r_c = st2.tile([128, ELOC], BF16, name="r_c", tag="r_c", bufs=2)
                    nc.scalar.activation(r_c[:], sp_c[:], AF.Exp, scale=float(a_scales[0]))
                    ri_c = st2.tile([128, ELOC], BF16, name="ri_c", tag="ri_c", bufs=2)
                    nc.scalar.activation(ri_c[:], sp_c[:], AF.Exp, scale=float(-a_scales[0]))

                    def chain(base, tag, bufs=2):
                        t_ = volp.tile([128, N * ELOC], BF16, name="chn", tag=tag, bufs=bufs)
                        v = t_[:].rearrange("p (n e) -> p n e", n=N)
                        nc.vector.tensor_copy(v[:, 0, :], base[:])
                        nc.vector.tensor_tensor(v[:, 1, :], base[:], base[:], OP.mult)
                        for lo in (2, 4, 8):
                            nc.vector.tensor_tensor(
                                v[:, lo:2 * lo, :], v[:, 0:lo, :],
                                v[:, lo - 1:lo, :].broadcast_to([128, lo, ELOC]),
                                OP.mult)
                        return t_

                    P_c = chain(r_c, "P_c")
                    E_c = chain(ri_c, "E_c", bufs=1)

                    hcs = []
                    for h in range(2):
                        hsl = slice(h * HW, (h + 1) * HW)
                        qv = volp.tile([128, HW], BF16, name="qv", tag=f"qv{h}", bufs=2)
                        nc.vector.tensor_tensor(
                            qv[:].rearrange("p (n e) -> p n e", n=NH),
                            u_c[:].unsqueeze(1).broadcast_to([128, NH, ELOC]),
                            bc_tm[:, c * 32 + h * NH:c * 32 + (h + 1) * NH]
                            .unsqueeze(2).broadcast_to([128, NH, ELOC]),
                            OP.mult)
                        nc.vector.tensor_tensor(qv[:], qv[:], E_c[:, hsl], OP.mult)
                        if c % CPB != 0:
                            if h == 0:
                                nc.vector.tensor_tensor(rowbuf[0:1, 0:N * ELOC],
                                                        rowbuf[0:1, 0:N * ELOC],
                                                        exr[:], OP.mult)
                            nc.vector.tensor_tensor(qv[0:1, :], qv[0:1, :],
                                                    cq_row[h], OP.add)
                        tp = ps_tri.tile([128, HW], FP32, name="tp", tag="tri")
                        for fsub in range(HW // 512):
                            nc.tensor.matmul(tp[:, fsub * 512:(fsub + 1) * 512], tri16[:],
                                             qv[:, fsub * 512:(fsub + 1) * 512],
                                             start=True, stop=True)
                        hc = volp.tile([128, HW], BF16, name="hc", tag=f"qv{h}", bufs=2)
                        nc.vector.tensor_tensor(hc[:], tp[:], P_c[:, hsl], OP.mult)
                        nc.sync.dma_start(cq_row[h], hc[127:128, :])
                        nc.vector.tensor_tensor(
                            hc[:].rearrange("p (n e) -> p n e", n=NH),
                            hc[:].rearrange("p (n e) -> p n e", n=NH),
                            bc_tm[:, c * 32 + N + h * NH:c * 32 + N + (h + 1) * NH]
                            .unsqueeze(2).broadcast_to([128, NH, ELOC]),
                            OP.mult)
                        hcs.append(hc)
                    nc.vector.tensor_tensor(hcs[0][:], hcs[0][:], hcs[1][:], OP.add)
                    h3 = hcs[0][:].rearrange("p (n e) -> p n e", n=NH)
                    for lev in (4, 2):
                        nc.vector.tensor_tensor(h3[:, 0:lev, :], h3[:, 0:lev, :],
                                                h3[:, lev:2 * lev, :], OP.add)
                    y_c = st2.tile([128, ELOC], BF16, name="y_c", tag="y_c", bufs=2)
                    nc.vector.tensor_tensor(y_c[:], h3[:, 0, :], h3[:, 1, :], OP.add)
                    if _DEBUG and li == 0:
                        ydd = st2.tile([128, ELOC], FP32, name="ydd", tag="ydd")
                        nc.vector.tensor_copy(ydd[:], y_c[:])
                        nc.sync.dma_start(dbg["yssm"][c * Q:(c + 1) * Q, :], ydd[:])
                    nc.sync.dma_start_transpose(y_fm[0][:, c * Q:(c + 1) * Q], y_c[:, 0:128])
                    pty = pmm([64, 128], BF16)
                    nc.tensor.transpose(pty[:], y_c[:, 128:ELOC], ident16[:])
                    nc.scalar.copy(y_fm[1][:64, c * Q:(c + 1) * Q], pty[:])

                # ---- D-term, z-gate ----
                for ti, (eo, el) in enumerate(_etiles()):
                    nc.vector.scalar_tensor_tensor(y_fm[ti][:el, :], xc[ti][:el, :],
                                                   w_D[0:el, ti:ti + 1], y_fm[ti][:el, :],
                                                   OP.mult, OP.add)
                    nc.scalar.activation(z_t[ti][:el, :], z_t[ti][:el, :], AF.Silu)
                    nc.vector.tensor_tensor(y_fm[ti][:el, :], y_fm[ti][:el, :],
                                            z_t[ti][:el, :], OP.mult)

                # ---- out_proj partial + AllReduce + residual update ----
                op_in = dramp.tile([DM, T], BF16, name="op_in", tag="op_in")
                op_out = dramp.tile([DM, T], BF16, name="op_out", tag="op_out")
                for mt in range(DTILES):
                    for f in range(T // 512):
                        fs = slice(f * 512, (f + 1) * 512)
                        pt = pmm([128, 512])
                        for ti, (eo, el) in enumerate(_etiles()):
                            nc.tensor.matmul(
                                pt[:], w_out[0:el, ti * DM + mt * 128:ti * DM + (mt + 1) * 128],
                                y_fm[ti][:el, fs], start=(ti == 0), stop=(ti == 1))
                        opf = st2.tile([128, 512], BF16, name="opf", tag="opf")
                        nc.scalar.copy(opf[:], pt[:])
                        nc.sync.dma_start(op_in[mt * 128:(mt + 1) * 128, fs], opf[:])
                nc.gpsimd.collective_compute("AllReduce", OP.add,
                                             replica_groups=[list(range(NC))],
                                             ins=[op_in[:]], outs=[op_out[:]])
                for j in range(DTILES):
                    for f in range(T // 512):
                        fs = slice(f * 512, (f + 1) * 512)
                        hs_f = st2.tile([128, 512], BF16, name="hs_f", tag="hs_f")
                        nc.sync.dma_start(hs_f[:], op_out[j * 128:(j + 1) * 128, fs])
                        nc.vector.tensor_tensor(residual[j][:, fs], residual[j][:, fs],
                                                hs_f[:], OP.add)
                        if _DEBUG and li == 0:
                            nc.sync.dma_start(dbg["hs"][j * 128:(j + 1) * 128, fs], hs_f[:])

            # ================= final stage =================
            mixed = [(actp.tile([128, T], BF16, name=f"mx{j}", tag=t) if j < 4 else
                      volp.tile([128, T], BF16, name=f"mx{j}", tag=t, bufs=(2 if t == "P_c" else 1)))
                     for j, t in enumerate(["xp_pad0", "z0", "xc0", "yfm0", "P_c", "E_c"])]

            def consume_mixed(f, sl6):
                fs = slice(f * 512, (f + 1) * 512)
                for j in range(DTILES):
                    nc.vector.tensor_copy(mixed[j][:, fs], sl6[j][:])

            ln_fm([nrmc[:, 2 * j:2 * j + 1] for j in range(DTILES)],
                  [nrmc[:, 2 * j + 1:2 * j + 2] for j in range(DTILES)],
                  consume_mixed)

            xfm16 = [(actp.tile([128, T], BF16, name=f"xfm{j}", tag=t) if j < 4 else
                      st2.tile([128, T], BF16, name=f"xfm{j}", tag=t, bufs=1))
                     for j, t in enumerate(["xp_pad1", "z1", "xc1", "yfm1", "opf", "hs_f"])]
            for c in range(NCH):
                x_tm_c = st2.tile([128, DM], FP32, name="x_tm_c2", tag="x_tm_c")
                nc.sync.dma_start(x_tm_c[:], x_d[c * Q:(c + 1) * Q, :])
                for j in range(DTILES):
                    ptt = pmm([128, 128])
                    nc.tensor.transpose(ptt[:], x_tm_c[:, j * 128:(j + 1) * 128], ident32[:])
                    nc.scalar.copy(xfm16[j][:, c * Q:(c + 1) * Q], ptt[:])
            brow = actp.tile([1, T], BF16, name="brow", tag="stats16")
            nc.sync.dma_start(brow[:], bprob_d[:])

            wc1 = wpool.tile([128, 7 * GDM], BF16, name="wc1", tag="w_in_sb")
            nc.sync.dma_start(wc1[:], w_c1_d[:])
            bc1 = wpool.tile([GDM, 1], FP32, name="bc1", tag="w_cb_sb")
            nc.sync.dma_start(bc1[:], b_c1_d[:])
            wc2 = wpool.tile([GDM + 1, DM], BF16, name="wc2", tag="w_out_sb")
            nc.sync.dma_start(wc2[:], w_c2_d[:])

            h1 = actp.tile([GDM + 1, T], BF16, name="h1", tag="h1")
            nc.vector.memset(h1[GDM:GDM + 1, :], 1.0)
            for f in range(T // 512):
                fs = slice(f * 512, (f + 1) * 512)
                pt = pmm([GDM, 512])
                for kt in range(DTILES):
                    nc.tensor.matmul(pt[:], wc1[:, kt * GDM:(kt + 1) * GDM],
                                     xfm16[kt][:, fs], start=(kt == 0), stop=False)
                nc.tensor.matmul(pt[:], wc1[0:1, 6 * GDM:7 * GDM], brow[:, fs],
                                 start=False, stop=True)
                nc.scalar.activation(h1[0:GDM, fs], pt[:], AF.Silu, bias=bc1[:, 0:1])

            g_in = dramp.tile([T, DM], FP32, name="g_in", tag="g_in")
            g_out = dramp.tile([T, DM], FP32, name="g_out", tag="g_out")
            for c in range(NCH):
                h2sb = st2.tile([128, DM], FP32, name="h2sb", tag="h2sb")
                for fs2 in range(2):
                    pt = pmm([128, 384])
                    nc.tensor.matmul(pt[:], h1[:, c * Q:(c + 1) * Q],
                                     wc2[:, fs2 * 384:(fs2 + 1) * 384],
                                     start=True, stop=True)
                    nc.scalar.copy(h2sb[:, fs2 * 384:(fs2 + 1) * 384], pt[:])
                nc.sync.dma_start(g_in[c * Q:(c + 1) * Q, :], h2sb[:])
            nc.gpsimd.collective_compute("AllReduce", OP.add,
                                         replica_groups=[list(range(NC))],
                                         ins=[g_in[:]], outs=[g_out[:]])

            n16 = actp.tile([1, DM], BF16, name="n16", tag="n16")
            n16b = actp.tile([1, DM], BF16, name="n16b", tag="n16b")
            nc.vector.tensor_copy(n16[:], nrow[64:65, :])
            nc.vector.tensor_copy(n16b[:], nrow[96:97, :])
            nfw_rep = actp.tile([128, DM], BF16, name="nfw_rep", tag="nfw_rep")
            nfb_rep = actp.tile([128, DM], BF16, name="nfb_rep", tag="nfb_rep")
            for fs2 in range(2):
                rp = pmm([128, 384])
                nc.tensor.matmul(rp[:], onesrow16[:1, :128],
                                 n16[0:1, fs2 * 384:(fs2 + 1) * 384], start=True, stop=True)
                nc.scalar.copy(nfw_rep[:, fs2 * 384:(fs2 + 1) * 384], rp[:])
                rp2 = pmm([128, 384])
                nc.tensor.matmul(rp2[:], onesrow16[:1, :128],
                                 n16b[0:1, fs2 * 384:(fs2 + 1) * 384], start=True, stop=True)
                nc.scalar.copy(nfb_rep[:, fs2 * 384:(fs2 + 1) * 384], rp2[:])

            for c in range(NCH):
                mixed_tm = st2.tile([128, DM], BF16, name="mixed_tm", tag="mixed_tm")
                for j in range(DTILES):
                    ptt = pmm([128, 128], BF16)
                    nc.tensor.transpose(ptt[:], mixed[j][:, c * Q:(c + 1) * Q], ident16[:])
                    nc.scalar.copy(mixed_tm[:, j * 128:(j + 1) * 128], ptt[:])
                xt = st2.tile([128, DM], FP32, name="xt", tag="x_tm_c")
                nc.sync.dma_start(xt[:], x_d[c * Q:(c + 1) * Q, :])
                gt = st2.tile([128, DM], FP32, name="gt", tag="cacc", bufs=2)
                nc.sync.dma_start(gt[:], g_out[c * Q:(c + 1) * Q, :])
                nc.scalar.activation(gt[:], gt[:], AF.Sigmoid)
                nc.sync.dma_start(gate_d[c * Q:(c + 1) * Q, :], gt[:])
                ot = st2.tile([128, DM], FP32, name="ot", tag="cacc", bufs=2)
                nc.vector.tensor_tensor(ot[:], mixed_tm[:], xt[:], OP.subtract)
                nc.vector.tensor_tensor(ot[:], ot[:], gt[:], OP.mult)
                nc.vector.tensor_tensor(ot[:], ot[:], xt[:], OP.add)
                st = st2.tile([128, 1], FP32, name="st", tag="st")
                nc.vector.tensor_reduce(st[:], ot[:], axis=AX.X, op=OP.add)
                nc.scalar.activation(st[:], st[:], AF.Copy, scale=1.0 / DM)
                nc.vector.tensor_scalar(ot[:], ot[:], st[:, 0:1], None, OP.subtract)
                sq2 = st2.tile([128, DM], FP32, name="sq2", tag="h2sb")
                nc.vector.tensor_tensor(sq2[:], ot[:], ot[:], OP.mult)
                v2 = st2.tile([128, 1], FP32, name="v2", tag="v2")
                nc.vector.tensor_reduce(v2[:], sq2[:], axis=AX.X, op=OP.add)
                nc.scalar.activation(v2[:], v2[:], AF.Ln, bias=eps_ap[:], scale=1.0 / DM)
                nc.scalar.activation(v2[:], v2[:], AF.Exp, scale=-0.5)
                nc.vector.tensor_scalar(ot[:], ot[:], v2[:, 0:1], None, OP.mult)
                nc.vector.tensor_tensor(ot[:], ot[:], nfw_rep[:], OP.mult)
                nc.vector.tensor_tensor(ot[:], ot[:], nfb_rep[:], OP.add)
                nc.sync.dma_start(out_d[c * Q:(c + 1) * Q, :], ot[:])

    nc.compile()
    return nc


def _pack_fm(arr, pad_to=128):
    arr = np.asarray(arr)
    if arr.ndim == 1:
        arr = arr[:, None]
    F, W = arr.shape
    nblk = (F + pad_to - 1) // pad_to
    outp = np.zeros((pad_to, nblk * W), dtype=arr.dtype)
    for b_ in range(nblk):
        blk = arr[b_ * pad_to:(b_ + 1) * pad_to]
        outp[:blk.shape[0], b_ * W:(b_ + 1) * W] = blk
    return outp


def _prep_inputs(inputs):
    f32 = np.float32
    x = np.ascontiguousarray(np.asarray(inputs["x"], f32).reshape(T, DM))
    bprob = np.ascontiguousarray(np.asarray(inputs["boundary_prob"], f32).reshape(1, T))
    idx = np.arange(128)
    tri = (idx[:, None] <= idx[None, :]).astype(f32)          # [tau, t']
    tricf = np.concatenate([tri - 0.5, np.full((128, 1), 0.5, f32)], axis=1)
    maps = []
    for c in range(NC):
        sl = slice(c * ELOC, (c + 1) * ELOC)
        w_in = np.stack([_pack_fm(
            np.concatenate([np.asarray(inputs["in_proj_w"][i])[sl],
                            np.asarray(inputs["in_proj_w"][i])[E + c * ELOC:E + (c + 1) * ELOC]],
                           axis=0).T.astype(f32))
            for i in range(NL)])
        w_xp = np.stack([_pack_fm(np.asarray(inputs["x_proj_w"][i], f32)[:, sl].T)
                         for i in range(NL)])
        w_dt = np.stack([
            np.concatenate([np.asarray(inputs["dt_proj_b"][i], f32)[None, sl],
                            np.asarray(inputs["dt_proj_w"][i], f32)[sl].T], axis=0)
            for i in range(NL)])
        w_out = np.stack([_pack_fm(np.asarray(inputs["out_proj_w"][i], f32)[:, sl].T)
                          for i in range(NL)])
        lnp = np.stack([_pack_fm(np.stack([np.asarray(inputs["ln_w"][i], f32),
                                           np.asarray(inputs["ln_b"][i], f32)], axis=1))
                        for i in range(NL)])
        gsl = slice(c * GDM, (c + 1) * GDM)
        cw1 = np.asarray(inputs["ctrl_w1"], f32)
        w_c1 = np.concatenate([_pack_fm(cw1[gsl, :DM].T),
                               _pack_fm(cw1[gsl, DM:DM + 1].T)], axis=1)
        w_c2 = np.concatenate([np.asarray(inputs["ctrl_w2"], f32)[:, gsl].T,
                               (np.asarray(inputs["ctrl_b2"], f32) / NC)[None, :]], axis=0)
        nrm = np.stack([np.asarray(inputs["normf_w"], f32), np.asarray(inputs["normf_b"], f32),
                        np.asarray(inputs["out_ln_w"], f32), np.asarray(inputs["out_ln_b"], f32)])
        nrmc = _pack_fm(np.stack([np.asarray(inputs["normf_w"], f32),
                                  np.asarray(inputs["normf_b"], f32)], axis=1))
        maps.append({
            "x": x, "bprob": bprob, "w_in": w_in,
            "conv_w": np.stack([_pack_fm(np.asarray(inputs["conv_w"][i], f32)[sl])
                                for i in range(NL)]),
            "conv_b": np.stack([_pack_fm(np.asarray(inputs["conv_b"][i], f32)[sl])
                                for i in range(NL)]),
            "w_xp": w_xp, "w_dt": w_dt, "w_out": w_out, "lnp": lnp,
            "ssmd": np.stack([_pack_fm(np.asarray(inputs["ssm_D"][i], f32)[sl])
                              for i in range(NL)]),
            "w_c1": w_c1,
            "b_c1": np.asarray(inputs["ctrl_b1"], f32)[gsl][:, None],
            "w_c2": w_c2, "nrm": nrm, "nrmc": nrmc,
            "tri16": tri, "tricf": tricf,
        })
    return maps


def kernel(**inputs):
    import ml_dtypes
    maps = _prep_inputs(inputs)
    A = -np.exp(np.asarray(inputs["A_log"], np.float32))
    a_scales = A[0, 0, :]
    for i in range(NL):
        assert np.allclose(A[i], np.broadcast_to(a_scales, (E, N)), rtol=1e-5, atol=1e-6), \
            "kernel assumes channel-independent A"
    key = tuple(np.round(np.asarray(a_scales, np.float64), 6).tolist())
    if key not in _CACHE:
        _CACHE[key] = _build(a_scales)
    nc = _CACHE[key]
    for m in maps:
        for k in ("w_in", "w_xp", "w_out", "w_c1", "w_c2", "bprob", "tri16"):
            m[k] = np.asarray(m[k], dtype=ml_dtypes.bfloat16)
    res = run_bass_kernel_spmd(nc, maps, list(range(NC)))
    kernel._res = res
    r0 = res.results[0]
    out = np.asarray(r0["out"], np.float32).reshape(B, L, DM)
    gate = np.asarray(r0["gate"], np.float32).reshape(B, L, DM)
    return out, gate
